# revision 5
# baseline (speedup 1.0000x reference)
"""Trainium2 Bass kernel for nn_Block (pre-LN transformer block with dense
self-attention where q=k=v=LN1(x), followed by a GELU MLP).

Sharding: data-parallel over batch B=8 across the 8 NeuronCores (one batch
element per core). Weights are replicated; host-side prep transposes/casts the
MLP weights to bf16 and pre-broadcasts the small LN/bias vectors so the device
kernel does pure compute.

Device algorithm per core (x: [2048, 768] fp32):
  1. LN1 -> y (bf16, natural + transposed layouts), diag D[n] = SCALE*||y_n||^2
  2. gmax = max_n D[n] (constant softmax shift -> E := exp(SCALE*S - gmax) is
     SYMMETRIC, so the second attention matmul reads E tiles directly as lhsT
     without any transposes); S = y@y^T via 6 accumulating K=128 matmuls per
     128-row block; one Exp activation per block also emits row sums Z.
  3. a = (E @ y) * (1/Z) per row; x2 = x + a
  4. LN2 -> zT (transposed via DMA-transpose)
  5. hT = gelu(fc1 @ z^T + b1) computed transposed; out = x2 + fc2 @ h^T + b2
"""

import os
import sys
from contextlib import ExitStack

for _p in ("/opt/trn_rl_repo",):
    if _p not in sys.path:
        sys.path.append(_p)

import numpy as np
import ml_dtypes

import concourse.bass as bass
import concourse.bacc as bacc
import concourse.tile as tile
import concourse.mybir as mybir
import concourse.bass_isa as bass_isa
from concourse.bass_utils import run_bass_kernel_spmd

f32 = mybir.dt.float32
bf16 = mybir.dt.bfloat16
AF = mybir.ActivationFunctionType
ALU = mybir.AluOpType
AX = mybir.AxisListType

B, N, C, H = 8, 2048, 768, 3072
P = 128
NB = N // P        # 16 row blocks of 128
CCK = C // P       # 6 channel chunks of 128
JB = H // P        # 24 hidden blocks of 128
NQ = 4             # MLP sequence chunks
QW = N // NQ       # 512 columns per MLP chunk
HEADS = 12
SCALE = 1.0 / float(np.sqrt(C // HEADS))   # 0.125
EPS = 1e-5

_cached_nc = None


def _emit(nc, tc, hs):
    ctx = ExitStack()
    with ctx:
        small = ctx.enter_context(tc.tile_pool(name="small", bufs=1))
        stats = ctx.enter_context(tc.tile_pool(name="stats", bufs=4))

        # Pre-broadcast params straight into SBUF.
        ln1w_t = small.tile([P, C], f32, tag="ln1w")
        ln1b_t = small.tile([P, C], f32, tag="ln1b")
        ln2w_t = small.tile([P, C], f32, tag="ln2w")
        ln2b_t = small.tile([P, C], f32, tag="ln2b")
        fc2b_t = small.tile([P, C], f32, tag="fc2b")
        fc1b_t = small.tile([P, JB], f32, tag="fc1b")
        nc.sync.dma_start(ln1w_t[:], hs["ln1w_b"].ap())
        nc.sync.dma_start(ln1b_t[:], hs["ln1b_b"].ap())
        nc.sync.dma_start(ln2w_t[:], hs["ln2w_b"].ap())
        nc.sync.dma_start(ln2b_t[:], hs["ln2b_b"].ap())
        nc.sync.dma_start(fc2b_t[:], hs["fc2b_b"].ap())
        nc.sync.dma_start(fc1b_t[:], hs["fc1b_r"].ap())

        eps_t = small.tile([P, 1], f32, tag="eps")
        nc.vector.memset(eps_t[:], EPS)

        D_t = small.tile([P, NB], f32, tag="D")      # diag(S) per block col
        Z_t = small.tile([P, NB], f32, tag="Z")      # softmax row sums
        rZ_t = small.tile([P, NB], f32, tag="rZ")
        negmax = small.tile([P, 1], f32, tag="negmax")

        x_ap = hs["x"].ap()
        out_ap = hs["out"].ap()

        # Big persistent tensors. left: y, E; right: yT then x2/zT.
        y_pool = tc.alloc_tile_pool(name="ybig", bufs=1)
        y_sb = y_pool.tile([P, NB * C], bf16, tag="y")
        yT_pool = tc.alloc_tile_pool(name="yTbig", bufs=1, side="right")
        yT_sb = yT_pool.tile([P, CCK * N], bf16, tag="yT")

        # ---- Stage 1: LN1 -> y, yT, D ----
        with tc.tile_pool(name="xin1", bufs=3) as xin_pool, \
             tc.tile_pool(name="lnscr1", bufs=2) as lnscr:
            for i in range(NB):
                xt = xin_pool.tile([P, C], f32, tag="xin")
                nc.sync.dma_start(xt[:], x_ap[i * P:(i + 1) * P, :])

                s1 = stats.tile([P, 1], f32, tag="s1")
                s2 = stats.tile([P, 1], f32, tag="s2")
                xsq = lnscr.tile([P, C], f32, tag="xsq")
                nc.vector.tensor_reduce(s1[:], xt[:], AX.X, ALU.add)
                nc.vector.scalar_tensor_tensor(
                    xsq[:], xt[:], 1.0, xt[:], ALU.mult, ALU.mult, accum_out=s2[:])
                mean = stats.tile([P, 1], f32, tag="mean")
                var = stats.tile([P, 1], f32, tag="var")
                nc.vector.tensor_scalar(mean[:], s1[:], 1.0 / C, None, ALU.mult)
                # var + eps = s2/C - mean^2 + eps
                m2t = stats.tile([P, 1], f32, tag="m2t")
                nc.vector.tensor_scalar(m2t[:], mean[:], mean[:, 0:1], -1.0,
                                        ALU.mult, ALU.mult)  # -mean^2
                nc.vector.tensor_scalar(var[:], s2[:], 1.0 / C, m2t[:, 0:1],
                                        ALU.mult, ALU.add)
                std = stats.tile([P, 1], f32, tag="std")
                nc.scalar.activation(std[:], var[:], AF.Sqrt, bias=eps_t[:, 0:1])
                rstd = stats.tile([P, 1], f32, tag="rstd")
                nc.vector.reciprocal(rstd[:], std[:])
                negmr = stats.tile([P, 1], f32, tag="negmr")  # -mean*rstd
                nc.vector.tensor_scalar(negmr[:], mean[:], rstd[:, 0:1], -1.0,
                                        ALU.mult, ALU.mult)

                u = lnscr.tile([P, C], f32, tag="u")
                nc.vector.tensor_scalar(u[:], xt[:], rstd[:, 0:1], negmr[:, 0:1],
                                        ALU.mult, ALU.add)
                v = lnscr.tile([P, C], f32, tag="v")
                nc.vector.scalar_tensor_tensor(
                    v[:], u[:], 1.0, ln1w_t[:], ALU.mult, ALU.mult)
                ysl = y_sb[:, i * C:(i + 1) * C]
                nc.vector.scalar_tensor_tensor(
                    ysl, v[:], 1.0, ln1b_t[:], ALU.mult, ALU.add)

                # D[:, i] = SCALE * sum(y_bf16^2) -- matches the matmul diag.
                ysq = lnscr.tile([P, C], bf16, tag="ysq")
                nc.scalar.activation(ysq[:], ysl, AF.Square,
                                     accum_out=D_t[:, i:i + 1])

                for c in range(CCK):
                    nc.sync.dma_start_transpose(
                        yT_sb[:, c * N + i * P: c * N + (i + 1) * P],
                        y_sb[:, i * C + c * P: i * C + (c + 1) * P])

        # gmax across all rows; negmax = -SCALE * gmax
        dmax = stats.tile([P, 1], f32, tag="dmax")
        nc.vector.tensor_reduce(dmax[:], D_t[:, 0:NB], AX.X, ALU.max)
        gall = stats.tile([P, 1], f32, tag="gall")
        nc.gpsimd.partition_all_reduce(gall[:], dmax[:], channels=P,
                                       reduce_op=bass_isa.ReduceOp.max)
        nc.vector.tensor_scalar(negmax[:], gall[:], -SCALE, None, ALU.mult)

        # ---- Stage 2: S = y y^T (blockwise); E = exp(SCALE*S - SCALE*gmax) ----
        E_pool = tc.alloc_tile_pool(name="Ebig", bufs=1)
        E_sb = E_pool.tile([P, NB * N], bf16, tag="E")
        with tc.tile_pool(name="spsum", bufs=2, space="PSUM") as sp_pool:
            for i in range(NB):
                s_ps = sp_pool.tile([P, N], f32, tag="s")
                for c in range(CCK):
                    lhsT = yT_sb[:, c * N + i * P: c * N + (i + 1) * P]
                    for m in range(N // 512):
                        nc.tensor.matmul(
                            s_ps[:, m * 512:(m + 1) * 512], lhsT,
                            yT_sb[:, c * N + m * 512: c * N + (m + 1) * 512],
                            start=(c == 0), stop=(c == CCK - 1))
                nc.scalar.activation(
                    E_sb[:, i * N:(i + 1) * N], s_ps[:], AF.Exp,
                    bias=negmax[:, 0:1], scale=SCALE,
                    accum_out=Z_t[:, i:i + 1])

        nc.vector.reciprocal(rZ_t[:, 0:NB], Z_t[:, 0:NB])

        # ---- Stage 4: a = E @ y; x2 = x + a/Z (into x2_sb, right side) ----
        yT_pool.release()
        x2_pool = tc.alloc_tile_pool(name="x2big", bufs=1, side="right")
        x2_sb = x2_pool.tile([P, NB * C], f32, tag="x2")
        with tc.tile_pool(name="apsum", bufs=2, space="PSUM") as a_pool, \
             tc.tile_pool(name="xin2", bufs=3) as xin2_pool:
            for i in range(NB):
                a_ps = a_pool.tile([P, 1024], f32, tag="a")
                for j in range(NB):
                    lhsT = E_sb[:, j * N + i * P: j * N + (i + 1) * P]
                    nc.tensor.matmul(a_ps[:, 0:512], lhsT,
                                     y_sb[:, j * C: j * C + 512],
                                     start=(j == 0), stop=(j == NB - 1))
                    nc.tensor.matmul(a_ps[:, 512:768], lhsT,
                                     y_sb[:, j * C + 512: j * C + C],
                                     start=(j == 0), stop=(j == NB - 1))
                xt = xin2_pool.tile([P, C], f32, tag="xin2")
                nc.sync.dma_start(xt[:], x_ap[i * P:(i + 1) * P, :])
                nc.vector.scalar_tensor_tensor(
                    x2_sb[:, i * C:(i + 1) * C], a_ps[:, 0:C],
                    rZ_t[:, i:i + 1], xt[:], ALU.mult, ALU.add)

        # ---- Stage 5: LN2 -> zT (right side) ----
        E_pool.release()
        y_pool.release()
        w_pool = ctx.enter_context(tc.tile_pool(name="wbig", bufs=1))
        fc1T_sb = w_pool.tile([P, CCK * H], bf16, tag="fc1T")
        fc2T_sb = w_pool.tile([P, JB * C], bf16, tag="fc2T")
        for c in range(CCK):
            nc.sync.dma_start(fc1T_sb[:, c * H:(c + 1) * H],
                              hs["fc1t"].ap()[c * P:(c + 1) * P, :])
        for j in range(JB):
            nc.sync.dma_start(fc2T_sb[:, j * C:(j + 1) * C],
                              hs["fc2t"].ap()[j * P:(j + 1) * P, :])

        zT_pool = tc.alloc_tile_pool(name="zTbig", bufs=1, side="right")
        zT_sb = zT_pool.tile([P, CCK * N], bf16, tag="zT")
        with tc.tile_pool(name="lnscr2", bufs=2) as lnscr:
            for i in range(NB):
                xt = x2_sb[:, i * C:(i + 1) * C]
                s1 = stats.tile([P, 1], f32, tag="s1")
                s2 = stats.tile([P, 1], f32, tag="s2")
                xsq = lnscr.tile([P, C], f32, tag="xsq2")
                nc.vector.tensor_reduce(s1[:], xt, AX.X, ALU.add)
                nc.vector.scalar_tensor_tensor(
                    xsq[:], xt, 1.0, xt, ALU.mult, ALU.mult, accum_out=s2[:])
                mean = stats.tile([P, 1], f32, tag="mean")
                var = stats.tile([P, 1], f32, tag="var")
                nc.vector.tensor_scalar(mean[:], s1[:], 1.0 / C, None, ALU.mult)
                m2t = stats.tile([P, 1], f32, tag="m2t")
                nc.vector.tensor_scalar(m2t[:], mean[:], mean[:, 0:1], -1.0,
                                        ALU.mult, ALU.mult)
                nc.vector.tensor_scalar(var[:], s2[:], 1.0 / C, m2t[:, 0:1],
                                        ALU.mult, ALU.add)
                std = stats.tile([P, 1], f32, tag="std")
                nc.scalar.activation(std[:], var[:], AF.Sqrt, bias=eps_t[:, 0:1])
                rstd = stats.tile([P, 1], f32, tag="rstd")
                nc.vector.reciprocal(rstd[:], std[:])
                negmr = stats.tile([P, 1], f32, tag="negmr")
                nc.vector.tensor_scalar(negmr[:], mean[:], rstd[:, 0:1], -1.0,
                                        ALU.mult, ALU.mult)
                u = lnscr.tile([P, C], f32, tag="u2")
                nc.vector.tensor_scalar(u[:], xt, rstd[:, 0:1], negmr[:, 0:1],
                                        ALU.mult, ALU.add)
                v = lnscr.tile([P, C], f32, tag="v2")
                nc.vector.scalar_tensor_tensor(
                    v[:], u[:], 1.0, ln2w_t[:], ALU.mult, ALU.mult)
                znat = lnscr.tile([P, C], bf16, tag="znat")
                nc.vector.scalar_tensor_tensor(
                    znat[:], v[:], 1.0, ln2b_t[:], ALU.mult, ALU.add)
                for c in range(CCK):
                    nc.sync.dma_start_transpose(
                        zT_sb[:, c * N + i * P: c * N + (i + 1) * P],
                        znat[:, c * P:(c + 1) * P])

        # ---- Stage 6: MLP (hT computed transposed, per 512-col chunk) ----
        hT_pool = ctx.enter_context(tc.tile_pool(name="hTbig", bufs=1))
        hT_sb = hT_pool.tile([P, JB * QW], bf16, tag="hT")
        with tc.tile_pool(name="hpsum", bufs=3, space="PSUM") as h_pool, \
             tc.tile_pool(name="opsum", bufs=2, space="PSUM") as o_pool, \
             tc.tile_pool(name="oscr", bufs=3) as oscr:
            for q in range(NQ):
                for j in range(JB):
                    h_ps = h_pool.tile([P, QW], f32, tag="h")
                    for c in range(CCK):
                        nc.tensor.matmul(
                            h_ps[:],
                            fc1T_sb[:, c * H + j * P: c * H + (j + 1) * P],
                            zT_sb[:, c * N + q * QW: c * N + (q + 1) * QW],
                            start=(c == 0), stop=(c == CCK - 1))
                    nc.scalar.activation(hT_sb[:, j * QW:(j + 1) * QW], h_ps[:],
                                         AF.Gelu, bias=fc1b_t[:, j:j + 1])
                for t in range(QW // P):
                    i = q * (QW // P) + t
                    o_ps = o_pool.tile([P, 1024], f32, tag="o")
                    for j in range(JB):
                        lhsT = hT_sb[:, j * QW + t * P: j * QW + (t + 1) * P]
                        nc.tensor.matmul(o_ps[:, 0:512], lhsT,
                                         fc2T_sb[:, j * C: j * C + 512],
                                         start=(j == 0), stop=(j == JB - 1))
                        nc.tensor.matmul(o_ps[:, 512:768], lhsT,
                                         fc2T_sb[:, j * C + 512: j * C + C],
                                         start=(j == 0), stop=(j == JB - 1))
                    o1 = oscr.tile([P, C], f32, tag="o1")
                    nc.vector.scalar_tensor_tensor(
                        o1[:], o_ps[:, 0:C], 1.0, fc2b_t[:], ALU.mult, ALU.add)
                    o2 = oscr.tile([P, C], f32, tag="o2")
                    nc.vector.scalar_tensor_tensor(
                        o2[:], o1[:], 1.0, x2_sb[:, i * C:(i + 1) * C],
                        ALU.mult, ALU.add)
                    nc.sync.dma_start(out_ap[i * P:(i + 1) * P, :], o2[:])

        zT_pool.release()
        x2_pool.release()


def _build():
    nc = bacc.Bacc("TRN2", target_bir_lowering=False, debug=False, num_devices=8)
    hs = {}
    hs["x"] = nc.declare_dram_parameter("x", [N, C], f32, isOutput=False)
    hs["ln1w_b"] = nc.declare_dram_parameter("ln1w_b", [P, C], f32, isOutput=False)
    hs["ln1b_b"] = nc.declare_dram_parameter("ln1b_b", [P, C], f32, isOutput=False)
    hs["ln2w_b"] = nc.declare_dram_parameter("ln2w_b", [P, C], f32, isOutput=False)
    hs["ln2b_b"] = nc.declare_dram_parameter("ln2b_b", [P, C], f32, isOutput=False)
    hs["fc1t"] = nc.declare_dram_parameter("fc1t", [C, H], bf16, isOutput=False)
    hs["fc2t"] = nc.declare_dram_parameter("fc2t", [H, C], bf16, isOutput=False)
    hs["fc1b_r"] = nc.declare_dram_parameter("fc1b_r", [P, JB], f32, isOutput=False)
    hs["fc2b_b"] = nc.declare_dram_parameter("fc2b_b", [P, C], f32, isOutput=False)
    hs["out"] = nc.declare_dram_parameter("out", [N, C], f32, isOutput=True)
    with tile.TileContext(nc) as tc:
        _emit(nc, tc, hs)
    nc.compile()
    return nc


def _maybe_install_ntff_hook():
    """Optional: lets BASS_TRACE=1 capture NTFF profiles under axon."""
    try:
        import types
        if "antenv.axon_hooks" in sys.modules:
            return
        import antenv
        mod = types.ModuleType("antenv.axon_hooks")
        _hook = [None]
        mod.set_axon_ntff_profile_hook = lambda h: _hook.__setitem__(0, h)
        mod.get_axon_ntff_profile_hook = lambda: _hook[0]
        sys.modules["antenv.axon_hooks"] = mod
        antenv.axon_hooks = mod
        from trn_agent_boot.trn_boot import _ntff_profile_via_ctypes
        mod.set_axon_ntff_profile_hook(
            _ntff_profile_via_ctypes("/opt/axon/libaxon_pjrt.so"))
    except Exception:
        pass


_last_results = None


def kernel(x, ln1_w, ln1_b, ln2_w, ln2_b, fc1_w, fc1_b, fc2_w, fc2_b):
    global _cached_nc, _last_results
    if _cached_nc is None:
        _cached_nc = _build()
    nc = _cached_nc

    bfl = ml_dtypes.bfloat16
    x = np.asarray(x, dtype=np.float32)
    prep = {
        "ln1w_b": np.ascontiguousarray(
            np.broadcast_to(np.asarray(ln1_w, np.float32), (P, C))),
        "ln1b_b": np.ascontiguousarray(
            np.broadcast_to(np.asarray(ln1_b, np.float32), (P, C))),
        "ln2w_b": np.ascontiguousarray(
            np.broadcast_to(np.asarray(ln2_w, np.float32), (P, C))),
        "ln2b_b": np.ascontiguousarray(
            np.broadcast_to(np.asarray(ln2_b, np.float32), (P, C))),
        "fc1t": np.ascontiguousarray(np.asarray(fc1_w, np.float32).T.astype(bfl)),
        "fc2t": np.ascontiguousarray(np.asarray(fc2_w, np.float32).T.astype(bfl)),
        "fc1b_r": np.ascontiguousarray(
            np.asarray(fc1_b, np.float32).reshape(JB, P).T),
        "fc2b_b": np.ascontiguousarray(
            np.broadcast_to(np.asarray(fc2_b, np.float32), (P, C))),
    }
    in_maps = [dict(prep, x=np.ascontiguousarray(x[b])) for b in range(B)]

    trace = bool(os.environ.get("BASS_TRACE"))
    if trace:
        _maybe_install_ntff_hook()
    res = run_bass_kernel_spmd(nc, in_maps, list(range(B)), trace=trace)
    _last_results = res
    return np.stack([res.results[b]["out"] for b in range(B)], axis=0)


# revision 8
# speedup vs baseline: 1.3767x; 1.3767x over previous
"""Trainium2 Bass kernel for nn_Block (pre-LN transformer block with dense
self-attention where q=k=v=LN1(x), followed by a GELU MLP).

Sharding: data-parallel over batch B=8 across the 8 NeuronCores (one batch
element per core). Weights are replicated; host-side prep transposes/casts the
MLP weights to bf16 and pre-broadcasts the small LN/bias vectors so the device
kernel does pure compute.

Device algorithm per core (x: [2048, 768] fp32):
  1. LN1 -> y (bf16 natural + transposed via TensorE transpose + ACT copy)
  2. E := exp(SCALE*S - c) with a host-computed constant shift
     c = SCALE*(sqrt(C)*max|w1| + ||b1||_2)^2 >= max S (Cauchy-Schwarz on the
     LN-normalized rows), so E is SYMMETRIC and the second attention matmul
     reads E tiles directly as lhsT without transposing the score matrix.
     S = y@y^T via accumulating K=128 matmuls, 512-wide PSUM quarters; the Exp
     activation per quarter also emits partial row sums (accum_out).
  3. a = (E @ y) * (1/Z); x2 = x + a, streamed to an HBM scratch; LN2 fused
     per block -> zT (TensorE transpose)
  4. hT = gelu(fc1 @ z^T + b1) computed transposed (double-buffered per
     512-column chunk); out = x2 + fc2 @ h^T + b2
"""

import os
import sys
from contextlib import ExitStack

for _p in ("/opt/trn_rl_repo",):
    if _p not in sys.path:
        sys.path.append(_p)

import numpy as np
import ml_dtypes

import concourse.bass as bass
import concourse.bacc as bacc
import concourse.tile as tile
import concourse.mybir as mybir
from concourse.bass_utils import run_bass_kernel_spmd

f32 = mybir.dt.float32
bf16 = mybir.dt.bfloat16
AF = mybir.ActivationFunctionType
ALU = mybir.AluOpType
AX = mybir.AxisListType

B, N, C, H = 8, 2048, 768, 3072
P = 128
NB = N // P        # 16 row blocks of 128
CCK = C // P       # 6 channel chunks of 128
JB = H // P        # 24 hidden blocks of 128
NQ = 4             # MLP sequence chunks
QW = N // NQ       # 512 columns per MLP chunk
SQ = 4             # S-phase quarters per row block
SW = N // SQ       # 512
HEADS = 12
SCALE = 1.0 / float(np.sqrt(C // HEADS))   # 0.125
EPS = 1e-5

_cached_nc = None


def _layernorm_ops(nc, stats, lnscr, xt_ap, w_t, b_t, out32, eps_t):
    """Emit LN stats + normalize: out32 = (xt-mean)*rstd*w + b (fp32)."""
    s1 = stats.tile([P, 1], f32, tag="s1")
    s2 = stats.tile([P, 1], f32, tag="s2")
    xsq = lnscr.tile([P, C], f32, tag="xsq")
    nc.vector.tensor_reduce(s1[:], xt_ap, AX.X, ALU.add)
    nc.vector.scalar_tensor_tensor(
        xsq[:], xt_ap, 1.0, xt_ap, ALU.mult, ALU.mult, accum_out=s2[:])
    mean = stats.tile([P, 1], f32, tag="mean")
    var = stats.tile([P, 1], f32, tag="var")
    nc.vector.tensor_scalar(mean[:], s1[:], 1.0 / C, None, ALU.mult)
    m2t = stats.tile([P, 1], f32, tag="m2t")
    nc.vector.tensor_scalar(m2t[:], mean[:], mean[:, 0:1], -1.0,
                            ALU.mult, ALU.mult)          # -mean^2
    nc.vector.tensor_scalar(var[:], s2[:], 1.0 / C, m2t[:, 0:1],
                            ALU.mult, ALU.add)           # E[x^2]-mean^2
    std = stats.tile([P, 1], f32, tag="std")
    nc.scalar.activation(std[:], var[:], AF.Sqrt, bias=eps_t[:, 0:1])
    rstd = stats.tile([P, 1], f32, tag="rstd")
    nc.vector.reciprocal(rstd[:], std[:])
    negmr = stats.tile([P, 1], f32, tag="negmr")         # -mean*rstd
    nc.vector.tensor_scalar(negmr[:], mean[:], rstd[:, 0:1], -1.0,
                            ALU.mult, ALU.mult)
    u = lnscr.tile([P, C], f32, tag="u")
    nc.vector.tensor_scalar(u[:], xt_ap, rstd[:, 0:1], negmr[:, 0:1],
                            ALU.mult, ALU.add)
    v = lnscr.tile([P, C], f32, tag="v")
    nc.vector.scalar_tensor_tensor(v[:], u[:], 1.0, w_t[:], ALU.mult, ALU.mult)
    nc.vector.scalar_tensor_tensor(out32, v[:], 1.0, b_t[:], ALU.mult, ALU.add)


def _emit(nc, tc, hs):
    ctx = ExitStack()
    with ctx:
        small = ctx.enter_context(tc.tile_pool(name="small", bufs=1))
        stats = ctx.enter_context(tc.tile_pool(name="stats", bufs=4))
        lnscr = ctx.enter_context(tc.tile_pool(name="lnscr", bufs=2))
        xio = ctx.enter_context(tc.tile_pool(name="xio", bufs=3))

        ln1w_t = small.tile([P, C], f32, tag="ln1w")
        ln1b_t = small.tile([P, C], f32, tag="ln1b")
        ln2w_t = small.tile([P, C], f32, tag="ln2w")
        ln2b_t = small.tile([P, C], f32, tag="ln2b")
        fc2b_t = small.tile([P, C], f32, tag="fc2b")
        fc1b_t = small.tile([P, JB], f32, tag="fc1b")
        expb_t = small.tile([P, 1], f32, tag="expb")
        ident = small.tile([P, P], f32, tag="ident")
        nc.sync.dma_start(ln1w_t[:], hs["ln1w_b"].ap())
        nc.sync.dma_start(ln1b_t[:], hs["ln1b_b"].ap())
        nc.sync.dma_start(ln2w_t[:], hs["ln2w_b"].ap())
        nc.sync.dma_start(ln2b_t[:], hs["ln2b_b"].ap())
        nc.sync.dma_start(fc2b_t[:], hs["fc2b_b"].ap())
        nc.sync.dma_start(fc1b_t[:], hs["fc1b_r"].ap())
        nc.sync.dma_start(expb_t[:], hs["expb"].ap())
        nc.sync.dma_start(ident[:], hs["ident"].ap())

        eps_t = small.tile([P, 1], f32, tag="eps")
        nc.vector.memset(eps_t[:], EPS)

        Z4_t = small.tile([P, NB * SQ], f32, tag="Z4")   # partial row sums
        Z_t = small.tile([P, NB], f32, tag="Z")
        rZ_t = small.tile([P, NB], f32, tag="rZ")

        x_ap = hs["x"].ap()
        out_ap = hs["out"].ap()
        x2s = nc.dram_tensor("x2scratch", [N, C], f32)
        x2s_ap = x2s.ap()

        y_pool = tc.alloc_tile_pool(name="ybig", bufs=1)
        y_sb = y_pool.tile([P, NB * C], bf16, tag="y")
        yT_pool = tc.alloc_tile_pool(name="yTbig", bufs=1, side="right")
        yT_sb = yT_pool.tile([P, CCK * N], bf16, tag="yT")

        # Persistent PSUM pool for TensorE transposes (right side).
        tp_pool = tc.alloc_tile_pool(name="tpsum", bufs=2, space="PSUM",
                                     side="right")

        # ---- Stage 1: LN1 -> y (bf16) + yT (PE transpose) ----
        for i in range(NB):
            xt = xio.tile([P, C], f32, tag="xio")
            nc.sync.dma_start(xt[:], x_ap[i * P:(i + 1) * P, :])
            y32 = lnscr.tile([P, C], f32, tag="w32")
            _layernorm_ops(nc, stats, lnscr, xt[:], ln1w_t, ln1b_t, y32[:],
                           eps_t)
            nc.vector.tensor_scalar(y_sb[:, i * C:(i + 1) * C], y32[:], 1.0,
                                    None, ALU.mult)
            for c in range(CCK):
                tp = tp_pool.tile([P, P], f32, tag="tp")
                nc.tensor.transpose(tp[:], y32[:, c * P:(c + 1) * P], ident[:])
                nc.scalar.copy(
                    yT_sb[:, c * N + i * P: c * N + (i + 1) * P], tp[:])

        # ---- Stage 2: S quarters + Exp -> E (bf16), partial Z ----
        E_pool = tc.alloc_tile_pool(name="Ebig", bufs=1)
        E_sb = E_pool.tile([P, NB * N], bf16, tag="E")
        with tc.tile_pool(name="spsum", bufs=4, space="PSUM") as sp_pool:
            for i in range(NB):
                for q in range(SQ):
                    s_ps = sp_pool.tile([P, SW], f32, tag="s")
                    for c in range(CCK):
                        nc.tensor.matmul(
                            s_ps[:],
                            yT_sb[:, c * N + i * P: c * N + (i + 1) * P],
                            yT_sb[:, c * N + q * SW: c * N + (q + 1) * SW],
                            start=(c == 0), stop=(c == CCK - 1))
                    nc.scalar.activation(
                        E_sb[:, i * N + q * SW: i * N + (q + 1) * SW],
                        s_ps[:], AF.Exp, bias=expb_t[:, 0:1], scale=SCALE,
                        accum_out=Z4_t[:, i * SQ + q: i * SQ + q + 1])

        zq = Z4_t[:, 0:NB * SQ].rearrange("p (i q) -> p i q", q=SQ)
        z01 = stats.tile([P, NB], f32, tag="z01")
        z23 = stats.tile([P, NB], f32, tag="z23")
        nc.vector.scalar_tensor_tensor(
            z01[:].rearrange("p (i o) -> p i o", o=1), zq[:, :, 0:1], 1.0,
            zq[:, :, 1:2], ALU.mult, ALU.add)
        nc.vector.scalar_tensor_tensor(
            z23[:].rearrange("p (i o) -> p i o", o=1), zq[:, :, 2:3], 1.0,
            zq[:, :, 3:4], ALU.mult, ALU.add)
        nc.vector.scalar_tensor_tensor(Z_t[:], z01[:], 1.0, z23[:],
                                       ALU.mult, ALU.add)
        nc.vector.reciprocal(rZ_t[:, 0:NB], Z_t[:, 0:NB])

        # ---- Stage 3 (fused): a = E@y; x2 = x + a/Z -> HBM; LN2 -> zT ----
        yT_pool.release()
        zT_pool = tc.alloc_tile_pool(name="zTbig", bufs=1, side="right")
        zT_sb = zT_pool.tile([P, CCK * N], bf16, tag="zT")
        with tc.tile_pool(name="apsum", bufs=2, space="PSUM") as a_pool:
            for i in range(NB):
                a_ps = a_pool.tile([P, 1024], f32, tag="a")
                for j in range(NB):
                    lhsT = E_sb[:, j * N + i * P: j * N + (i + 1) * P]
                    nc.tensor.matmul(a_ps[:, 0:512], lhsT,
                                     y_sb[:, j * C: j * C + 512],
                                     start=(j == 0), stop=(j == NB - 1))
                    nc.tensor.matmul(a_ps[:, 512:768], lhsT,
                                     y_sb[:, j * C + 512: j * C + C],
                                     start=(j == 0), stop=(j == NB - 1))
                xt = xio.tile([P, C], f32, tag="xio")
                nc.sync.dma_start(xt[:], x_ap[i * P:(i + 1) * P, :])
                x2t = lnscr.tile([P, C], f32, tag="x2t")
                nc.vector.scalar_tensor_tensor(
                    x2t[:], a_ps[:, 0:C], rZ_t[:, i:i + 1], xt[:],
                    ALU.mult, ALU.add)
                nc.sync.dma_start(x2s_ap[i * P:(i + 1) * P, :], x2t[:])
                z32 = lnscr.tile([P, C], f32, tag="w32")
                _layernorm_ops(nc, stats, lnscr, x2t[:], ln2w_t, ln2b_t,
                               z32[:], eps_t)
                for c in range(CCK):
                    tp = tp_pool.tile([P, P], f32, tag="tp")
                    nc.tensor.transpose(tp[:], z32[:, c * P:(c + 1) * P],
                                        ident[:])
                    nc.scalar.copy(
                        zT_sb[:, c * N + i * P: c * N + (i + 1) * P], tp[:])

        # ---- Stage 4: MLP ----
        E_pool.release()
        y_pool.release()
        tp_pool.release()
        w_pool = tc.alloc_tile_pool(name="wbig", bufs=1)
        fc1T_sb = w_pool.tile([P, CCK * H], bf16, tag="fc1T")
        fc2T_sb = w_pool.tile([P, JB * C], bf16, tag="fc2T")
        for c in range(CCK):
            nc.sync.dma_start(fc1T_sb[:, c * H:(c + 1) * H],
                              hs["fc1t"].ap()[c * P:(c + 1) * P, :])
        for j in range(JB):
            nc.sync.dma_start(fc2T_sb[:, j * C:(j + 1) * C],
                              hs["fc2t"].ap()[j * P:(j + 1) * P, :])

        hT_pool = tc.alloc_tile_pool(name="hTbig", bufs=2)
        with tc.tile_pool(name="hpsum", bufs=3, space="PSUM") as h_pool, \
             tc.tile_pool(name="opsum", bufs=2, space="PSUM") as o_pool:
            for q in range(NQ):
                hT_sb = hT_pool.tile([P, JB * QW], bf16, tag="hT")
                for j in range(JB):
                    h_ps = h_pool.tile([P, QW], f32, tag="h")
                    for c in range(CCK):
                        nc.tensor.matmul(
                            h_ps[:],
                            fc1T_sb[:, c * H + j * P: c * H + (j + 1) * P],
                            zT_sb[:, c * N + q * QW: c * N + (q + 1) * QW],
                            start=(c == 0), stop=(c == CCK - 1))
                    nc.scalar.activation(hT_sb[:, j * QW:(j + 1) * QW],
                                         h_ps[:], AF.Gelu,
                                         bias=fc1b_t[:, j:j + 1])
                for t in range(QW // P):
                    i = q * (QW // P) + t
                    o_ps = o_pool.tile([P, 1024], f32, tag="o")
                    for j in range(JB):
                        lhsT = hT_sb[:, j * QW + t * P: j * QW + (t + 1) * P]
                        nc.tensor.matmul(o_ps[:, 0:512], lhsT,
                                         fc2T_sb[:, j * C: j * C + 512],
                                         start=(j == 0), stop=(j == JB - 1))
                        nc.tensor.matmul(o_ps[:, 512:768], lhsT,
                                         fc2T_sb[:, j * C + 512: j * C + C],
                                         start=(j == 0), stop=(j == JB - 1))
                    xre = xio.tile([P, C], f32, tag="xio")
                    nc.sync.dma_start(xre[:], x2s_ap[i * P:(i + 1) * P, :])
                    o1 = lnscr.tile([P, C], f32, tag="xsq")
                    nc.vector.scalar_tensor_tensor(
                        o1[:], o_ps[:, 0:C], 1.0, fc2b_t[:], ALU.mult, ALU.add)
                    o2 = lnscr.tile([P, C], f32, tag="u")
                    nc.vector.scalar_tensor_tensor(
                        o2[:], o1[:], 1.0, xre[:], ALU.mult, ALU.add)
                    nc.sync.dma_start(out_ap[i * P:(i + 1) * P, :], o2[:])

        hT_pool.release()
        w_pool.release()
        zT_pool.release()


def _build():
    nc = bacc.Bacc("TRN2", target_bir_lowering=False, debug=False, num_devices=8)
    hs = {}
    hs["x"] = nc.declare_dram_parameter("x", [N, C], f32, isOutput=False)
    hs["ln1w_b"] = nc.declare_dram_parameter("ln1w_b", [P, C], f32, isOutput=False)
    hs["ln1b_b"] = nc.declare_dram_parameter("ln1b_b", [P, C], f32, isOutput=False)
    hs["ln2w_b"] = nc.declare_dram_parameter("ln2w_b", [P, C], f32, isOutput=False)
    hs["ln2b_b"] = nc.declare_dram_parameter("ln2b_b", [P, C], f32, isOutput=False)
    hs["fc1t"] = nc.declare_dram_parameter("fc1t", [C, H], bf16, isOutput=False)
    hs["fc2t"] = nc.declare_dram_parameter("fc2t", [H, C], bf16, isOutput=False)
    hs["fc1b_r"] = nc.declare_dram_parameter("fc1b_r", [P, JB], f32, isOutput=False)
    hs["fc2b_b"] = nc.declare_dram_parameter("fc2b_b", [P, C], f32, isOutput=False)
    hs["expb"] = nc.declare_dram_parameter("expb", [P, 1], f32, isOutput=False)
    hs["ident"] = nc.declare_dram_parameter("ident", [P, P], f32, isOutput=False)
    hs["out"] = nc.declare_dram_parameter("out", [N, C], f32, isOutput=True)
    with tile.TileContext(nc) as tc:
        _emit(nc, tc, hs)
    nc.compile()
    return nc


def _maybe_install_ntff_hook():
    """Optional: lets BASS_TRACE=1 capture NTFF profiles under axon."""
    try:
        import types
        if "antenv.axon_hooks" in sys.modules:
            return
        import antenv
        mod = types.ModuleType("antenv.axon_hooks")
        _hook = [None]
        mod.set_axon_ntff_profile_hook = lambda h: _hook.__setitem__(0, h)
        mod.get_axon_ntff_profile_hook = lambda: _hook[0]
        sys.modules["antenv.axon_hooks"] = mod
        antenv.axon_hooks = mod
        from trn_agent_boot.trn_boot import _ntff_profile_via_ctypes
        mod.set_axon_ntff_profile_hook(
            _ntff_profile_via_ctypes("/opt/axon/libaxon_pjrt.so"))
    except Exception:
        pass


_last_results = None


def kernel(x, ln1_w, ln1_b, ln2_w, ln2_b, fc1_w, fc1_b, fc2_w, fc2_b):
    global _cached_nc, _last_results
    if _cached_nc is None:
        _cached_nc = _build()
    nc = _cached_nc

    bfl = ml_dtypes.bfloat16
    x = np.asarray(x, dtype=np.float32)
    ln1_w = np.asarray(ln1_w, np.float32)
    ln1_b = np.asarray(ln1_b, np.float32)
    # Constant softmax shift: SCALE*(sqrt(C)*max|w| + ||b||_2)^2 upper-bounds
    # every score S[n,m] (Cauchy-Schwarz on rows of y = LN(x)*w + b, each of
    # which has ||y_n|| <= sqrt(C)*max|w| + ||b||), so exp never overflows and
    # the shift is row-constant => softmax is exact and E stays symmetric.
    ybound = float(np.sqrt(C) * np.abs(ln1_w).max() + np.linalg.norm(ln1_b))
    expb = np.full((P, 1), -SCALE * ybound * ybound, np.float32)
    prep = {
        "ln1w_b": np.ascontiguousarray(np.broadcast_to(ln1_w, (P, C))),
        "ln1b_b": np.ascontiguousarray(np.broadcast_to(ln1_b, (P, C))),
        "ln2w_b": np.ascontiguousarray(
            np.broadcast_to(np.asarray(ln2_w, np.float32), (P, C))),
        "ln2b_b": np.ascontiguousarray(
            np.broadcast_to(np.asarray(ln2_b, np.float32), (P, C))),
        "fc1t": np.ascontiguousarray(np.asarray(fc1_w, np.float32).T.astype(bfl)),
        "fc2t": np.ascontiguousarray(np.asarray(fc2_w, np.float32).T.astype(bfl)),
        "fc1b_r": np.ascontiguousarray(
            np.asarray(fc1_b, np.float32).reshape(JB, P).T),
        "fc2b_b": np.ascontiguousarray(
            np.broadcast_to(np.asarray(fc2_b, np.float32), (P, C))),
        "expb": expb,
        "ident": np.eye(P, dtype=np.float32),
    }
    in_maps = [dict(prep, x=np.ascontiguousarray(x[b])) for b in range(B)]

    trace = bool(os.environ.get("BASS_TRACE"))
    if trace:
        _maybe_install_ntff_hook()
    res = run_bass_kernel_spmd(nc, in_maps, list(range(B)), trace=trace)
    _last_results = res
    return np.stack([res.results[b]["out"] for b in range(B)], axis=0)


# revision 9
# speedup vs baseline: 1.4582x; 1.0592x over previous
"""Trainium2 Bass kernel for nn_Block (pre-LN transformer block with dense
self-attention where q=k=v=LN1(x), followed by a GELU MLP).

Sharding: data-parallel over batch B=8 across the 8 NeuronCores (one batch
element per core). Weights are replicated; host-side prep transposes/casts the
MLP weights to bf16 and pre-broadcasts the small LN/bias vectors so the device
kernel does pure compute.

Device algorithm per core (x: [2048, 768] fp32):
  1. LN1 -> y (bf16 natural + transposed via TensorE transpose + ACT copy).
     LN stats via bn_stats/bn_aggr (equal 384-wide chunks); when the LN
     weight/bias inputs are exactly ones/zeros (they are for this problem)
     the scale/shift application is skipped entirely.
  2. E := exp(SCALE*S - c) with a host-computed constant shift
     c = SCALE*(sqrt(C)*max|w1| + ||b1||_2)^2 >= max S (Cauchy-Schwarz on the
     LN-normalized rows), so E is SYMMETRIC and the second attention matmul
     reads E tiles directly as lhsT without transposing the score matrix.
     S = y@y^T via accumulating K=128 matmuls into 512-wide PSUM quarters.
  3. a_unnorm = E @ [y | 1] (ones column makes the softmax denominator Z a
     free extra output column); x2 = x + a_unnorm/Z, streamed to an HBM
     scratch; LN2 fused per block -> zT (TensorE transpose).
  4. hT = gelu(fc1 @ z^T + b1) computed transposed (double-buffered per
     512-column chunk); out = x2 + fc2 @ h^T + b2.
"""

import os
import sys
from contextlib import ExitStack

for _p in ("/opt/trn_rl_repo",):
    if _p not in sys.path:
        sys.path.append(_p)

import numpy as np
import ml_dtypes

import concourse.bass as bass
import concourse.bacc as bacc
import concourse.tile as tile
import concourse.mybir as mybir
from concourse.bass_utils import run_bass_kernel_spmd

f32 = mybir.dt.float32
bf16 = mybir.dt.bfloat16
AF = mybir.ActivationFunctionType
ALU = mybir.AluOpType
AX = mybir.AxisListType

B, N, C, H = 8, 2048, 768, 3072
P = 128
NB = N // P        # 16 row blocks of 128
CCK = C // P       # 6 channel chunks of 128
JB = H // P        # 24 hidden blocks of 128
NQ = 4             # MLP sequence chunks
QW = N // NQ       # 512 columns per MLP chunk
SQ = 4             # S-phase quarters per row block
SW = N // SQ       # 512
YW = C + 4         # y block stride (768 data + ones column + pad)
HEADS = 12
SCALE = 1.0 / float(np.sqrt(C // HEADS))   # 0.125
EPS = 1e-5

_cache = {}


def _ln_normalize(nc, stats, lnscr, xt_ap, w_t, b_t, out_ap, eps_t, skip_wb):
    """out = LN(xt) (*w + b unless skip_wb). out_ap may be bf16."""
    st = stats.tile([P, 12], f32, tag="bn")
    nc.vector.bn_stats(st[:, 0:6], xt_ap[:, 0:384])
    nc.vector.bn_stats(st[:, 6:12], xt_ap[:, 384:768])
    mv = stats.tile([P, 2], f32, tag="mv")
    nc.vector.bn_aggr(mv[:], st[:])
    std = stats.tile([P, 1], f32, tag="std")
    nc.scalar.activation(std[:], mv[:, 1:2], AF.Sqrt, bias=eps_t[:, 0:1])
    rstd = stats.tile([P, 1], f32, tag="rstd")
    nc.vector.reciprocal(rstd[:], std[:])
    negmr = stats.tile([P, 1], f32, tag="negmr")         # -mean*rstd
    nc.vector.tensor_scalar(negmr[:], mv[:, 0:1], rstd[:, 0:1], -1.0,
                            ALU.mult, ALU.mult)
    if skip_wb:
        nc.vector.tensor_scalar(out_ap, xt_ap, rstd[:, 0:1], negmr[:, 0:1],
                                ALU.mult, ALU.add)
    else:
        u = lnscr.tile([P, C], f32, tag="u")
        nc.vector.tensor_scalar(u[:], xt_ap, rstd[:, 0:1], negmr[:, 0:1],
                                ALU.mult, ALU.add)
        v = lnscr.tile([P, C], f32, tag="v")
        nc.vector.scalar_tensor_tensor(v[:], u[:], 1.0, w_t[:],
                                       ALU.mult, ALU.mult)
        nc.vector.scalar_tensor_tensor(out_ap, v[:], 1.0, b_t[:],
                                       ALU.mult, ALU.add)


def _emit(nc, tc, hs, flags):
    skip1, skip2, skipb2 = flags
    ctx = ExitStack()
    with ctx:
        small = ctx.enter_context(tc.tile_pool(name="small", bufs=1))
        stats = ctx.enter_context(tc.tile_pool(name="stats", bufs=4))
        lnscr = ctx.enter_context(tc.tile_pool(name="lnscr", bufs=2))
        xio = ctx.enter_context(tc.tile_pool(name="xio", bufs=3))

        def param(name, shape, tag):
            t = small.tile(shape, f32, tag=tag)
            nc.sync.dma_start(t[:], hs[name].ap())
            return t

        ln1w_t = ln1b_t = ln2w_t = ln2b_t = None
        if not skip1:
            ln1w_t = param("ln1w_b", [P, C], "ln1w")
            ln1b_t = param("ln1b_b", [P, C], "ln1b")
        if not skip2:
            ln2w_t = param("ln2w_b", [P, C], "ln2w")
            ln2b_t = param("ln2b_b", [P, C], "ln2b")
        fc2b_t = None
        if not skipb2:
            fc2b_t = param("fc2b_b", [P, C], "fc2b")
        fc1b_t = param("fc1b_r", [P, JB], "fc1b")
        expb_t = param("expb", [P, 1], "expb")
        identb = small.tile([P, P], bf16, tag="identb")
        nc.sync.dma_start(identb[:], hs["identb"].ap())

        eps_t = small.tile([P, 1], f32, tag="eps")
        nc.vector.memset(eps_t[:], EPS)

        x_ap = hs["x"].ap()
        out_ap = hs["out"].ap()
        x2s = nc.dram_tensor("x2scratch", [N, C], f32)
        x2s_ap = x2s.ap()

        y_pool = tc.alloc_tile_pool(name="ybig", bufs=1)
        y_sb = y_pool.tile([P, NB * YW], bf16, tag="y")
        nc.vector.memset(y_sb[:], 1.0)   # ones column at offset C per block
        yT_pool = tc.alloc_tile_pool(name="yTbig", bufs=1, side="right")
        yT_sb = yT_pool.tile([P, CCK * N], bf16, tag="yT")

        tp_pool = tc.alloc_tile_pool(name="tpsum", bufs=2, space="PSUM",
                                     side="right")

        # ---- Stage 1: LN1 -> y (bf16) + yT (PE transpose) ----
        for i in range(NB):
            xt = xio.tile([P, C], f32, tag="xio")
            nc.sync.dma_start(xt[:], x_ap[i * P:(i + 1) * P, :])
            ysl = y_sb[:, i * YW: i * YW + C]
            _ln_normalize(nc, stats, lnscr, xt[:], ln1w_t, ln1b_t, ysl,
                          eps_t, skip1)
            for c in range(CCK):
                tp = tp_pool.tile([P, P], bf16, tag="tp")
                nc.tensor.transpose(
                    tp[:], y_sb[:, i * YW + c * P: i * YW + (c + 1) * P],
                    identb[:])
                nc.scalar.copy(
                    yT_sb[:, c * N + i * P: c * N + (i + 1) * P], tp[:])

        # ---- Stage 2: S quarters + Exp -> E (bf16) ----
        E_pool = tc.alloc_tile_pool(name="Ebig", bufs=1)
        E_sb = E_pool.tile([P, NB * N], bf16, tag="E")
        with tc.tile_pool(name="spsum", bufs=4, space="PSUM") as sp_pool:
            for i in range(NB):
                for q in range(SQ):
                    s_ps = sp_pool.tile([P, SW], f32, tag="s")
                    for c in range(CCK):
                        nc.tensor.matmul(
                            s_ps[:],
                            yT_sb[:, c * N + i * P: c * N + (i + 1) * P],
                            yT_sb[:, c * N + q * SW: c * N + (q + 1) * SW],
                            start=(c == 0), stop=(c == CCK - 1))
                    nc.scalar.activation(
                        E_sb[:, i * N + q * SW: i * N + (q + 1) * SW],
                        s_ps[:], AF.Exp, bias=expb_t[:, 0:1], scale=SCALE)

        # ---- Stage 3 (fused): a|Z = E@[y|1]; x2 = x + a/Z -> HBM; LN2 -> zT
        yT_pool.release()
        zT_pool = tc.alloc_tile_pool(name="zTbig", bufs=1, side="right")
        zT_sb = zT_pool.tile([P, CCK * N], bf16, tag="zT")
        with tc.tile_pool(name="apsum", bufs=2, space="PSUM") as a_pool:
            for i in range(NB):
                a_ps = a_pool.tile([P, 1024], f32, tag="a")
                for j in range(NB):
                    lhsT = E_sb[:, j * N + i * P: j * N + (i + 1) * P]
                    nc.tensor.matmul(a_ps[:, 0:512], lhsT,
                                     y_sb[:, j * YW: j * YW + 512],
                                     start=(j == 0), stop=(j == NB - 1))
                    nc.tensor.matmul(a_ps[:, 512:769], lhsT,
                                     y_sb[:, j * YW + 512: j * YW + C + 1],
                                     start=(j == 0), stop=(j == NB - 1))
                rZ = stats.tile([P, 1], f32, tag="rZ")
                nc.vector.reciprocal(rZ[:], a_ps[:, 768:769])
                xt = xio.tile([P, C], f32, tag="xio")
                nc.sync.dma_start(xt[:], x_ap[i * P:(i + 1) * P, :])
                x2t = lnscr.tile([P, C], f32, tag="x2t")
                nc.vector.scalar_tensor_tensor(
                    x2t[:], a_ps[:, 0:C], rZ[:, 0:1], xt[:],
                    ALU.mult, ALU.add)
                nc.sync.dma_start(x2s_ap[i * P:(i + 1) * P, :], x2t[:])
                znat = lnscr.tile([P, C], bf16, tag="znat")
                _ln_normalize(nc, stats, lnscr, x2t[:], ln2w_t, ln2b_t,
                              znat[:], eps_t, skip2)
                for c in range(CCK):
                    tp = tp_pool.tile([P, P], bf16, tag="tp")
                    nc.tensor.transpose(tp[:], znat[:, c * P:(c + 1) * P],
                                        identb[:])
                    nc.scalar.copy(
                        zT_sb[:, c * N + i * P: c * N + (i + 1) * P], tp[:])

        # ---- Stage 4: MLP ----
        E_pool.release()
        y_pool.release()
        tp_pool.release()
        w_pool = tc.alloc_tile_pool(name="wbig", bufs=1)
        fc1T_sb = w_pool.tile([P, CCK * H], bf16, tag="fc1T")
        fc2T_sb = w_pool.tile([P, JB * C], bf16, tag="fc2T")
        for c in range(CCK):
            nc.sync.dma_start(fc1T_sb[:, c * H:(c + 1) * H],
                              hs["fc1t"].ap()[c * P:(c + 1) * P, :])
        for j in range(JB):
            nc.sync.dma_start(fc2T_sb[:, j * C:(j + 1) * C],
                              hs["fc2t"].ap()[j * P:(j + 1) * P, :])

        hT_pool = tc.alloc_tile_pool(name="hTbig", bufs=2)
        with tc.tile_pool(name="hpsum", bufs=3, space="PSUM") as h_pool, \
             tc.tile_pool(name="opsum", bufs=2, space="PSUM") as o_pool:
            for q in range(NQ):
                hT_sb = hT_pool.tile([P, JB * QW], bf16, tag="hT")
                for j in range(JB):
                    h_ps = h_pool.tile([P, QW], f32, tag="h")
                    for c in range(CCK):
                        nc.tensor.matmul(
                            h_ps[:],
                            fc1T_sb[:, c * H + j * P: c * H + (j + 1) * P],
                            zT_sb[:, c * N + q * QW: c * N + (q + 1) * QW],
                            start=(c == 0), stop=(c == CCK - 1))
                    nc.scalar.activation(hT_sb[:, j * QW:(j + 1) * QW],
                                         h_ps[:], AF.Gelu,
                                         bias=fc1b_t[:, j:j + 1])
                for t in range(QW // P):
                    i = q * (QW // P) + t
                    o_ps = o_pool.tile([P, 1024], f32, tag="o")
                    for j in range(JB):
                        lhsT = hT_sb[:, j * QW + t * P: j * QW + (t + 1) * P]
                        nc.tensor.matmul(o_ps[:, 0:512], lhsT,
                                         fc2T_sb[:, j * C: j * C + 512],
                                         start=(j == 0), stop=(j == JB - 1))
                        nc.tensor.matmul(o_ps[:, 512:768], lhsT,
                                         fc2T_sb[:, j * C + 512: j * C + C],
                                         start=(j == 0), stop=(j == JB - 1))
                    xre = xio.tile([P, C], f32, tag="xio")
                    nc.sync.dma_start(xre[:], x2s_ap[i * P:(i + 1) * P, :])
                    if skipb2:
                        o2 = lnscr.tile([P, C], f32, tag="o2")
                        nc.vector.scalar_tensor_tensor(
                            o2[:], o_ps[:, 0:C], 1.0, xre[:],
                            ALU.mult, ALU.add)
                    else:
                        o1 = lnscr.tile([P, C], f32, tag="o1")
                        nc.vector.scalar_tensor_tensor(
                            o1[:], o_ps[:, 0:C], 1.0, fc2b_t[:],
                            ALU.mult, ALU.add)
                        o2 = lnscr.tile([P, C], f32, tag="o2")
                        nc.vector.scalar_tensor_tensor(
                            o2[:], o1[:], 1.0, xre[:], ALU.mult, ALU.add)
                    nc.sync.dma_start(out_ap[i * P:(i + 1) * P, :], o2[:])

        hT_pool.release()
        w_pool.release()
        zT_pool.release()


def _build(flags):
    nc = bacc.Bacc("TRN2", target_bir_lowering=False, debug=False, num_devices=8)
    hs = {}
    skip1, skip2, skipb2 = flags
    hs["x"] = nc.declare_dram_parameter("x", [N, C], f32, isOutput=False)
    if not skip1:
        hs["ln1w_b"] = nc.declare_dram_parameter("ln1w_b", [P, C], f32, isOutput=False)
        hs["ln1b_b"] = nc.declare_dram_parameter("ln1b_b", [P, C], f32, isOutput=False)
    if not skip2:
        hs["ln2w_b"] = nc.declare_dram_parameter("ln2w_b", [P, C], f32, isOutput=False)
        hs["ln2b_b"] = nc.declare_dram_parameter("ln2b_b", [P, C], f32, isOutput=False)
    hs["fc1t"] = nc.declare_dram_parameter("fc1t", [C, H], bf16, isOutput=False)
    hs["fc2t"] = nc.declare_dram_parameter("fc2t", [H, C], bf16, isOutput=False)
    hs["fc1b_r"] = nc.declare_dram_parameter("fc1b_r", [P, JB], f32, isOutput=False)
    if not skipb2:
        hs["fc2b_b"] = nc.declare_dram_parameter("fc2b_b", [P, C], f32, isOutput=False)
    hs["expb"] = nc.declare_dram_parameter("expb", [P, 1], f32, isOutput=False)
    hs["identb"] = nc.declare_dram_parameter("identb", [P, P], bf16, isOutput=False)
    hs["out"] = nc.declare_dram_parameter("out", [N, C], f32, isOutput=True)
    with tile.TileContext(nc) as tc:
        _emit(nc, tc, hs, flags)
    nc.compile()
    return nc


def _maybe_install_ntff_hook():
    """Optional: lets BASS_TRACE=1 capture NTFF profiles under axon."""
    try:
        import types
        if "antenv.axon_hooks" in sys.modules:
            return
        import antenv
        mod = types.ModuleType("antenv.axon_hooks")
        _hook = [None]
        mod.set_axon_ntff_profile_hook = lambda h: _hook.__setitem__(0, h)
        mod.get_axon_ntff_profile_hook = lambda: _hook[0]
        sys.modules["antenv.axon_hooks"] = mod
        antenv.axon_hooks = mod
        from trn_agent_boot.trn_boot import _ntff_profile_via_ctypes
        mod.set_axon_ntff_profile_hook(
            _ntff_profile_via_ctypes("/opt/axon/libaxon_pjrt.so"))
    except Exception:
        pass


_last_results = None


def kernel(x, ln1_w, ln1_b, ln2_w, ln2_b, fc1_w, fc1_b, fc2_w, fc2_b):
    global _last_results
    bfl = ml_dtypes.bfloat16
    x = np.asarray(x, dtype=np.float32)
    ln1_w = np.asarray(ln1_w, np.float32)
    ln1_b = np.asarray(ln1_b, np.float32)
    ln2_w = np.asarray(ln2_w, np.float32)
    ln2_b = np.asarray(ln2_b, np.float32)
    fc2_b = np.asarray(fc2_b, np.float32)
    skip1 = bool(np.all(ln1_w == 1.0) and np.all(ln1_b == 0.0))
    skip2 = bool(np.all(ln2_w == 1.0) and np.all(ln2_b == 0.0))
    skipb2 = bool(np.all(fc2_b == 0.0))
    flags = (skip1, skip2, skipb2)
    if flags not in _cache:
        _cache[flags] = _build(flags)
    nc = _cache[flags]

    # Constant softmax shift: SCALE*(sqrt(C)*max|w| + ||b||_2)^2 upper-bounds
    # every score S[n,m] (Cauchy-Schwarz on rows of y = LN(x)*w + b, each of
    # which has ||y_n|| <= sqrt(C)*max|w| + ||b||), so exp never overflows and
    # the shift is row-constant => softmax is exact and E stays symmetric.
    ybound = float(np.sqrt(C) * np.abs(ln1_w).max() + np.linalg.norm(ln1_b))
    expb = np.full((P, 1), -SCALE * ybound * ybound, np.float32)
    prep = {
        "fc1t": np.ascontiguousarray(np.asarray(fc1_w, np.float32).T.astype(bfl)),
        "fc2t": np.ascontiguousarray(np.asarray(fc2_w, np.float32).T.astype(bfl)),
        "fc1b_r": np.ascontiguousarray(
            np.asarray(fc1_b, np.float32).reshape(JB, P).T),
        "expb": expb,
        "identb": np.eye(P, dtype=np.float32).astype(bfl),
    }
    if not skip1:
        prep["ln1w_b"] = np.ascontiguousarray(np.broadcast_to(ln1_w, (P, C)))
        prep["ln1b_b"] = np.ascontiguousarray(np.broadcast_to(ln1_b, (P, C)))
    if not skip2:
        prep["ln2w_b"] = np.ascontiguousarray(np.broadcast_to(ln2_w, (P, C)))
        prep["ln2b_b"] = np.ascontiguousarray(np.broadcast_to(ln2_b, (P, C)))
    if not skipb2:
        prep["fc2b_b"] = np.ascontiguousarray(np.broadcast_to(fc2_b, (P, C)))
    in_maps = [dict(prep, x=np.ascontiguousarray(x[b])) for b in range(B)]

    trace = bool(os.environ.get("BASS_TRACE"))
    if trace:
        _maybe_install_ntff_hook()
    res = run_bass_kernel_spmd(nc, in_maps, list(range(B)), trace=trace)
    _last_results = res
    return np.stack([res.results[b]["out"] for b in range(B)], axis=0)


# revision 11
# speedup vs baseline: 1.4794x; 1.0145x over previous
"""Trainium2 Bass kernel for nn_Block (pre-LN transformer block with dense
self-attention where q=k=v=LN1(x), followed by a GELU MLP).

Sharding: data-parallel over batch B=8 across the 8 NeuronCores (one batch
element per core). Weights are replicated; host-side prep transposes/casts the
MLP weights to bf16 and pre-broadcasts the small LN/bias vectors so the device
kernel does pure compute.

Device algorithm per core (x: [2048, 768] fp32):
  1. LN1 -> y (bf16 natural + transposed via TensorE transpose + ACT copy).
     LN stats via bn_stats/bn_aggr (equal 384-wide chunks); when the LN
     weight/bias inputs are exactly ones/zeros (they are for this problem)
     the scale/shift application is skipped entirely.
  2. E := exp(SCALE*S - c) with a host-computed constant shift
     c = SCALE*(sqrt(C)*max|w1| + ||b1||_2)^2 >= max S (Cauchy-Schwarz on the
     LN-normalized rows), so E is SYMMETRIC and the second attention matmul
     reads E tiles directly as lhsT without transposing the score matrix.
     S = y@y^T via accumulating K=128 matmuls into 512-wide PSUM quarters.
  3. a_unnorm = E @ [y | 1] (ones column makes the softmax denominator Z a
     free extra output column); x2 = x + a_unnorm/Z, streamed to an HBM
     scratch; LN2 fused per block -> zT (TensorE transpose).
  4. hT = gelu(fc1 @ z^T + b1) computed transposed (double-buffered per
     512-column chunk); out = x2 + fc2 @ h^T + b2.
"""

import os
import sys
from contextlib import ExitStack

for _p in ("/opt/trn_rl_repo",):
    if _p not in sys.path:
        sys.path.append(_p)

import numpy as np
import ml_dtypes

import concourse.bass as bass
import concourse.bacc as bacc
import concourse.tile as tile
import concourse.mybir as mybir
from concourse.bass_utils import run_bass_kernel_spmd

f32 = mybir.dt.float32
bf16 = mybir.dt.bfloat16
AF = mybir.ActivationFunctionType
ALU = mybir.AluOpType
AX = mybir.AxisListType

B, N, C, H = 8, 2048, 768, 3072
P = 128
NB = N // P        # 16 row blocks of 128
CCK = C // P       # 6 channel chunks of 128
JB = H // P        # 24 hidden blocks of 128
NQ = 4             # MLP sequence chunks
QW = N // NQ       # 512 columns per MLP chunk
SQ = 4             # S-phase quarters per row block
SW = N // SQ       # 512
YW = C + 4         # y block stride (768 data + ones column + pad)
HEADS = 12
SCALE = 1.0 / float(np.sqrt(C // HEADS))   # 0.125
EPS = 1e-5

_cache = {}


def _ln_normalize(nc, stats, lnscr, xt_ap, w_t, b_t, out_ap, eps_t, skip_wb):
    """out = LN(xt) (*w + b unless skip_wb). out_ap may be bf16."""
    st = stats.tile([P, 12], f32, tag="bn")
    nc.vector.bn_stats(st[:, 0:6], xt_ap[:, 0:384])
    nc.vector.bn_stats(st[:, 6:12], xt_ap[:, 384:768])
    mv = stats.tile([P, 2], f32, tag="mv")
    nc.vector.bn_aggr(mv[:], st[:])
    std = stats.tile([P, 1], f32, tag="std")
    nc.scalar.activation(std[:], mv[:, 1:2], AF.Sqrt, bias=eps_t[:, 0:1])
    rstd = stats.tile([P, 1], f32, tag="rstd")
    nc.vector.reciprocal(rstd[:], std[:])
    negmr = stats.tile([P, 1], f32, tag="negmr")         # -mean*rstd
    nc.vector.tensor_scalar(negmr[:], mv[:, 0:1], rstd[:, 0:1], -1.0,
                            ALU.mult, ALU.mult)
    if skip_wb:
        nc.vector.tensor_scalar(out_ap, xt_ap, rstd[:, 0:1], negmr[:, 0:1],
                                ALU.mult, ALU.add)
    else:
        u = lnscr.tile([P, C], f32, tag="u")
        nc.vector.tensor_scalar(u[:], xt_ap, rstd[:, 0:1], negmr[:, 0:1],
                                ALU.mult, ALU.add)
        v = lnscr.tile([P, C], f32, tag="v")
        nc.vector.scalar_tensor_tensor(v[:], u[:], 1.0, w_t[:],
                                       ALU.mult, ALU.mult)
        nc.vector.scalar_tensor_tensor(out_ap, v[:], 1.0, b_t[:],
                                       ALU.mult, ALU.add)


def _emit(nc, tc, hs, flags):
    skip1, skip2, skipb2 = flags
    ctx = ExitStack()
    with ctx:
        small = ctx.enter_context(tc.tile_pool(name="small", bufs=1))
        stats = ctx.enter_context(tc.tile_pool(name="stats", bufs=4))
        lnscr = ctx.enter_context(tc.tile_pool(name="lnscr", bufs=2))
        xio = ctx.enter_context(tc.tile_pool(name="xio", bufs=3))

        def param(name, shape, tag):
            t = small.tile(shape, f32, tag=tag)
            nc.sync.dma_start(t[:], hs[name].ap())
            return t

        ln1w_t = ln1b_t = ln2w_t = ln2b_t = None
        if not skip1:
            ln1w_t = param("ln1w_b", [P, C], "ln1w")
            ln1b_t = param("ln1b_b", [P, C], "ln1b")
        if not skip2:
            ln2w_t = param("ln2w_b", [P, C], "ln2w")
            ln2b_t = param("ln2b_b", [P, C], "ln2b")
        fc2b_t = None
        if not skipb2:
            fc2b_t = param("fc2b_b", [P, C], "fc2b")
        fc1b_t = param("fc1b_r", [P, JB], "fc1b")
        expb_t = param("expb", [P, 1], "expb")
        identb = small.tile([P, P], bf16, tag="identb")
        nc.sync.dma_start(identb[:], hs["identb"].ap())

        eps_t = small.tile([P, 1], f32, tag="eps")
        nc.vector.memset(eps_t[:], EPS)

        x_ap = hs["x"].ap()
        out_ap = hs["out"].ap()
        x2s = nc.dram_tensor("x2scratch", [N, C], f32)
        x2s_ap = x2s.ap()

        y_pool = tc.alloc_tile_pool(name="ybig", bufs=1)
        y_sb = y_pool.tile([P, NB * YW], bf16, tag="y")
        nc.vector.memset(y_sb[:], 1.0)   # ones column at offset C per block
        yT_pool = tc.alloc_tile_pool(name="yTbig", bufs=1, side="right")
        yT_sb = yT_pool.tile([P, CCK * N], bf16, tag="yT")

        tp_pool = tc.alloc_tile_pool(name="tpsum", bufs=2, space="PSUM",
                                     side="right")

        # ---- Stage 1: LN1 -> y (bf16) + yT (PE transpose) ----
        for i in range(NB):
            xt = xio.tile([P, C], f32, tag="xio")
            nc.sync.dma_start(xt[:], x_ap[i * P:(i + 1) * P, :])
            ysl = y_sb[:, i * YW: i * YW + C]
            _ln_normalize(nc, stats, lnscr, xt[:], ln1w_t, ln1b_t, ysl,
                          eps_t, skip1)
            for c in range(CCK):
                tp = tp_pool.tile([P, P], bf16, tag="tp")
                nc.tensor.transpose(
                    tp[:], y_sb[:, i * YW + c * P: i * YW + (c + 1) * P],
                    identb[:])
                nc.scalar.copy(
                    yT_sb[:, c * N + i * P: c * N + (i + 1) * P], tp[:])

        # ---- Stage 2: S quarters + Exp -> E (bf16) ----
        # S is symmetric: compute only quarters covering m-blocks >= i
        # (q >= i//4), then mirror the strictly-lower 128x128 tiles via
        # TensorE transpose + DVE copy.
        E_pool = tc.alloc_tile_pool(name="Ebig", bufs=1)
        E_sb = E_pool.tile([P, NB * N], bf16, tag="E")
        with tc.tile_pool(name="spsum", bufs=4, space="PSUM") as sp_pool:
            for i in range(NB):
                qs = list(range(i // 4, SQ))
                for g in range(0, len(qs), 2):
                    pair = qs[g:g + 2]
                    sps = {q: sp_pool.tile([P, SW], f32, tag="s",
                                           name=f"s_{i}_{q}")
                           for q in pair}
                    for c in range(CCK):
                        lhsT = yT_sb[:, c * N + i * P: c * N + (i + 1) * P]
                        for q in pair:
                            nc.tensor.matmul(
                                sps[q][:], lhsT,
                                yT_sb[:, c * N + q * SW: c * N + (q + 1) * SW],
                                start=(c == 0), stop=(c == CCK - 1))
                    for q in pair:
                        nc.scalar.activation(
                            E_sb[:, i * N + q * SW: i * N + (q + 1) * SW],
                            sps[q][:], AF.Exp, bias=expb_t[:, 0:1],
                            scale=SCALE)
            for r in range(NB):
                for m in range(r):
                    tp = tp_pool.tile([P, P], bf16, tag="tp")
                    nc.tensor.transpose(
                        tp[:], E_sb[:, m * N + r * P: m * N + (r + 1) * P],
                        identb[:])
                    nc.vector.tensor_copy(
                        E_sb[:, r * N + m * P: r * N + (m + 1) * P], tp[:])

        # ---- Stage 3 (fused): a|Z = E@[y|1]; x2 = x + a/Z -> HBM; LN2 -> zT
        yT_pool.release()
        zT_pool = tc.alloc_tile_pool(name="zTbig", bufs=1, side="right")
        zT_sb = zT_pool.tile([P, CCK * N], bf16, tag="zT")
        with tc.tile_pool(name="apsum", bufs=2, space="PSUM") as a_pool:
            for i in range(NB):
                a_ps = a_pool.tile([P, 1024], f32, tag="a")
                for j in range(NB):
                    lhsT = E_sb[:, j * N + i * P: j * N + (i + 1) * P]
                    nc.tensor.matmul(a_ps[:, 0:512], lhsT,
                                     y_sb[:, j * YW: j * YW + 512],
                                     start=(j == 0), stop=(j == NB - 1))
                    nc.tensor.matmul(a_ps[:, 512:769], lhsT,
                                     y_sb[:, j * YW + 512: j * YW + C + 1],
                                     start=(j == 0), stop=(j == NB - 1))
                rZ = stats.tile([P, 1], f32, tag="rZ")
                nc.vector.reciprocal(rZ[:], a_ps[:, 768:769])
                xt = xio.tile([P, C], f32, tag="xio")
                nc.sync.dma_start(xt[:], x_ap[i * P:(i + 1) * P, :])
                x2t = lnscr.tile([P, C], f32, tag="x2t")
                nc.vector.scalar_tensor_tensor(
                    x2t[:], a_ps[:, 0:C], rZ[:, 0:1], xt[:],
                    ALU.mult, ALU.add)
                nc.sync.dma_start(x2s_ap[i * P:(i + 1) * P, :], x2t[:])
                znat = lnscr.tile([P, C], bf16, tag="znat")
                _ln_normalize(nc, stats, lnscr, x2t[:], ln2w_t, ln2b_t,
                              znat[:], eps_t, skip2)
                for c in range(CCK):
                    tp = tp_pool.tile([P, P], bf16, tag="tp")
                    nc.tensor.transpose(tp[:], znat[:, c * P:(c + 1) * P],
                                        identb[:])
                    nc.scalar.copy(
                        zT_sb[:, c * N + i * P: c * N + (i + 1) * P], tp[:])

        # ---- Stage 4: MLP ----
        E_pool.release()
        y_pool.release()
        tp_pool.release()
        w_pool = tc.alloc_tile_pool(name="wbig", bufs=1)
        fc1T_sb = w_pool.tile([P, CCK * H], bf16, tag="fc1T")
        fc2T_sb = w_pool.tile([P, JB * C], bf16, tag="fc2T")
        for c in range(CCK):
            nc.sync.dma_start(fc1T_sb[:, c * H:(c + 1) * H],
                              hs["fc1t"].ap()[c * P:(c + 1) * P, :])
        for j in range(JB):
            nc.sync.dma_start(fc2T_sb[:, j * C:(j + 1) * C],
                              hs["fc2t"].ap()[j * P:(j + 1) * P, :])

        hT_pool = tc.alloc_tile_pool(name="hTbig", bufs=2)
        with tc.tile_pool(name="hpsum", bufs=3, space="PSUM") as h_pool, \
             tc.tile_pool(name="opsum", bufs=2, space="PSUM") as o_pool:
            for q in range(NQ):
                hT_sb = hT_pool.tile([P, JB * QW], bf16, tag="hT")
                for j in range(JB):
                    h_ps = h_pool.tile([P, QW], f32, tag="h")
                    for c in range(CCK):
                        nc.tensor.matmul(
                            h_ps[:],
                            fc1T_sb[:, c * H + j * P: c * H + (j + 1) * P],
                            zT_sb[:, c * N + q * QW: c * N + (q + 1) * QW],
                            start=(c == 0), stop=(c == CCK - 1))
                    nc.scalar.activation(hT_sb[:, j * QW:(j + 1) * QW],
                                         h_ps[:], AF.Gelu,
                                         bias=fc1b_t[:, j:j + 1])
                for t in range(QW // P):
                    i = q * (QW // P) + t
                    o_ps = o_pool.tile([P, 1024], f32, tag="o")
                    for j in range(JB):
                        lhsT = hT_sb[:, j * QW + t * P: j * QW + (t + 1) * P]
                        nc.tensor.matmul(o_ps[:, 0:512], lhsT,
                                         fc2T_sb[:, j * C: j * C + 512],
                                         start=(j == 0), stop=(j == JB - 1))
                        nc.tensor.matmul(o_ps[:, 512:768], lhsT,
                                         fc2T_sb[:, j * C + 512: j * C + C],
                                         start=(j == 0), stop=(j == JB - 1))
                    xre = xio.tile([P, C], f32, tag="xio")
                    nc.sync.dma_start(xre[:], x2s_ap[i * P:(i + 1) * P, :])
                    if skipb2:
                        o2 = lnscr.tile([P, C], f32, tag="o2")
                        nc.vector.scalar_tensor_tensor(
                            o2[:], o_ps[:, 0:C], 1.0, xre[:],
                            ALU.mult, ALU.add)
                    else:
                        o1 = lnscr.tile([P, C], f32, tag="o1")
                        nc.vector.scalar_tensor_tensor(
                            o1[:], o_ps[:, 0:C], 1.0, fc2b_t[:],
                            ALU.mult, ALU.add)
                        o2 = lnscr.tile([P, C], f32, tag="o2")
                        nc.vector.scalar_tensor_tensor(
                            o2[:], o1[:], 1.0, xre[:], ALU.mult, ALU.add)
                    nc.sync.dma_start(out_ap[i * P:(i + 1) * P, :], o2[:])

        hT_pool.release()
        w_pool.release()
        zT_pool.release()


def _build(flags):
    nc = bacc.Bacc("TRN2", target_bir_lowering=False, debug=False, num_devices=8)
    hs = {}
    skip1, skip2, skipb2 = flags
    hs["x"] = nc.declare_dram_parameter("x", [N, C], f32, isOutput=False)
    if not skip1:
        hs["ln1w_b"] = nc.declare_dram_parameter("ln1w_b", [P, C], f32, isOutput=False)
        hs["ln1b_b"] = nc.declare_dram_parameter("ln1b_b", [P, C], f32, isOutput=False)
    if not skip2:
        hs["ln2w_b"] = nc.declare_dram_parameter("ln2w_b", [P, C], f32, isOutput=False)
        hs["ln2b_b"] = nc.declare_dram_parameter("ln2b_b", [P, C], f32, isOutput=False)
    hs["fc1t"] = nc.declare_dram_parameter("fc1t", [C, H], bf16, isOutput=False)
    hs["fc2t"] = nc.declare_dram_parameter("fc2t", [H, C], bf16, isOutput=False)
    hs["fc1b_r"] = nc.declare_dram_parameter("fc1b_r", [P, JB], f32, isOutput=False)
    if not skipb2:
        hs["fc2b_b"] = nc.declare_dram_parameter("fc2b_b", [P, C], f32, isOutput=False)
    hs["expb"] = nc.declare_dram_parameter("expb", [P, 1], f32, isOutput=False)
    hs["identb"] = nc.declare_dram_parameter("identb", [P, P], bf16, isOutput=False)
    hs["out"] = nc.declare_dram_parameter("out", [N, C], f32, isOutput=True)
    with tile.TileContext(nc) as tc:
        _emit(nc, tc, hs, flags)
    nc.compile()
    return nc


def _maybe_install_ntff_hook():
    """Optional: lets BASS_TRACE=1 capture NTFF profiles under axon."""
    try:
        import types
        if "antenv.axon_hooks" in sys.modules:
            return
        import antenv
        mod = types.ModuleType("antenv.axon_hooks")
        _hook = [None]
        mod.set_axon_ntff_profile_hook = lambda h: _hook.__setitem__(0, h)
        mod.get_axon_ntff_profile_hook = lambda: _hook[0]
        sys.modules["antenv.axon_hooks"] = mod
        antenv.axon_hooks = mod
        from trn_agent_boot.trn_boot import _ntff_profile_via_ctypes
        mod.set_axon_ntff_profile_hook(
            _ntff_profile_via_ctypes("/opt/axon/libaxon_pjrt.so"))
    except Exception:
        pass


_last_results = None


def kernel(x, ln1_w, ln1_b, ln2_w, ln2_b, fc1_w, fc1_b, fc2_w, fc2_b):
    global _last_results
    bfl = ml_dtypes.bfloat16
    x = np.asarray(x, dtype=np.float32)
    ln1_w = np.asarray(ln1_w, np.float32)
    ln1_b = np.asarray(ln1_b, np.float32)
    ln2_w = np.asarray(ln2_w, np.float32)
    ln2_b = np.asarray(ln2_b, np.float32)
    fc2_b = np.asarray(fc2_b, np.float32)
    skip1 = bool(np.all(ln1_w == 1.0) and np.all(ln1_b == 0.0))
    skip2 = bool(np.all(ln2_w == 1.0) and np.all(ln2_b == 0.0))
    skipb2 = bool(np.all(fc2_b == 0.0))
    flags = (skip1, skip2, skipb2)
    if flags not in _cache:
        _cache[flags] = _build(flags)
    nc = _cache[flags]

    # Constant softmax shift: SCALE*(sqrt(C)*max|w| + ||b||_2)^2 upper-bounds
    # every score S[n,m] (Cauchy-Schwarz on rows of y = LN(x)*w + b, each of
    # which has ||y_n|| <= sqrt(C)*max|w| + ||b||), so exp never overflows and
    # the shift is row-constant => softmax is exact and E stays symmetric.
    ybound = float(np.sqrt(C) * np.abs(ln1_w).max() + np.linalg.norm(ln1_b))
    expb = np.full((P, 1), -SCALE * ybound * ybound, np.float32)
    prep = {
        "fc1t": np.ascontiguousarray(np.asarray(fc1_w, np.float32).T.astype(bfl)),
        "fc2t": np.ascontiguousarray(np.asarray(fc2_w, np.float32).T.astype(bfl)),
        "fc1b_r": np.ascontiguousarray(
            np.asarray(fc1_b, np.float32).reshape(JB, P).T),
        "expb": expb,
        "identb": np.eye(P, dtype=np.float32).astype(bfl),
    }
    if not skip1:
        prep["ln1w_b"] = np.ascontiguousarray(np.broadcast_to(ln1_w, (P, C)))
        prep["ln1b_b"] = np.ascontiguousarray(np.broadcast_to(ln1_b, (P, C)))
    if not skip2:
        prep["ln2w_b"] = np.ascontiguousarray(np.broadcast_to(ln2_w, (P, C)))
        prep["ln2b_b"] = np.ascontiguousarray(np.broadcast_to(ln2_b, (P, C)))
    if not skipb2:
        prep["fc2b_b"] = np.ascontiguousarray(np.broadcast_to(fc2_b, (P, C)))
    in_maps = [dict(prep, x=np.ascontiguousarray(x[b])) for b in range(B)]

    trace = bool(os.environ.get("BASS_TRACE"))
    if trace:
        _maybe_install_ntff_hook()
    res = run_bass_kernel_spmd(nc, in_maps, list(range(B)), trace=trace)
    _last_results = res
    return np.stack([res.results[b]["out"] for b in range(B)], axis=0)


# revision 13
# speedup vs baseline: 1.5234x; 1.0298x over previous
"""Trainium2 Bass kernel for nn_Block (pre-LN transformer block with dense
self-attention where q=k=v=LN1(x), followed by a GELU MLP).

Sharding: data-parallel over batch B=8 across the 8 NeuronCores (one batch
element per core). Weights are replicated; host-side prep transposes/casts the
MLP weights to bf16 and pre-broadcasts the small LN/bias vectors so the device
kernel does pure compute.

Device algorithm per core (x: [2048, 768] fp32):
  1. LN1 -> y (bf16 natural + transposed via TensorE transpose + ACT copy).
     LN stats via bn_stats/bn_aggr (equal 384-wide chunks); when the LN
     weight/bias inputs are exactly ones/zeros (they are for this problem)
     the scale/shift application is skipped entirely.
  2. E := exp(SCALE*S - c) with a host-computed constant shift
     c = SCALE*(sqrt(C)*max|w1| + ||b1||_2)^2 >= max S (Cauchy-Schwarz on the
     LN-normalized rows), so E is SYMMETRIC and the second attention matmul
     reads E tiles directly as lhsT without transposing the score matrix.
     S = y@y^T via accumulating K=128 matmuls into 512-wide PSUM quarters.
  3. a_unnorm = E @ [y | 1] (ones column makes the softmax denominator Z a
     free extra output column); x2 = x + a_unnorm/Z, streamed to an HBM
     scratch; LN2 fused per block -> zT (TensorE transpose).
  4. hT = gelu(fc1 @ z^T + b1) computed transposed (double-buffered per
     512-column chunk); out = x2 + fc2 @ h^T + b2.
"""

import os
import sys
from contextlib import ExitStack

for _p in ("/opt/trn_rl_repo",):
    if _p not in sys.path:
        sys.path.append(_p)

import numpy as np
import ml_dtypes

import concourse.bass as bass
import concourse.bacc as bacc
import concourse.tile as tile
import concourse.mybir as mybir
from concourse.bass_utils import run_bass_kernel_spmd

f32 = mybir.dt.float32
bf16 = mybir.dt.bfloat16
AF = mybir.ActivationFunctionType
ALU = mybir.AluOpType
AX = mybir.AxisListType

B, N, C, H = 8, 2048, 768, 3072
P = 128
NB = N // P        # 16 row blocks of 128
CCK = C // P       # 6 channel chunks of 128
JB = H // P        # 24 hidden blocks of 128
NQ = 4             # MLP sequence chunks
QW = N // NQ       # 512 columns per MLP chunk
SQ = 4             # S-phase quarters per row block
SW = N // SQ       # 512
YW = C + 4         # y block stride (768 data + ones column + pad)
HEADS = 12
SCALE = 1.0 / float(np.sqrt(C // HEADS))   # 0.125
EPS = 1e-5

_cache = {}


def _ln_normalize(nc, stats, lnscr, xt_ap, w_t, b_t, out_ap, eps_t, skip_wb):
    """out = LN(xt) (*w + b unless skip_wb). out_ap may be bf16."""
    st = stats.tile([P, 12], f32, tag="bn")
    nc.vector.bn_stats(st[:, 0:6], xt_ap[:, 0:384])
    nc.vector.bn_stats(st[:, 6:12], xt_ap[:, 384:768])
    mv = stats.tile([P, 2], f32, tag="mv")
    nc.vector.bn_aggr(mv[:], st[:])
    std = stats.tile([P, 1], f32, tag="std")
    nc.scalar.activation(std[:], mv[:, 1:2], AF.Sqrt, bias=eps_t[:, 0:1])
    rstd = stats.tile([P, 1], f32, tag="rstd")
    nc.vector.reciprocal(rstd[:], std[:])
    negmr = stats.tile([P, 1], f32, tag="negmr")         # -mean*rstd
    nc.vector.tensor_scalar(negmr[:], mv[:, 0:1], rstd[:, 0:1], -1.0,
                            ALU.mult, ALU.mult)
    if skip_wb:
        nc.vector.tensor_scalar(out_ap, xt_ap, rstd[:, 0:1], negmr[:, 0:1],
                                ALU.mult, ALU.add)
    else:
        u = lnscr.tile([P, C], f32, tag="u")
        nc.vector.tensor_scalar(u[:], xt_ap, rstd[:, 0:1], negmr[:, 0:1],
                                ALU.mult, ALU.add)
        v = lnscr.tile([P, C], f32, tag="v")
        nc.vector.scalar_tensor_tensor(v[:], u[:], 1.0, w_t[:],
                                       ALU.mult, ALU.mult)
        nc.vector.scalar_tensor_tensor(out_ap, v[:], 1.0, b_t[:],
                                       ALU.mult, ALU.add)


def _emit(nc, tc, hs, flags):
    skip1, skip2, skipb2 = flags
    ctx = ExitStack()
    with ctx:
        small = ctx.enter_context(tc.tile_pool(name="small", bufs=1))
        stats = ctx.enter_context(tc.tile_pool(name="stats", bufs=4))
        lnscr = ctx.enter_context(tc.tile_pool(name="lnscr", bufs=2))
        xio = ctx.enter_context(tc.tile_pool(name="xio", bufs=3))

        def param(name, shape, tag):
            t = small.tile(shape, f32, tag=tag)
            nc.sync.dma_start(t[:], hs[name].ap())
            return t

        ln1w_t = ln1b_t = ln2w_t = ln2b_t = None
        if not skip1:
            ln1w_t = param("ln1w_b", [P, C], "ln1w")
            ln1b_t = param("ln1b_b", [P, C], "ln1b")
        if not skip2:
            ln2w_t = param("ln2w_b", [P, C], "ln2w")
            ln2b_t = param("ln2b_b", [P, C], "ln2b")
        fc2b_t = None
        if not skipb2:
            fc2b_t = param("fc2b_b", [P, C], "fc2b")
        fc1b_t = param("fc1b_r", [P, JB], "fc1b")
        expb_t = param("expb", [P, 1], "expb")
        identb = small.tile([P, P], bf16, tag="identb")
        nc.sync.dma_start(identb[:], hs["identb"].ap())

        eps_t = small.tile([P, 1], f32, tag="eps")
        nc.vector.memset(eps_t[:], EPS)

        x_ap = hs["x"].ap()
        out_ap = hs["out"].ap()
        x2s = nc.dram_tensor("x2scratch", [N, C], f32)
        x2s_ap = x2s.ap()

        y_pool = tc.alloc_tile_pool(name="ybig", bufs=1)
        y_sb = y_pool.tile([P, NB * YW], bf16, tag="y")
        # ones column at offset C per block (strided memset of pad cols only)
        nc.vector.memset(
            y_sb[:].rearrange("p (i w) -> p i w", w=YW)[:, :, C:YW], 1.0)
        yT_pool = tc.alloc_tile_pool(name="yTbig", bufs=1, side="right")
        yT_sb = yT_pool.tile([P, CCK * N], bf16, tag="yT")

        tp_pool = tc.alloc_tile_pool(name="tpsum", bufs=2, space="PSUM",
                                     side="right")

        # ---- Stage 1: LN1 -> y (bf16) + yT (PE transpose) ----
        for i in range(NB):
            xt = xio.tile([P, C], f32, tag="xio")
            nc.sync.dma_start(xt[:], x_ap[i * P:(i + 1) * P, :])
            ysl = y_sb[:, i * YW: i * YW + C]
            _ln_normalize(nc, stats, lnscr, xt[:], ln1w_t, ln1b_t, ysl,
                          eps_t, skip1)
            for c in range(CCK):
                tp = tp_pool.tile([P, P], bf16, tag="tp")
                nc.tensor.transpose(
                    tp[:], y_sb[:, i * YW + c * P: i * YW + (c + 1) * P],
                    identb[:])
                nc.scalar.copy(
                    yT_sb[:, c * N + i * P: c * N + (i + 1) * P], tp[:])

        # ---- Stage 2: S quarters + Exp -> E (bf16) ----
        # S is symmetric: compute only quarters covering m-blocks >= i
        # (q >= i//4), then mirror the strictly-lower 128x128 tiles via
        # TensorE transpose + DVE copy.
        E_pool = tc.alloc_tile_pool(name="Ebig", bufs=1)
        E_sb = E_pool.tile([P, NB * N], bf16, tag="E")
        with tc.tile_pool(name="spsum", bufs=4, space="PSUM") as sp_pool:
            # Emit quarters in input-availability order: quarter (i, q) needs
            # LN1 tiles <= max(i, 4q+3), so sweep q ascending, i ascending.
            for q in range(SQ):
                for i in range(4 * q + 4) if q < SQ - 1 else range(NB):
                    if q < i // 4:
                        continue
                    s_ps = sp_pool.tile([P, SW], f32, tag="s",
                                        name=f"s_{i}_{q}")
                    for c in range(CCK):
                        nc.tensor.matmul(
                            s_ps[:],
                            yT_sb[:, c * N + i * P: c * N + (i + 1) * P],
                            yT_sb[:, c * N + q * SW: c * N + (q + 1) * SW],
                            start=(c == 0), stop=(c == CCK - 1))
                    nc.scalar.activation(
                        E_sb[:, i * N + q * SW: i * N + (q + 1) * SW],
                        s_ps[:], AF.Exp, bias=expb_t[:, 0:1], scale=SCALE)
            for r in range(NB):
                for m in range(r):
                    tp = tp_pool.tile([P, P], bf16, tag="tp")
                    nc.tensor.transpose(
                        tp[:], E_sb[:, m * N + r * P: m * N + (r + 1) * P],
                        identb[:])
                    nc.vector.tensor_copy(
                        E_sb[:, r * N + m * P: r * N + (m + 1) * P], tp[:])

        # ---- Stage 3 (fused): a|Z = E@[y|1]; x2 = x + a/Z -> HBM; LN2 -> zT
        yT_pool.release()
        zT_pool = tc.alloc_tile_pool(name="zTbig", bufs=1, side="right")
        zT_sb = zT_pool.tile([P, CCK * N], bf16, tag="zT")
        with tc.tile_pool(name="apsum", bufs=2, space="PSUM") as a_pool:
            for i in range(NB):
                a_ps = a_pool.tile([P, 1024], f32, tag="a")
                for j in range(NB):
                    lhsT = E_sb[:, j * N + i * P: j * N + (i + 1) * P]
                    nc.tensor.matmul(a_ps[:, 0:512], lhsT,
                                     y_sb[:, j * YW: j * YW + 512],
                                     start=(j == 0), stop=(j == NB - 1))
                    nc.tensor.matmul(a_ps[:, 512:769], lhsT,
                                     y_sb[:, j * YW + 512: j * YW + C + 1],
                                     start=(j == 0), stop=(j == NB - 1))
                rZ = stats.tile([P, 1], f32, tag="rZ")
                nc.vector.reciprocal(rZ[:], a_ps[:, 768:769])
                xt = xio.tile([P, C], f32, tag="xio")
                nc.sync.dma_start(xt[:], x_ap[i * P:(i + 1) * P, :])
                x2t = lnscr.tile([P, C], f32, tag="x2t")
                nc.vector.scalar_tensor_tensor(
                    x2t[:], a_ps[:, 0:C], rZ[:, 0:1], xt[:],
                    ALU.mult, ALU.add)
                nc.sync.dma_start(x2s_ap[i * P:(i + 1) * P, :], x2t[:])
                znat = lnscr.tile([P, C], bf16, tag="znat")
                _ln_normalize(nc, stats, lnscr, x2t[:], ln2w_t, ln2b_t,
                              znat[:], eps_t, skip2)
                for c in range(CCK):
                    tp = tp_pool.tile([P, P], bf16, tag="tp")
                    nc.tensor.transpose(tp[:], znat[:, c * P:(c + 1) * P],
                                        identb[:])
                    nc.scalar.copy(
                        zT_sb[:, c * N + i * P: c * N + (i + 1) * P], tp[:])

        # ---- Stage 4: MLP ----
        E_pool.release()
        y_pool.release()
        tp_pool.release()
        w_pool = tc.alloc_tile_pool(name="wbig", bufs=1)
        fc1T_sb = w_pool.tile([P, CCK * H], bf16, tag="fc1T")
        fc2T_sb = w_pool.tile([P, JB * C], bf16, tag="fc2T")
        for c in range(CCK):
            nc.sync.dma_start(fc1T_sb[:, c * H:(c + 1) * H],
                              hs["fc1t"].ap()[c * P:(c + 1) * P, :])
        for j in range(JB):
            nc.sync.dma_start(fc2T_sb[:, j * C:(j + 1) * C],
                              hs["fc2t"].ap()[j * P:(j + 1) * P, :])

        hT_pool = tc.alloc_tile_pool(name="hTbig", bufs=2)
        with tc.tile_pool(name="hpsum", bufs=3, space="PSUM") as h_pool, \
             tc.tile_pool(name="opsum", bufs=2, space="PSUM") as o_pool:
            for q in range(NQ):
                hT_sb = hT_pool.tile([P, JB * QW], bf16, tag="hT")
                for j in range(JB):
                    h_ps = h_pool.tile([P, QW], f32, tag="h")
                    for c in range(CCK):
                        nc.tensor.matmul(
                            h_ps[:],
                            fc1T_sb[:, c * H + j * P: c * H + (j + 1) * P],
                            zT_sb[:, c * N + q * QW: c * N + (q + 1) * QW],
                            start=(c == 0), stop=(c == CCK - 1))
                    nc.scalar.activation(hT_sb[:, j * QW:(j + 1) * QW],
                                         h_ps[:], AF.Gelu,
                                         bias=fc1b_t[:, j:j + 1])
                for t in range(QW // P):
                    i = q * (QW // P) + t
                    o_ps = o_pool.tile([P, 1024], f32, tag="o")
                    for j in range(JB):
                        lhsT = hT_sb[:, j * QW + t * P: j * QW + (t + 1) * P]
                        nc.tensor.matmul(o_ps[:, 0:512], lhsT,
                                         fc2T_sb[:, j * C: j * C + 512],
                                         start=(j == 0), stop=(j == JB - 1))
                        nc.tensor.matmul(o_ps[:, 512:768], lhsT,
                                         fc2T_sb[:, j * C + 512: j * C + C],
                                         start=(j == 0), stop=(j == JB - 1))
                    xre = xio.tile([P, C], f32, tag="xio")
                    nc.sync.dma_start(xre[:], x2s_ap[i * P:(i + 1) * P, :])
                    if skipb2:
                        o2 = lnscr.tile([P, C], f32, tag="o2")
                        nc.vector.scalar_tensor_tensor(
                            o2[:], o_ps[:, 0:C], 1.0, xre[:],
                            ALU.mult, ALU.add)
                    else:
                        o1 = lnscr.tile([P, C], f32, tag="o1")
                        nc.vector.scalar_tensor_tensor(
                            o1[:], o_ps[:, 0:C], 1.0, fc2b_t[:],
                            ALU.mult, ALU.add)
                        o2 = lnscr.tile([P, C], f32, tag="o2")
                        nc.vector.scalar_tensor_tensor(
                            o2[:], o1[:], 1.0, xre[:], ALU.mult, ALU.add)
                    nc.sync.dma_start(out_ap[i * P:(i + 1) * P, :], o2[:])

        hT_pool.release()
        w_pool.release()
        zT_pool.release()


def _build(flags):
    nc = bacc.Bacc("TRN2", target_bir_lowering=False, debug=False, num_devices=8)
    hs = {}
    skip1, skip2, skipb2 = flags
    hs["x"] = nc.declare_dram_parameter("x", [N, C], f32, isOutput=False)
    if not skip1:
        hs["ln1w_b"] = nc.declare_dram_parameter("ln1w_b", [P, C], f32, isOutput=False)
        hs["ln1b_b"] = nc.declare_dram_parameter("ln1b_b", [P, C], f32, isOutput=False)
    if not skip2:
        hs["ln2w_b"] = nc.declare_dram_parameter("ln2w_b", [P, C], f32, isOutput=False)
        hs["ln2b_b"] = nc.declare_dram_parameter("ln2b_b", [P, C], f32, isOutput=False)
    hs["fc1t"] = nc.declare_dram_parameter("fc1t", [C, H], bf16, isOutput=False)
    hs["fc2t"] = nc.declare_dram_parameter("fc2t", [H, C], bf16, isOutput=False)
    hs["fc1b_r"] = nc.declare_dram_parameter("fc1b_r", [P, JB], f32, isOutput=False)
    if not skipb2:
        hs["fc2b_b"] = nc.declare_dram_parameter("fc2b_b", [P, C], f32, isOutput=False)
    hs["expb"] = nc.declare_dram_parameter("expb", [P, 1], f32, isOutput=False)
    hs["identb"] = nc.declare_dram_parameter("identb", [P, P], bf16, isOutput=False)
    hs["out"] = nc.declare_dram_parameter("out", [N, C], f32, isOutput=True)
    with tile.TileContext(nc) as tc:
        _emit(nc, tc, hs, flags)
    nc.compile()
    return nc


def _maybe_install_ntff_hook():
    """Optional: lets BASS_TRACE=1 capture NTFF profiles under axon."""
    try:
        import types
        if "antenv.axon_hooks" in sys.modules:
            return
        import antenv
        mod = types.ModuleType("antenv.axon_hooks")
        _hook = [None]
        mod.set_axon_ntff_profile_hook = lambda h: _hook.__setitem__(0, h)
        mod.get_axon_ntff_profile_hook = lambda: _hook[0]
        sys.modules["antenv.axon_hooks"] = mod
        antenv.axon_hooks = mod
        from trn_agent_boot.trn_boot import _ntff_profile_via_ctypes
        mod.set_axon_ntff_profile_hook(
            _ntff_profile_via_ctypes("/opt/axon/libaxon_pjrt.so"))
    except Exception:
        pass


_last_results = None


def kernel(x, ln1_w, ln1_b, ln2_w, ln2_b, fc1_w, fc1_b, fc2_w, fc2_b):
    global _last_results
    bfl = ml_dtypes.bfloat16
    x = np.asarray(x, dtype=np.float32)
    ln1_w = np.asarray(ln1_w, np.float32)
    ln1_b = np.asarray(ln1_b, np.float32)
    ln2_w = np.asarray(ln2_w, np.float32)
    ln2_b = np.asarray(ln2_b, np.float32)
    fc2_b = np.asarray(fc2_b, np.float32)
    skip1 = bool(np.all(ln1_w == 1.0) and np.all(ln1_b == 0.0))
    skip2 = bool(np.all(ln2_w == 1.0) and np.all(ln2_b == 0.0))
    skipb2 = bool(np.all(fc2_b == 0.0))
    flags = (skip1, skip2, skipb2)
    if flags not in _cache:
        _cache[flags] = _build(flags)
    nc = _cache[flags]

    # Constant softmax shift: SCALE*(sqrt(C)*max|w| + ||b||_2)^2 upper-bounds
    # every score S[n,m] (Cauchy-Schwarz on rows of y = LN(x)*w + b, each of
    # which has ||y_n|| <= sqrt(C)*max|w| + ||b||), so exp never overflows and
    # the shift is row-constant => softmax is exact and E stays symmetric.
    ybound = float(np.sqrt(C) * np.abs(ln1_w).max() + np.linalg.norm(ln1_b))
    expb = np.full((P, 1), -SCALE * ybound * ybound, np.float32)
    prep = {
        "fc1t": np.ascontiguousarray(np.asarray(fc1_w, np.float32).T.astype(bfl)),
        "fc2t": np.ascontiguousarray(np.asarray(fc2_w, np.float32).T.astype(bfl)),
        "fc1b_r": np.ascontiguousarray(
            np.asarray(fc1_b, np.float32).reshape(JB, P).T),
        "expb": expb,
        "identb": np.eye(P, dtype=np.float32).astype(bfl),
    }
    if not skip1:
        prep["ln1w_b"] = np.ascontiguousarray(np.broadcast_to(ln1_w, (P, C)))
        prep["ln1b_b"] = np.ascontiguousarray(np.broadcast_to(ln1_b, (P, C)))
    if not skip2:
        prep["ln2w_b"] = np.ascontiguousarray(np.broadcast_to(ln2_w, (P, C)))
        prep["ln2b_b"] = np.ascontiguousarray(np.broadcast_to(ln2_b, (P, C)))
    if not skipb2:
        prep["fc2b_b"] = np.ascontiguousarray(np.broadcast_to(fc2_b, (P, C)))
    in_maps = [dict(prep, x=np.ascontiguousarray(x[b])) for b in range(B)]

    trace = bool(os.environ.get("BASS_TRACE"))
    if trace:
        _maybe_install_ntff_hook()
    res = run_bass_kernel_spmd(nc, in_maps, list(range(B)), trace=trace)
    _last_results = res
    return np.stack([res.results[b]["out"] for b in range(B)], axis=0)


# revision 14
# speedup vs baseline: 1.5676x; 1.0290x over previous
"""Trainium2 Bass kernel for nn_Block (pre-LN transformer block with dense
self-attention where q=k=v=LN1(x), followed by a GELU MLP).

Sharding: data-parallel over batch B=8 across the 8 NeuronCores (one batch
element per core). Weights are replicated; host-side prep transposes/casts the
MLP weights to bf16 and pre-broadcasts the small LN/bias vectors so the device
kernel does pure compute.

Device algorithm per core (x: [2048, 768] fp32):
  1. LN1 -> y (bf16 natural + transposed via TensorE transpose + ACT copy).
     LN stats via bn_stats/bn_aggr (equal 384-wide chunks); when the LN
     weight/bias inputs are exactly ones/zeros (they are for this problem)
     the scale/shift application is skipped entirely.
  2. E := exp(SCALE*S - c) with a host-computed constant shift
     c = SCALE*(sqrt(C)*max|w1| + ||b1||_2)^2 >= max S (Cauchy-Schwarz on the
     LN-normalized rows), so E is SYMMETRIC and the second attention matmul
     reads E tiles directly as lhsT without transposing the score matrix.
     S = y@y^T via accumulating K=128 matmuls into 512-wide PSUM quarters.
  3. a_unnorm = E @ [y | 1] (ones column makes the softmax denominator Z a
     free extra output column); x2 = x + a_unnorm/Z, streamed to an HBM
     scratch; LN2 fused per block -> zT (TensorE transpose).
  4. hT = gelu(fc1 @ z^T + b1) computed transposed (double-buffered per
     512-column chunk); out = x2 + fc2 @ h^T + b2.
"""

import os
import sys
from contextlib import ExitStack

for _p in ("/opt/trn_rl_repo",):
    if _p not in sys.path:
        sys.path.append(_p)

import numpy as np
import ml_dtypes

import concourse.bass as bass
import concourse.bacc as bacc
import concourse.tile as tile
import concourse.mybir as mybir
from concourse.bass_utils import run_bass_kernel_spmd

f32 = mybir.dt.float32
bf16 = mybir.dt.bfloat16
AF = mybir.ActivationFunctionType
ALU = mybir.AluOpType
AX = mybir.AxisListType

B, N, C, H = 8, 2048, 768, 3072
P = 128
NB = N // P        # 16 row blocks of 128
CCK = C // P       # 6 channel chunks of 128
JB = H // P        # 24 hidden blocks of 128
NQ = 4             # MLP sequence chunks
QW = N // NQ       # 512 columns per MLP chunk
SQ = 4             # S-phase quarters per row block
SW = N // SQ       # 512
YW = C + 4         # y block stride (768 data + ones column + pad)
HEADS = 12
SCALE = 1.0 / float(np.sqrt(C // HEADS))   # 0.125
EPS = 1e-5

_cache = {}


def _ln_normalize(nc, stats, lnscr, xt_ap, w_t, b_t, out_ap, eps_t, skip_wb):
    """out = LN(xt) (*w + b unless skip_wb). out_ap may be bf16."""
    st = stats.tile([P, 12], f32, tag="bn")
    nc.vector.bn_stats(st[:, 0:6], xt_ap[:, 0:384])
    nc.vector.bn_stats(st[:, 6:12], xt_ap[:, 384:768])
    mv = stats.tile([P, 2], f32, tag="mv")
    nc.vector.bn_aggr(mv[:], st[:])
    std = stats.tile([P, 1], f32, tag="std")
    nc.scalar.activation(std[:], mv[:, 1:2], AF.Sqrt, bias=eps_t[:, 0:1])
    rstd = stats.tile([P, 1], f32, tag="rstd")
    nc.vector.reciprocal(rstd[:], std[:])
    negmr = stats.tile([P, 1], f32, tag="negmr")         # -mean*rstd
    nc.vector.tensor_scalar(negmr[:], mv[:, 0:1], rstd[:, 0:1], -1.0,
                            ALU.mult, ALU.mult)
    if skip_wb:
        nc.vector.tensor_scalar(out_ap, xt_ap, rstd[:, 0:1], negmr[:, 0:1],
                                ALU.mult, ALU.add)
    else:
        u = lnscr.tile([P, C], f32, tag="u")
        nc.vector.tensor_scalar(u[:], xt_ap, rstd[:, 0:1], negmr[:, 0:1],
                                ALU.mult, ALU.add)
        v = lnscr.tile([P, C], f32, tag="v")
        nc.vector.scalar_tensor_tensor(v[:], u[:], 1.0, w_t[:],
                                       ALU.mult, ALU.mult)
        nc.vector.scalar_tensor_tensor(out_ap, v[:], 1.0, b_t[:],
                                       ALU.mult, ALU.add)


def _emit(nc, tc, hs, flags):
    skip1, skip2, skipb2 = flags
    ctx = ExitStack()
    with ctx:
        small = ctx.enter_context(tc.tile_pool(name="small", bufs=1))
        stats = ctx.enter_context(tc.tile_pool(name="stats", bufs=4))
        lnscr = ctx.enter_context(tc.tile_pool(name="lnscr", bufs=2))
        xio = ctx.enter_context(tc.tile_pool(name="xio", bufs=3))

        def param(name, shape, tag):
            t = small.tile(shape, f32, tag=tag)
            nc.sync.dma_start(t[:], hs[name].ap())
            return t

        ln1w_t = ln1b_t = ln2w_t = ln2b_t = None
        if not skip1:
            ln1w_t = param("ln1w_b", [P, C], "ln1w")
            ln1b_t = param("ln1b_b", [P, C], "ln1b")
        if not skip2:
            ln2w_t = param("ln2w_b", [P, C], "ln2w")
            ln2b_t = param("ln2b_b", [P, C], "ln2b")
        fc2b_t = None
        if not skipb2:
            fc2b_t = param("fc2b_b", [P, C], "fc2b")
        fc1b_t = param("fc1b_r", [P, JB], "fc1b")
        expb_t = param("expb", [P, 1], "expb")
        identb = small.tile([P, P], bf16, tag="identb")
        nc.sync.dma_start(identb[:], hs["identb"].ap())

        eps_t = small.tile([P, 1], f32, tag="eps")
        nc.vector.memset(eps_t[:], EPS)

        x_ap = hs["x"].ap()
        out_ap = hs["out"].ap()
        x2s = nc.dram_tensor("x2scratch", [N, C], f32)
        x2s_ap = x2s.ap()

        y_pool = tc.alloc_tile_pool(name="ybig", bufs=1)
        y_sb = y_pool.tile([P, NB * YW], bf16, tag="y")
        # ones column at offset C per block (strided memset of pad cols only)
        nc.vector.memset(
            y_sb[:].rearrange("p (i w) -> p i w", w=YW)[:, :, C:YW], 1.0)
        yT_pool = tc.alloc_tile_pool(name="yTbig", bufs=1, side="right")
        yT_sb = yT_pool.tile([P, CCK * N], bf16, tag="yT")

        tp_pool = tc.alloc_tile_pool(name="tpsum", bufs=2, space="PSUM",
                                     side="right")

        # ---- Stage 1: LN1 -> y (bf16) + yT (PE transpose) ----
        for i in range(NB):
            xt = xio.tile([P, C], f32, tag="xio")
            nc.sync.dma_start(xt[:], x_ap[i * P:(i + 1) * P, :])
            ysl = y_sb[:, i * YW: i * YW + C]
            _ln_normalize(nc, stats, lnscr, xt[:], ln1w_t, ln1b_t, ysl,
                          eps_t, skip1)
            for c in range(CCK):
                tp = tp_pool.tile([P, P], bf16, tag="tp")
                nc.tensor.transpose(
                    tp[:], y_sb[:, i * YW + c * P: i * YW + (c + 1) * P],
                    identb[:])
                nc.scalar.copy(
                    yT_sb[:, c * N + i * P: c * N + (i + 1) * P], tp[:])

        # ---- Stage 2: S quarters + Exp -> E (bf16) ----
        # S is symmetric: compute only quarters covering m-blocks >= i
        # (q >= i//4), then mirror the strictly-lower 128x128 tiles via
        # TensorE transpose + DVE copy.
        E_pool = tc.alloc_tile_pool(name="Ebig", bufs=1)
        E_sb = E_pool.tile([P, NB * N], bf16, tag="E")
        with tc.tile_pool(name="spsum", bufs=6, space="PSUM") as sp_pool:
            # Emit quarters in input-availability order: quarter (i, q) needs
            # LN1 tiles <= max(i, 4q+3), so sweep q ascending, i ascending.
            for q in range(SQ):
                for i in range(4 * q + 4) if q < SQ - 1 else range(NB):
                    if q < i // 4:
                        continue
                    s_ps = sp_pool.tile([P, SW], f32, tag="s",
                                        name=f"s_{i}_{q}")
                    for c in range(CCK):
                        nc.tensor.matmul(
                            s_ps[:],
                            yT_sb[:, c * N + i * P: c * N + (i + 1) * P],
                            yT_sb[:, c * N + q * SW: c * N + (q + 1) * SW],
                            start=(c == 0), stop=(c == CCK - 1))
                    nc.scalar.activation(
                        E_sb[:, i * N + q * SW: i * N + (q + 1) * SW],
                        s_ps[:], AF.Exp, bias=expb_t[:, 0:1], scale=SCALE)
            for r in range(NB):
                for m in range(r):
                    tp = tp_pool.tile([P, P], bf16, tag="tp")
                    nc.tensor.transpose(
                        tp[:], E_sb[:, m * N + r * P: m * N + (r + 1) * P],
                        identb[:])
                    nc.vector.tensor_copy(
                        E_sb[:, r * N + m * P: r * N + (m + 1) * P], tp[:])

        # ---- Stage 3 (fused): a|Z = E@[y|1]; x2 = x + a/Z -> HBM; LN2 -> zT
        yT_pool.release()
        zT_pool = tc.alloc_tile_pool(name="zTbig", bufs=1, side="right")
        zT_sb = zT_pool.tile([P, CCK * N], bf16, tag="zT")
        with tc.tile_pool(name="apsum", bufs=3, space="PSUM") as a_pool:
            for i in range(NB):
                a_ps = a_pool.tile([P, 1024], f32, tag="a")
                for j in range(NB):
                    lhsT = E_sb[:, j * N + i * P: j * N + (i + 1) * P]
                    nc.tensor.matmul(a_ps[:, 0:512], lhsT,
                                     y_sb[:, j * YW: j * YW + 512],
                                     start=(j == 0), stop=(j == NB - 1))
                    nc.tensor.matmul(a_ps[:, 512:769], lhsT,
                                     y_sb[:, j * YW + 512: j * YW + C + 1],
                                     start=(j == 0), stop=(j == NB - 1))
                rZ = stats.tile([P, 1], f32, tag="rZ")
                nc.vector.reciprocal(rZ[:], a_ps[:, 768:769])
                xt = xio.tile([P, C], f32, tag="xio")
                nc.sync.dma_start(xt[:], x_ap[i * P:(i + 1) * P, :])
                x2t = lnscr.tile([P, C], f32, tag="x2t")
                nc.vector.scalar_tensor_tensor(
                    x2t[:], a_ps[:, 0:C], rZ[:, 0:1], xt[:],
                    ALU.mult, ALU.add)
                nc.sync.dma_start(x2s_ap[i * P:(i + 1) * P, :], x2t[:])
                znat = lnscr.tile([P, C], bf16, tag="znat")
                _ln_normalize(nc, stats, lnscr, x2t[:], ln2w_t, ln2b_t,
                              znat[:], eps_t, skip2)
                for c in range(CCK):
                    tp = tp_pool.tile([P, P], bf16, tag="tp")
                    nc.tensor.transpose(tp[:], znat[:, c * P:(c + 1) * P],
                                        identb[:])
                    nc.scalar.copy(
                        zT_sb[:, c * N + i * P: c * N + (i + 1) * P], tp[:])

        # ---- Stage 4: MLP ----
        E_pool.release()
        y_pool.release()
        tp_pool.release()
        w_pool = tc.alloc_tile_pool(name="wbig", bufs=1)
        fc1T_sb = w_pool.tile([P, CCK * H], bf16, tag="fc1T")
        fc2T_sb = w_pool.tile([P, JB * C], bf16, tag="fc2T")
        for c in range(CCK):
            nc.sync.dma_start(fc1T_sb[:, c * H:(c + 1) * H],
                              hs["fc1t"].ap()[c * P:(c + 1) * P, :])
        for j in range(JB):
            nc.sync.dma_start(fc2T_sb[:, j * C:(j + 1) * C],
                              hs["fc2t"].ap()[j * P:(j + 1) * P, :])

        hT_pool = tc.alloc_tile_pool(name="hTbig", bufs=2)
        with tc.tile_pool(name="hpsum", bufs=4, space="PSUM") as h_pool, \
             tc.tile_pool(name="opsum", bufs=2, space="PSUM") as o_pool:
            for q in range(NQ):
                hT_sb = hT_pool.tile([P, JB * QW], bf16, tag="hT")
                for j in range(JB):
                    h_ps = h_pool.tile([P, QW], f32, tag="h")
                    for c in range(CCK):
                        nc.tensor.matmul(
                            h_ps[:],
                            fc1T_sb[:, c * H + j * P: c * H + (j + 1) * P],
                            zT_sb[:, c * N + q * QW: c * N + (q + 1) * QW],
                            start=(c == 0), stop=(c == CCK - 1))
                    nc.scalar.activation(hT_sb[:, j * QW:(j + 1) * QW],
                                         h_ps[:], AF.Gelu,
                                         bias=fc1b_t[:, j:j + 1])
                for t in range(QW // P):
                    i = q * (QW // P) + t
                    o_ps = o_pool.tile([P, 1024], f32, tag="o")
                    for j in range(JB):
                        lhsT = hT_sb[:, j * QW + t * P: j * QW + (t + 1) * P]
                        nc.tensor.matmul(o_ps[:, 0:512], lhsT,
                                         fc2T_sb[:, j * C: j * C + 512],
                                         start=(j == 0), stop=(j == JB - 1))
                        nc.tensor.matmul(o_ps[:, 512:768], lhsT,
                                         fc2T_sb[:, j * C + 512: j * C + C],
                                         start=(j == 0), stop=(j == JB - 1))
                    xre = xio.tile([P, C], f32, tag="xio")
                    nc.sync.dma_start(xre[:], x2s_ap[i * P:(i + 1) * P, :])
                    if skipb2:
                        o2 = lnscr.tile([P, C], f32, tag="o2")
                        nc.vector.scalar_tensor_tensor(
                            o2[:], o_ps[:, 0:C], 1.0, xre[:],
                            ALU.mult, ALU.add)
                    else:
                        o1 = lnscr.tile([P, C], f32, tag="o1")
                        nc.vector.scalar_tensor_tensor(
                            o1[:], o_ps[:, 0:C], 1.0, fc2b_t[:],
                            ALU.mult, ALU.add)
                        o2 = lnscr.tile([P, C], f32, tag="o2")
                        nc.vector.scalar_tensor_tensor(
                            o2[:], o1[:], 1.0, xre[:], ALU.mult, ALU.add)
                    nc.sync.dma_start(out_ap[i * P:(i + 1) * P, :], o2[:])

        hT_pool.release()
        w_pool.release()
        zT_pool.release()


def _build(flags):
    nc = bacc.Bacc("TRN2", target_bir_lowering=False, debug=False, num_devices=8)
    hs = {}
    skip1, skip2, skipb2 = flags
    hs["x"] = nc.declare_dram_parameter("x", [N, C], f32, isOutput=False)
    if not skip1:
        hs["ln1w_b"] = nc.declare_dram_parameter("ln1w_b", [P, C], f32, isOutput=False)
        hs["ln1b_b"] = nc.declare_dram_parameter("ln1b_b", [P, C], f32, isOutput=False)
    if not skip2:
        hs["ln2w_b"] = nc.declare_dram_parameter("ln2w_b", [P, C], f32, isOutput=False)
        hs["ln2b_b"] = nc.declare_dram_parameter("ln2b_b", [P, C], f32, isOutput=False)
    hs["fc1t"] = nc.declare_dram_parameter("fc1t", [C, H], bf16, isOutput=False)
    hs["fc2t"] = nc.declare_dram_parameter("fc2t", [H, C], bf16, isOutput=False)
    hs["fc1b_r"] = nc.declare_dram_parameter("fc1b_r", [P, JB], f32, isOutput=False)
    if not skipb2:
        hs["fc2b_b"] = nc.declare_dram_parameter("fc2b_b", [P, C], f32, isOutput=False)
    hs["expb"] = nc.declare_dram_parameter("expb", [P, 1], f32, isOutput=False)
    hs["identb"] = nc.declare_dram_parameter("identb", [P, P], bf16, isOutput=False)
    hs["out"] = nc.declare_dram_parameter("out", [N, C], f32, isOutput=True)
    with tile.TileContext(nc) as tc:
        _emit(nc, tc, hs, flags)
    nc.compile()
    return nc


def _maybe_install_ntff_hook():
    """Optional: lets BASS_TRACE=1 capture NTFF profiles under axon."""
    try:
        import types
        if "antenv.axon_hooks" in sys.modules:
            return
        import antenv
        mod = types.ModuleType("antenv.axon_hooks")
        _hook = [None]
        mod.set_axon_ntff_profile_hook = lambda h: _hook.__setitem__(0, h)
        mod.get_axon_ntff_profile_hook = lambda: _hook[0]
        sys.modules["antenv.axon_hooks"] = mod
        antenv.axon_hooks = mod
        from trn_agent_boot.trn_boot import _ntff_profile_via_ctypes
        mod.set_axon_ntff_profile_hook(
            _ntff_profile_via_ctypes("/opt/axon/libaxon_pjrt.so"))
    except Exception:
        pass


_last_results = None


def kernel(x, ln1_w, ln1_b, ln2_w, ln2_b, fc1_w, fc1_b, fc2_w, fc2_b):
    global _last_results
    bfl = ml_dtypes.bfloat16
    x = np.asarray(x, dtype=np.float32)
    ln1_w = np.asarray(ln1_w, np.float32)
    ln1_b = np.asarray(ln1_b, np.float32)
    ln2_w = np.asarray(ln2_w, np.float32)
    ln2_b = np.asarray(ln2_b, np.float32)
    fc2_b = np.asarray(fc2_b, np.float32)
    skip1 = bool(np.all(ln1_w == 1.0) and np.all(ln1_b == 0.0))
    skip2 = bool(np.all(ln2_w == 1.0) and np.all(ln2_b == 0.0))
    skipb2 = bool(np.all(fc2_b == 0.0))
    flags = (skip1, skip2, skipb2)
    if flags not in _cache:
        _cache[flags] = _build(flags)
    nc = _cache[flags]

    # Constant softmax shift: SCALE*(sqrt(C)*max|w| + ||b||_2)^2 upper-bounds
    # every score S[n,m] (Cauchy-Schwarz on rows of y = LN(x)*w + b, each of
    # which has ||y_n|| <= sqrt(C)*max|w| + ||b||), so exp never overflows and
    # the shift is row-constant => softmax is exact and E stays symmetric.
    ybound = float(np.sqrt(C) * np.abs(ln1_w).max() + np.linalg.norm(ln1_b))
    expb = np.full((P, 1), -SCALE * ybound * ybound, np.float32)
    prep = {
        "fc1t": np.ascontiguousarray(np.asarray(fc1_w, np.float32).T.astype(bfl)),
        "fc2t": np.ascontiguousarray(np.asarray(fc2_w, np.float32).T.astype(bfl)),
        "fc1b_r": np.ascontiguousarray(
            np.asarray(fc1_b, np.float32).reshape(JB, P).T),
        "expb": expb,
        "identb": np.eye(P, dtype=np.float32).astype(bfl),
    }
    if not skip1:
        prep["ln1w_b"] = np.ascontiguousarray(np.broadcast_to(ln1_w, (P, C)))
        prep["ln1b_b"] = np.ascontiguousarray(np.broadcast_to(ln1_b, (P, C)))
    if not skip2:
        prep["ln2w_b"] = np.ascontiguousarray(np.broadcast_to(ln2_w, (P, C)))
        prep["ln2b_b"] = np.ascontiguousarray(np.broadcast_to(ln2_b, (P, C)))
    if not skipb2:
        prep["fc2b_b"] = np.ascontiguousarray(np.broadcast_to(fc2_b, (P, C)))
    in_maps = [dict(prep, x=np.ascontiguousarray(x[b])) for b in range(B)]

    trace = bool(os.environ.get("BASS_TRACE"))
    if trace:
        _maybe_install_ntff_hook()
    res = run_bass_kernel_spmd(nc, in_maps, list(range(B)), trace=trace)
    _last_results = res
    return np.stack([res.results[b]["out"] for b in range(B)], axis=0)


# revision 18
# speedup vs baseline: 1.6162x; 1.0310x over previous
"""Trainium2 Bass kernel for nn_Block (pre-LN transformer block with dense
self-attention where q=k=v=LN1(x), followed by a GELU MLP).

Sharding: data-parallel over batch B=8 across the 8 NeuronCores (one batch
element per core). Weights are replicated; host-side prep transposes/casts the
MLP weights to bf16 and pre-broadcasts the small LN/bias vectors so the device
kernel does pure compute.

Device algorithm per core (x: [2048, 768] fp32):
  1. LN1 -> y (bf16 natural + transposed via TensorE transpose + ACT copy).
     LN stats via bn_stats/bn_aggr (equal 384-wide chunks); when the LN
     weight/bias inputs are exactly ones/zeros (they are for this problem)
     the scale/shift application is skipped entirely.
  2. E := exp(SCALE*S - c) with a host-computed constant shift
     c = SCALE*(sqrt(C)*max|w1| + ||b1||_2)^2 >= max S (Cauchy-Schwarz on the
     LN-normalized rows), so E is SYMMETRIC and the second attention matmul
     reads E tiles directly as lhsT without transposing the score matrix.
     S = y@y^T via accumulating K=128 matmuls into 512-wide PSUM quarters.
  3. a_unnorm = E @ [y | 1] (ones column makes the softmax denominator Z a
     free extra output column); x2 = x + a_unnorm/Z, streamed to an HBM
     scratch; LN2 fused per block -> zT (TensorE transpose).
  4. hT = gelu(fc1 @ z^T + b1) computed transposed (double-buffered per
     512-column chunk); out = x2 + fc2 @ h^T + b2.
"""

import os
import sys
from contextlib import ExitStack

for _p in ("/opt/trn_rl_repo",):
    if _p not in sys.path:
        sys.path.append(_p)

import numpy as np
import ml_dtypes

import concourse.bass as bass
import concourse.bacc as bacc
import concourse.tile as tile
import concourse.mybir as mybir
from concourse.bass_utils import run_bass_kernel_spmd

f32 = mybir.dt.float32
bf16 = mybir.dt.bfloat16
AF = mybir.ActivationFunctionType
ALU = mybir.AluOpType
AX = mybir.AxisListType

B, N, C, H = 8, 2048, 768, 3072
P = 128
NB = N // P        # 16 row blocks of 128
CCK = C // P       # 6 channel chunks of 128
JB = H // P        # 24 hidden blocks of 128
NQ = 4             # MLP sequence chunks
QW = N // NQ       # 512 columns per MLP chunk
SQ = 4             # S-phase quarters per row block
SW = N // SQ       # 512
YW = C + 4         # y block stride (768 data + ones column + pad)
HEADS = 12
SCALE = 1.0 / float(np.sqrt(C // HEADS))   # 0.125
EPS = 1e-5

_cache = {}


def _ln_normalize(nc, stats, lnscr, xt_ap, w_t, b_t, out_ap, eps_t, skip_wb):
    """out = LN(xt) (*w + b unless skip_wb). out_ap may be bf16."""
    st = stats.tile([P, 12], f32, tag="bn")
    nc.vector.bn_stats(st[:, 0:6], xt_ap[:, 0:384])
    nc.vector.bn_stats(st[:, 6:12], xt_ap[:, 384:768])
    mv = stats.tile([P, 2], f32, tag="mv")
    nc.vector.bn_aggr(mv[:], st[:])
    std = stats.tile([P, 1], f32, tag="std")
    nc.scalar.activation(std[:], mv[:, 1:2], AF.Sqrt, bias=eps_t[:, 0:1])
    rstd = stats.tile([P, 1], f32, tag="rstd")
    nc.vector.reciprocal(rstd[:], std[:])
    negmr = stats.tile([P, 1], f32, tag="negmr")         # -mean*rstd
    nc.vector.tensor_scalar(negmr[:], mv[:, 0:1], rstd[:, 0:1], -1.0,
                            ALU.mult, ALU.mult)
    if skip_wb:
        nc.vector.tensor_scalar(out_ap, xt_ap, rstd[:, 0:1], negmr[:, 0:1],
                                ALU.mult, ALU.add)
    else:
        u = lnscr.tile([P, C], f32, tag="u")
        nc.vector.tensor_scalar(u[:], xt_ap, rstd[:, 0:1], negmr[:, 0:1],
                                ALU.mult, ALU.add)
        v = lnscr.tile([P, C], f32, tag="v")
        nc.vector.scalar_tensor_tensor(v[:], u[:], 1.0, w_t[:],
                                       ALU.mult, ALU.mult)
        nc.vector.scalar_tensor_tensor(out_ap, v[:], 1.0, b_t[:],
                                       ALU.mult, ALU.add)


def _emit(nc, tc, hs, flags):
    skip1, skip2, skipb2 = flags
    ctx = ExitStack()
    with ctx:
        small = ctx.enter_context(tc.tile_pool(name="small", bufs=1))
        stats = ctx.enter_context(tc.tile_pool(name="stats", bufs=6))
        lnscr = ctx.enter_context(tc.tile_pool(name="lnscr", bufs=3))
        xio = ctx.enter_context(tc.tile_pool(name="xio", bufs=4))

        def param(name, shape, tag):
            t = small.tile(shape, f32, tag=tag)
            nc.sync.dma_start(t[:], hs[name].ap())
            return t

        ln1w_t = ln1b_t = ln2w_t = ln2b_t = None
        if not skip1:
            ln1w_t = param("ln1w_b", [P, C], "ln1w")
            ln1b_t = param("ln1b_b", [P, C], "ln1b")
        if not skip2:
            ln2w_t = param("ln2w_b", [P, C], "ln2w")
            ln2b_t = param("ln2b_b", [P, C], "ln2b")
        fc2b_t = None
        if not skipb2:
            fc2b_t = param("fc2b_b", [P, C], "fc2b")
        fc1b_t = param("fc1b_r", [P, JB], "fc1b")
        expb_t = param("expb", [P, 1], "expb")
        identb = small.tile([P, P], bf16, tag="identb")
        nc.sync.dma_start(identb[:], hs["identb"].ap())

        eps_t = small.tile([P, 1], f32, tag="eps")
        nc.vector.memset(eps_t[:], EPS)

        x_ap = hs["x"].ap()
        out_ap = hs["out"].ap()
        x2s = nc.dram_tensor("x2scratch", [N, C], f32)
        x2s_ap = x2s.ap()

        y_pool = tc.alloc_tile_pool(name="ybig", bufs=1)
        y_sb = y_pool.tile([P, NB * YW], bf16, tag="y")
        # ones column at offset C per block (strided memset of pad cols only)
        nc.vector.memset(
            y_sb[:].rearrange("p (i w) -> p i w", w=YW)[:, :, C:YW], 1.0)
        yT_pool = tc.alloc_tile_pool(name="yTbig", bufs=1, side="right")
        yT_sb = yT_pool.tile([P, CCK * N], bf16, tag="yT")

        tp_pool = tc.alloc_tile_pool(name="tpsum", bufs=2, space="PSUM",
                                     side="right")

        # ---- Stage 1: LN1 -> y (bf16) + yT (PE transpose) ----
        for i in range(NB):
            xt = xio.tile([P, C], f32, tag="xio")
            nc.sync.dma_start(xt[:], x_ap[i * P:(i + 1) * P, :])
            ysl = y_sb[:, i * YW: i * YW + C]
            _ln_normalize(nc, stats, lnscr, xt[:], ln1w_t, ln1b_t, ysl,
                          eps_t, skip1)
            for c in range(CCK):
                tp = tp_pool.tile([P, P], bf16, tag="tp")
                nc.tensor.transpose(
                    tp[:], y_sb[:, i * YW + c * P: i * YW + (c + 1) * P],
                    identb[:])
                nc.scalar.copy(
                    yT_sb[:, c * N + i * P: c * N + (i + 1) * P], tp[:])

        # ---- Stage 2: S quarters + Exp -> E (bf16) ----
        # S is symmetric: compute only quarters covering m-blocks >= i
        # (q >= i//4), then mirror the strictly-lower 128x128 tiles via
        # TensorE transpose + DVE copy.
        E_pool = tc.alloc_tile_pool(name="Ebig", bufs=1)
        E_sb = E_pool.tile([P, NB * N], bf16, tag="E")
        with tc.tile_pool(name="spsum", bufs=6, space="PSUM") as sp_pool:
            # Emit quarters in input-availability order: quarter (i, q) needs
            # LN1 tiles <= max(i, 4q+3), so sweep q ascending, i ascending.
            for q in range(SQ):
                for i in range(4 * q + 4) if q < SQ - 1 else range(NB):
                    if q < i // 4:
                        continue
                    s_ps = sp_pool.tile([P, SW], f32, tag="s",
                                        name=f"s_{i}_{q}")
                    for c in range(CCK):
                        nc.tensor.matmul(
                            s_ps[:],
                            yT_sb[:, c * N + i * P: c * N + (i + 1) * P],
                            yT_sb[:, c * N + q * SW: c * N + (q + 1) * SW],
                            start=(c == 0), stop=(c == CCK - 1))
                    nc.scalar.activation(
                        E_sb[:, i * N + q * SW: i * N + (q + 1) * SW],
                        s_ps[:], AF.Exp, bias=expb_t[:, 0:1], scale=SCALE)
            for r in range(NB):
                for m in range(r):
                    tp = tp_pool.tile([P, P], bf16, tag="tp")
                    nc.tensor.transpose(
                        tp[:], E_sb[:, m * N + r * P: m * N + (r + 1) * P],
                        identb[:])
                    nc.vector.tensor_copy(
                        E_sb[:, r * N + m * P: r * N + (m + 1) * P], tp[:])

        # ---- Stage 3 (fused): a|Z = E@[y|1]; x2 = x + a/Z -> HBM; LN2 -> zT
        yT_pool.release()
        zT_pool = tc.alloc_tile_pool(name="zTbig", bufs=1, side="right")
        zT_sb = zT_pool.tile([P, CCK * N], bf16, tag="zT")
        # fc1T on the right stack so its loads overlap the a-phase (the left
        # stack still holds E until the MLP starts).
        w1_pool = tc.alloc_tile_pool(name="w1big", bufs=1, side="right")
        fc1T_sb = w1_pool.tile([P, CCK * H], bf16, tag="fc1T")
        for c in range(CCK):
            nc.sync.dma_start(fc1T_sb[:, c * H:(c + 1) * H],
                              hs["fc1t"].ap()[c * P:(c + 1) * P, :])
        with tc.tile_pool(name="apsum", bufs=3, space="PSUM") as a_pool:
            for i in range(NB):
                a_ps = a_pool.tile([P, 1024], f32, tag="a")
                for j in range(NB):
                    lhsT = E_sb[:, j * N + i * P: j * N + (i + 1) * P]
                    nc.tensor.matmul(a_ps[:, 0:512], lhsT,
                                     y_sb[:, j * YW: j * YW + 512],
                                     start=(j == 0), stop=(j == NB - 1))
                    nc.tensor.matmul(a_ps[:, 512:769], lhsT,
                                     y_sb[:, j * YW + 512: j * YW + C + 1],
                                     start=(j == 0), stop=(j == NB - 1))
                rZ = stats.tile([P, 1], f32, tag="rZ")
                nc.vector.reciprocal(rZ[:], a_ps[:, 768:769])
                xt = xio.tile([P, C], f32, tag="xio")
                nc.sync.dma_start(xt[:], x_ap[i * P:(i + 1) * P, :])
                x2t = lnscr.tile([P, C], f32, tag="x2t")
                nc.vector.scalar_tensor_tensor(
                    x2t[:], a_ps[:, 0:C], rZ[:, 0:1], xt[:],
                    ALU.mult, ALU.add)
                nc.sync.dma_start(x2s_ap[i * P:(i + 1) * P, :], x2t[:])
                znat = lnscr.tile([P, C], bf16, tag="znat")
                _ln_normalize(nc, stats, lnscr, x2t[:], ln2w_t, ln2b_t,
                              znat[:], eps_t, skip2)
                for c in range(CCK):
                    tp = tp_pool.tile([P, P], bf16, tag="tp")
                    nc.tensor.transpose(tp[:], znat[:, c * P:(c + 1) * P],
                                        identb[:])
                    nc.scalar.copy(
                        zT_sb[:, c * N + i * P: c * N + (i + 1) * P], tp[:])

        # ---- Stage 4: MLP ----
        E_pool.release()
        y_pool.release()
        tp_pool.release()
        w_pool = tc.alloc_tile_pool(name="wbig", bufs=1)
        fc2T_sb = w_pool.tile([P, JB * C], bf16, tag="fc2T")
        for j in range(JB):
            nc.sync.dma_start(fc2T_sb[:, j * C:(j + 1) * C],
                              hs["fc2t"].ap()[j * P:(j + 1) * P, :])

        hT_pool = tc.alloc_tile_pool(name="hTbig", bufs=2)
        with tc.tile_pool(name="hpsum", bufs=4, space="PSUM") as h_pool, \
             tc.tile_pool(name="opsum", bufs=2, space="PSUM") as o_pool:
            for q in range(NQ):
                hT_sb = hT_pool.tile([P, JB * QW], bf16, tag="hT")
                for j in range(JB):
                    h_ps = h_pool.tile([P, QW], f32, tag="h")
                    for c in range(CCK):
                        nc.tensor.matmul(
                            h_ps[:],
                            fc1T_sb[:, c * H + j * P: c * H + (j + 1) * P],
                            zT_sb[:, c * N + q * QW: c * N + (q + 1) * QW],
                            start=(c == 0), stop=(c == CCK - 1))
                    nc.scalar.activation(hT_sb[:, j * QW:(j + 1) * QW],
                                         h_ps[:], AF.Gelu,
                                         bias=fc1b_t[:, j:j + 1])
                for t in range(QW // P):
                    i = q * (QW // P) + t
                    o_ps = o_pool.tile([P, 1024], f32, tag="o")
                    for j in range(JB):
                        lhsT = hT_sb[:, j * QW + t * P: j * QW + (t + 1) * P]
                        nc.tensor.matmul(o_ps[:, 0:512], lhsT,
                                         fc2T_sb[:, j * C: j * C + 512],
                                         start=(j == 0), stop=(j == JB - 1))
                        nc.tensor.matmul(o_ps[:, 512:768], lhsT,
                                         fc2T_sb[:, j * C + 512: j * C + C],
                                         start=(j == 0), stop=(j == JB - 1))
                    xre = xio.tile([P, C], f32, tag="xio")
                    nc.sync.dma_start(xre[:], x2s_ap[i * P:(i + 1) * P, :])
                    if skipb2:
                        o2 = lnscr.tile([P, C], f32, tag="o2")
                        nc.vector.scalar_tensor_tensor(
                            o2[:], o_ps[:, 0:C], 1.0, xre[:],
                            ALU.mult, ALU.add)
                    else:
                        o1 = lnscr.tile([P, C], f32, tag="o1")
                        nc.vector.scalar_tensor_tensor(
                            o1[:], o_ps[:, 0:C], 1.0, fc2b_t[:],
                            ALU.mult, ALU.add)
                        o2 = lnscr.tile([P, C], f32, tag="o2")
                        nc.vector.scalar_tensor_tensor(
                            o2[:], o1[:], 1.0, xre[:], ALU.mult, ALU.add)
                    nc.sync.dma_start(out_ap[i * P:(i + 1) * P, :], o2[:])

        hT_pool.release()
        w_pool.release()
        w1_pool.release()
        zT_pool.release()


def _build(flags):
    nc = bacc.Bacc("TRN2", target_bir_lowering=False, debug=False, num_devices=8)
    hs = {}
    skip1, skip2, skipb2 = flags
    hs["x"] = nc.declare_dram_parameter("x", [N, C], f32, isOutput=False)
    if not skip1:
        hs["ln1w_b"] = nc.declare_dram_parameter("ln1w_b", [P, C], f32, isOutput=False)
        hs["ln1b_b"] = nc.declare_dram_parameter("ln1b_b", [P, C], f32, isOutput=False)
    if not skip2:
        hs["ln2w_b"] = nc.declare_dram_parameter("ln2w_b", [P, C], f32, isOutput=False)
        hs["ln2b_b"] = nc.declare_dram_parameter("ln2b_b", [P, C], f32, isOutput=False)
    hs["fc1t"] = nc.declare_dram_parameter("fc1t", [C, H], bf16, isOutput=False)
    hs["fc2t"] = nc.declare_dram_parameter("fc2t", [H, C], bf16, isOutput=False)
    hs["fc1b_r"] = nc.declare_dram_parameter("fc1b_r", [P, JB], f32, isOutput=False)
    if not skipb2:
        hs["fc2b_b"] = nc.declare_dram_parameter("fc2b_b", [P, C], f32, isOutput=False)
    hs["expb"] = nc.declare_dram_parameter("expb", [P, 1], f32, isOutput=False)
    hs["identb"] = nc.declare_dram_parameter("identb", [P, P], bf16, isOutput=False)
    hs["out"] = nc.declare_dram_parameter("out", [N, C], f32, isOutput=True)
    with tile.TileContext(nc) as tc:
        _emit(nc, tc, hs, flags)
    nc.compile()
    return nc


def _maybe_install_ntff_hook():
    """Optional: lets BASS_TRACE=1 capture NTFF profiles under axon."""
    try:
        import types
        if "antenv.axon_hooks" in sys.modules:
            return
        import antenv
        mod = types.ModuleType("antenv.axon_hooks")
        _hook = [None]
        mod.set_axon_ntff_profile_hook = lambda h: _hook.__setitem__(0, h)
        mod.get_axon_ntff_profile_hook = lambda: _hook[0]
        sys.modules["antenv.axon_hooks"] = mod
        antenv.axon_hooks = mod
        from trn_agent_boot.trn_boot import _ntff_profile_via_ctypes
        mod.set_axon_ntff_profile_hook(
            _ntff_profile_via_ctypes("/opt/axon/libaxon_pjrt.so"))
    except Exception:
        pass


_last_results = None


def kernel(x, ln1_w, ln1_b, ln2_w, ln2_b, fc1_w, fc1_b, fc2_w, fc2_b):
    global _last_results
    bfl = ml_dtypes.bfloat16
    x = np.asarray(x, dtype=np.float32)
    ln1_w = np.asarray(ln1_w, np.float32)
    ln1_b = np.asarray(ln1_b, np.float32)
    ln2_w = np.asarray(ln2_w, np.float32)
    ln2_b = np.asarray(ln2_b, np.float32)
    fc2_b = np.asarray(fc2_b, np.float32)
    skip1 = bool(np.all(ln1_w == 1.0) and np.all(ln1_b == 0.0))
    skip2 = bool(np.all(ln2_w == 1.0) and np.all(ln2_b == 0.0))
    skipb2 = bool(np.all(fc2_b == 0.0))
    flags = (skip1, skip2, skipb2)
    if flags not in _cache:
        _cache[flags] = _build(flags)
    nc = _cache[flags]

    # Constant softmax shift: SCALE*(sqrt(C)*max|w| + ||b||_2)^2 upper-bounds
    # every score S[n,m] (Cauchy-Schwarz on rows of y = LN(x)*w + b, each of
    # which has ||y_n|| <= sqrt(C)*max|w| + ||b||), so exp never overflows and
    # the shift is row-constant => softmax is exact and E stays symmetric.
    ybound = float(np.sqrt(C) * np.abs(ln1_w).max() + np.linalg.norm(ln1_b))
    expb = np.full((P, 1), -SCALE * ybound * ybound, np.float32)
    prep = {
        "fc1t": np.ascontiguousarray(np.asarray(fc1_w, np.float32).T.astype(bfl)),
        "fc2t": np.ascontiguousarray(np.asarray(fc2_w, np.float32).T.astype(bfl)),
        "fc1b_r": np.ascontiguousarray(
            np.asarray(fc1_b, np.float32).reshape(JB, P).T),
        "expb": expb,
        "identb": np.eye(P, dtype=np.float32).astype(bfl),
    }
    if not skip1:
        prep["ln1w_b"] = np.ascontiguousarray(np.broadcast_to(ln1_w, (P, C)))
        prep["ln1b_b"] = np.ascontiguousarray(np.broadcast_to(ln1_b, (P, C)))
    if not skip2:
        prep["ln2w_b"] = np.ascontiguousarray(np.broadcast_to(ln2_w, (P, C)))
        prep["ln2b_b"] = np.ascontiguousarray(np.broadcast_to(ln2_b, (P, C)))
    if not skipb2:
        prep["fc2b_b"] = np.ascontiguousarray(np.broadcast_to(fc2_b, (P, C)))
    in_maps = [dict(prep, x=np.ascontiguousarray(x[b])) for b in range(B)]

    trace = bool(os.environ.get("BASS_TRACE"))
    if trace:
        _maybe_install_ntff_hook()
    res = run_bass_kernel_spmd(nc, in_maps, list(range(B)), trace=trace)
    _last_results = res
    return np.stack([res.results[b]["out"] for b in range(B)], axis=0)


# revision 19
# speedup vs baseline: 1.6201x; 1.0025x over previous
"""Trainium2 Bass kernel for nn_Block (pre-LN transformer block with dense
self-attention where q=k=v=LN1(x), followed by a GELU MLP).

Sharding: data-parallel over batch B=8 across the 8 NeuronCores (one batch
element per core). Weights are replicated; host-side prep transposes/casts the
MLP weights to bf16 and pre-broadcasts the small LN/bias vectors so the device
kernel does pure compute.

Device algorithm per core (x: [2048, 768] fp32):
  1. LN1 -> y (bf16 natural + transposed via TensorE transpose + ACT copy).
     LN stats via bn_stats/bn_aggr (equal 384-wide chunks); when the LN
     weight/bias inputs are exactly ones/zeros (they are for this problem)
     the scale/shift application is skipped entirely.
  2. E := exp(SCALE*S - c) with a host-computed constant shift
     c = SCALE*(sqrt(C)*max|w1| + ||b1||_2)^2 >= max S (Cauchy-Schwarz on the
     LN-normalized rows), so E is SYMMETRIC and the second attention matmul
     reads E tiles directly as lhsT without transposing the score matrix.
     S = y@y^T via accumulating K=128 matmuls into 512-wide PSUM quarters.
  3. a_unnorm = E @ [y | 1] (ones column makes the softmax denominator Z a
     free extra output column); x2 = x + a_unnorm/Z, streamed to an HBM
     scratch; LN2 fused per block -> zT (TensorE transpose).
  4. hT = gelu(fc1 @ z^T + b1) computed transposed (double-buffered per
     512-column chunk); out = x2 + fc2 @ h^T + b2.
"""

import os
import sys
from contextlib import ExitStack

for _p in ("/opt/trn_rl_repo",):
    if _p not in sys.path:
        sys.path.append(_p)

import numpy as np
import ml_dtypes

import concourse.bass as bass
import concourse.bacc as bacc
import concourse.tile as tile
import concourse.mybir as mybir
from concourse.bass_utils import run_bass_kernel_spmd

f32 = mybir.dt.float32
bf16 = mybir.dt.bfloat16
AF = mybir.ActivationFunctionType
ALU = mybir.AluOpType
AX = mybir.AxisListType

B, N, C, H = 8, 2048, 768, 3072
P = 128
NB = N // P        # 16 row blocks of 128
CCK = C // P       # 6 channel chunks of 128
JB = H // P        # 24 hidden blocks of 128
NQ = 4             # MLP sequence chunks
QW = N // NQ       # 512 columns per MLP chunk
SQ = 4             # S-phase quarters per row block
SW = N // SQ       # 512
YW = C + 4         # y block stride (768 data + ones column + pad)
HEADS = 12
SCALE = 1.0 / float(np.sqrt(C // HEADS))   # 0.125
EPS = 1e-5

_cache = {}


def _ln_normalize(nc, stats, lnscr, xt_ap, w_t, b_t, out_ap, eps_t, skip_wb):
    """out = LN(xt) (*w + b unless skip_wb). out_ap may be bf16."""
    st = stats.tile([P, 12], f32, tag="bn")
    nc.vector.bn_stats(st[:, 0:6], xt_ap[:, 0:384])
    nc.vector.bn_stats(st[:, 6:12], xt_ap[:, 384:768])
    mv = stats.tile([P, 2], f32, tag="mv")
    nc.vector.bn_aggr(mv[:], st[:])
    std = stats.tile([P, 1], f32, tag="std")
    nc.scalar.activation(std[:], mv[:, 1:2], AF.Sqrt, bias=eps_t[:, 0:1])
    rstd = stats.tile([P, 1], f32, tag="rstd")
    nc.vector.reciprocal(rstd[:], std[:])
    negmr = stats.tile([P, 1], f32, tag="negmr")         # -mean*rstd
    nc.vector.tensor_scalar(negmr[:], mv[:, 0:1], rstd[:, 0:1], -1.0,
                            ALU.mult, ALU.mult)
    if skip_wb:
        nc.vector.tensor_scalar(out_ap, xt_ap, rstd[:, 0:1], negmr[:, 0:1],
                                ALU.mult, ALU.add)
    else:
        u = lnscr.tile([P, C], f32, tag="u")
        nc.vector.tensor_scalar(u[:], xt_ap, rstd[:, 0:1], negmr[:, 0:1],
                                ALU.mult, ALU.add)
        v = lnscr.tile([P, C], f32, tag="v")
        nc.vector.scalar_tensor_tensor(v[:], u[:], 1.0, w_t[:],
                                       ALU.mult, ALU.mult)
        nc.vector.scalar_tensor_tensor(out_ap, v[:], 1.0, b_t[:],
                                       ALU.mult, ALU.add)


def _emit(nc, tc, hs, flags):
    skip1, skip2, skipb2 = flags
    ctx = ExitStack()
    with ctx:
        small = ctx.enter_context(tc.tile_pool(name="small", bufs=1))
        stats = ctx.enter_context(tc.tile_pool(name="stats", bufs=6))
        lnscr = ctx.enter_context(tc.tile_pool(name="lnscr", bufs=3))
        xio = ctx.enter_context(tc.tile_pool(name="xio", bufs=4))

        def param(name, shape, tag):
            t = small.tile(shape, f32, tag=tag)
            nc.sync.dma_start(t[:], hs[name].ap())
            return t

        ln1w_t = ln1b_t = ln2w_t = ln2b_t = None
        if not skip1:
            ln1w_t = param("ln1w_b", [P, C], "ln1w")
            ln1b_t = param("ln1b_b", [P, C], "ln1b")
        if not skip2:
            ln2w_t = param("ln2w_b", [P, C], "ln2w")
            ln2b_t = param("ln2b_b", [P, C], "ln2b")
        fc2b_t = None
        if not skipb2:
            fc2b_t = param("fc2b_b", [P, C], "fc2b")
        fc1b_t = param("fc1b_r", [P, JB], "fc1b")
        expb_t = param("expb", [P, 1], "expb")
        identb = small.tile([P, P], bf16, tag="identb")
        nc.sync.dma_start(identb[:], hs["identb"].ap())

        eps_t = small.tile([P, 1], f32, tag="eps")
        nc.vector.memset(eps_t[:], EPS)

        x_ap = hs["x"].ap()
        out_ap = hs["out"].ap()
        x2s = nc.dram_tensor("x2scratch", [N, C], f32)
        x2s_ap = x2s.ap()

        y_pool = tc.alloc_tile_pool(name="ybig", bufs=1)
        y_sb = y_pool.tile([P, NB * YW], bf16, tag="y")
        # ones column at offset C per block (strided memset of pad cols only)
        nc.vector.memset(
            y_sb[:].rearrange("p (i w) -> p i w", w=YW)[:, :, C:YW], 1.0)
        yT_pool = tc.alloc_tile_pool(name="yTbig", bufs=1, side="right")
        yT_sb = yT_pool.tile([P, CCK * N], bf16, tag="yT")

        tp_pool = tc.alloc_tile_pool(name="tpsum", bufs=2, space="PSUM",
                                     side="right")

        # ---- Stage 1: LN1 -> y (bf16) + yT (PE transpose) ----
        for i in range(NB):
            xt = xio.tile([P, C], f32, tag="xio")
            nc.sync.dma_start(xt[:], x_ap[i * P:(i + 1) * P, :])
            ysl = y_sb[:, i * YW: i * YW + C]
            _ln_normalize(nc, stats, lnscr, xt[:], ln1w_t, ln1b_t, ysl,
                          eps_t, skip1)
            for c in range(CCK):
                tp = tp_pool.tile([P, P], bf16, tag="tp")
                nc.tensor.transpose(
                    tp[:], y_sb[:, i * YW + c * P: i * YW + (c + 1) * P],
                    identb[:])
                nc.scalar.copy(
                    yT_sb[:, c * N + i * P: c * N + (i + 1) * P], tp[:])

        # ---- Stage 2: S quarters + Exp -> E (bf16) ----
        # S is symmetric: compute only quarters covering m-blocks >= i
        # (q >= i//4), then mirror the strictly-lower 128x128 tiles via
        # TensorE transpose + DVE copy.
        E_pool = tc.alloc_tile_pool(name="Ebig", bufs=1)
        E_sb = E_pool.tile([P, NB * N], bf16, tag="E")
        with tc.tile_pool(name="spsum", bufs=6, space="PSUM") as sp_pool:
            # Emit quarters in input-availability order: quarter (i, q) needs
            # LN1 tiles <= max(i, 4q+3), so sweep q ascending, i ascending.
            for q in range(SQ):
                for i in range(4 * q + 4) if q < SQ - 1 else range(NB):
                    if q < i // 4:
                        continue
                    s_ps = sp_pool.tile([P, SW], f32, tag="s",
                                        name=f"s_{i}_{q}")
                    for c in range(CCK):
                        nc.tensor.matmul(
                            s_ps[:],
                            yT_sb[:, c * N + i * P: c * N + (i + 1) * P],
                            yT_sb[:, c * N + q * SW: c * N + (q + 1) * SW],
                            start=(c == 0), stop=(c == CCK - 1))
                    nc.scalar.activation(
                        E_sb[:, i * N + q * SW: i * N + (q + 1) * SW],
                        s_ps[:], AF.Exp, bias=expb_t[:, 0:1], scale=SCALE)
                    # Mirror lower tiles (r, i) fed by this quarter, split
                    # across ACT and DVE so neither stalls the a-phase.
                    for r in range(max(i + 1, 4 * q), 4 * q + 4):
                        tp = tp_pool.tile([P, P], bf16, tag="tp",
                                          name=f"tp_{r}_{i}")
                        nc.tensor.transpose(
                            tp[:], E_sb[:, i * N + r * P: i * N + (r + 1) * P],
                            identb[:])
                        dst = E_sb[:, r * N + i * P: r * N + (i + 1) * P]
                        if (r + i) % 2 == 0:
                            nc.vector.tensor_copy(dst, tp[:])
                        else:
                            nc.scalar.copy(dst, tp[:])

        # ---- Stage 3 (fused): a|Z = E@[y|1]; x2 = x + a/Z -> HBM; LN2 -> zT
        yT_pool.release()
        zT_pool = tc.alloc_tile_pool(name="zTbig", bufs=1, side="right")
        zT_sb = zT_pool.tile([P, CCK * N], bf16, tag="zT")
        # fc1T on the right stack so its loads overlap the a-phase (the left
        # stack still holds E until the MLP starts).
        w1_pool = tc.alloc_tile_pool(name="w1big", bufs=1, side="right")
        fc1T_sb = w1_pool.tile([P, CCK * H], bf16, tag="fc1T")
        for c in range(CCK):
            nc.sync.dma_start(fc1T_sb[:, c * H:(c + 1) * H],
                              hs["fc1t"].ap()[c * P:(c + 1) * P, :])
        with tc.tile_pool(name="apsum", bufs=3, space="PSUM") as a_pool:
            for i in range(NB):
                a_ps = a_pool.tile([P, 1024], f32, tag="a")
                for j in range(NB):
                    lhsT = E_sb[:, j * N + i * P: j * N + (i + 1) * P]
                    nc.tensor.matmul(a_ps[:, 0:512], lhsT,
                                     y_sb[:, j * YW: j * YW + 512],
                                     start=(j == 0), stop=(j == NB - 1))
                    nc.tensor.matmul(a_ps[:, 512:769], lhsT,
                                     y_sb[:, j * YW + 512: j * YW + C + 1],
                                     start=(j == 0), stop=(j == NB - 1))
                rZ = stats.tile([P, 1], f32, tag="rZ")
                nc.vector.reciprocal(rZ[:], a_ps[:, 768:769])
                xt = xio.tile([P, C], f32, tag="xio")
                nc.sync.dma_start(xt[:], x_ap[i * P:(i + 1) * P, :])
                x2t = lnscr.tile([P, C], f32, tag="x2t")
                nc.vector.scalar_tensor_tensor(
                    x2t[:], a_ps[:, 0:C], rZ[:, 0:1], xt[:],
                    ALU.mult, ALU.add)
                nc.sync.dma_start(x2s_ap[i * P:(i + 1) * P, :], x2t[:])
                znat = lnscr.tile([P, C], bf16, tag="znat")
                _ln_normalize(nc, stats, lnscr, x2t[:], ln2w_t, ln2b_t,
                              znat[:], eps_t, skip2)
                for c in range(CCK):
                    tp = tp_pool.tile([P, P], bf16, tag="tp")
                    nc.tensor.transpose(tp[:], znat[:, c * P:(c + 1) * P],
                                        identb[:])
                    nc.scalar.copy(
                        zT_sb[:, c * N + i * P: c * N + (i + 1) * P], tp[:])

        # ---- Stage 4: MLP ----
        E_pool.release()
        y_pool.release()
        tp_pool.release()
        w_pool = tc.alloc_tile_pool(name="wbig", bufs=1)
        fc2T_sb = w_pool.tile([P, JB * C], bf16, tag="fc2T")
        for j in range(JB):
            nc.sync.dma_start(fc2T_sb[:, j * C:(j + 1) * C],
                              hs["fc2t"].ap()[j * P:(j + 1) * P, :])

        hT_pool = tc.alloc_tile_pool(name="hTbig", bufs=2)
        with tc.tile_pool(name="hpsum", bufs=4, space="PSUM") as h_pool, \
             tc.tile_pool(name="opsum", bufs=2, space="PSUM") as o_pool:
            for q in range(NQ):
                hT_sb = hT_pool.tile([P, JB * QW], bf16, tag="hT")
                for j in range(JB):
                    h_ps = h_pool.tile([P, QW], f32, tag="h")
                    for c in range(CCK):
                        nc.tensor.matmul(
                            h_ps[:],
                            fc1T_sb[:, c * H + j * P: c * H + (j + 1) * P],
                            zT_sb[:, c * N + q * QW: c * N + (q + 1) * QW],
                            start=(c == 0), stop=(c == CCK - 1))
                    nc.scalar.activation(hT_sb[:, j * QW:(j + 1) * QW],
                                         h_ps[:], AF.Gelu,
                                         bias=fc1b_t[:, j:j + 1])
                for t in range(QW // P):
                    i = q * (QW // P) + t
                    o_ps = o_pool.tile([P, 1024], f32, tag="o")
                    for j in range(JB):
                        lhsT = hT_sb[:, j * QW + t * P: j * QW + (t + 1) * P]
                        nc.tensor.matmul(o_ps[:, 0:512], lhsT,
                                         fc2T_sb[:, j * C: j * C + 512],
                                         start=(j == 0), stop=(j == JB - 1))
                        nc.tensor.matmul(o_ps[:, 512:768], lhsT,
                                         fc2T_sb[:, j * C + 512: j * C + C],
                                         start=(j == 0), stop=(j == JB - 1))
                    xre = xio.tile([P, C], f32, tag="xio")
                    nc.sync.dma_start(xre[:], x2s_ap[i * P:(i + 1) * P, :])
                    if skipb2:
                        o2 = lnscr.tile([P, C], f32, tag="o2")
                        nc.vector.scalar_tensor_tensor(
                            o2[:], o_ps[:, 0:C], 1.0, xre[:],
                            ALU.mult, ALU.add)
                    else:
                        o1 = lnscr.tile([P, C], f32, tag="o1")
                        nc.vector.scalar_tensor_tensor(
                            o1[:], o_ps[:, 0:C], 1.0, fc2b_t[:],
                            ALU.mult, ALU.add)
                        o2 = lnscr.tile([P, C], f32, tag="o2")
                        nc.vector.scalar_tensor_tensor(
                            o2[:], o1[:], 1.0, xre[:], ALU.mult, ALU.add)
                    nc.sync.dma_start(out_ap[i * P:(i + 1) * P, :], o2[:])

        hT_pool.release()
        w_pool.release()
        w1_pool.release()
        zT_pool.release()


def _build(flags):
    nc = bacc.Bacc("TRN2", target_bir_lowering=False, debug=False, num_devices=8)
    hs = {}
    skip1, skip2, skipb2 = flags
    hs["x"] = nc.declare_dram_parameter("x", [N, C], f32, isOutput=False)
    if not skip1:
        hs["ln1w_b"] = nc.declare_dram_parameter("ln1w_b", [P, C], f32, isOutput=False)
        hs["ln1b_b"] = nc.declare_dram_parameter("ln1b_b", [P, C], f32, isOutput=False)
    if not skip2:
        hs["ln2w_b"] = nc.declare_dram_parameter("ln2w_b", [P, C], f32, isOutput=False)
        hs["ln2b_b"] = nc.declare_dram_parameter("ln2b_b", [P, C], f32, isOutput=False)
    hs["fc1t"] = nc.declare_dram_parameter("fc1t", [C, H], bf16, isOutput=False)
    hs["fc2t"] = nc.declare_dram_parameter("fc2t", [H, C], bf16, isOutput=False)
    hs["fc1b_r"] = nc.declare_dram_parameter("fc1b_r", [P, JB], f32, isOutput=False)
    if not skipb2:
        hs["fc2b_b"] = nc.declare_dram_parameter("fc2b_b", [P, C], f32, isOutput=False)
    hs["expb"] = nc.declare_dram_parameter("expb", [P, 1], f32, isOutput=False)
    hs["identb"] = nc.declare_dram_parameter("identb", [P, P], bf16, isOutput=False)
    hs["out"] = nc.declare_dram_parameter("out", [N, C], f32, isOutput=True)
    with tile.TileContext(nc) as tc:
        _emit(nc, tc, hs, flags)
    nc.compile()
    return nc


def _maybe_install_ntff_hook():
    """Optional: lets BASS_TRACE=1 capture NTFF profiles under axon."""
    try:
        import types
        if "antenv.axon_hooks" in sys.modules:
            return
        import antenv
        mod = types.ModuleType("antenv.axon_hooks")
        _hook = [None]
        mod.set_axon_ntff_profile_hook = lambda h: _hook.__setitem__(0, h)
        mod.get_axon_ntff_profile_hook = lambda: _hook[0]
        sys.modules["antenv.axon_hooks"] = mod
        antenv.axon_hooks = mod
        from trn_agent_boot.trn_boot import _ntff_profile_via_ctypes
        mod.set_axon_ntff_profile_hook(
            _ntff_profile_via_ctypes("/opt/axon/libaxon_pjrt.so"))
    except Exception:
        pass


_last_results = None


def kernel(x, ln1_w, ln1_b, ln2_w, ln2_b, fc1_w, fc1_b, fc2_w, fc2_b):
    global _last_results
    bfl = ml_dtypes.bfloat16
    x = np.asarray(x, dtype=np.float32)
    ln1_w = np.asarray(ln1_w, np.float32)
    ln1_b = np.asarray(ln1_b, np.float32)
    ln2_w = np.asarray(ln2_w, np.float32)
    ln2_b = np.asarray(ln2_b, np.float32)
    fc2_b = np.asarray(fc2_b, np.float32)
    skip1 = bool(np.all(ln1_w == 1.0) and np.all(ln1_b == 0.0))
    skip2 = bool(np.all(ln2_w == 1.0) and np.all(ln2_b == 0.0))
    skipb2 = bool(np.all(fc2_b == 0.0))
    flags = (skip1, skip2, skipb2)
    if flags not in _cache:
        _cache[flags] = _build(flags)
    nc = _cache[flags]

    # Constant softmax shift: SCALE*(sqrt(C)*max|w| + ||b||_2)^2 upper-bounds
    # every score S[n,m] (Cauchy-Schwarz on rows of y = LN(x)*w + b, each of
    # which has ||y_n|| <= sqrt(C)*max|w| + ||b||), so exp never overflows and
    # the shift is row-constant => softmax is exact and E stays symmetric.
    ybound = float(np.sqrt(C) * np.abs(ln1_w).max() + np.linalg.norm(ln1_b))
    expb = np.full((P, 1), -SCALE * ybound * ybound, np.float32)
    prep = {
        "fc1t": np.ascontiguousarray(np.asarray(fc1_w, np.float32).T.astype(bfl)),
        "fc2t": np.ascontiguousarray(np.asarray(fc2_w, np.float32).T.astype(bfl)),
        "fc1b_r": np.ascontiguousarray(
            np.asarray(fc1_b, np.float32).reshape(JB, P).T),
        "expb": expb,
        "identb": np.eye(P, dtype=np.float32).astype(bfl),
    }
    if not skip1:
        prep["ln1w_b"] = np.ascontiguousarray(np.broadcast_to(ln1_w, (P, C)))
        prep["ln1b_b"] = np.ascontiguousarray(np.broadcast_to(ln1_b, (P, C)))
    if not skip2:
        prep["ln2w_b"] = np.ascontiguousarray(np.broadcast_to(ln2_w, (P, C)))
        prep["ln2b_b"] = np.ascontiguousarray(np.broadcast_to(ln2_b, (P, C)))
    if not skipb2:
        prep["fc2b_b"] = np.ascontiguousarray(np.broadcast_to(fc2_b, (P, C)))
    in_maps = [dict(prep, x=np.ascontiguousarray(x[b])) for b in range(B)]

    trace = bool(os.environ.get("BASS_TRACE"))
    if trace:
        _maybe_install_ntff_hook()
    res = run_bass_kernel_spmd(nc, in_maps, list(range(B)), trace=trace)
    _last_results = res
    return np.stack([res.results[b]["out"] for b in range(B)], axis=0)


# revision 29
# speedup vs baseline: 1.6222x; 1.0013x over previous
"""Trainium2 Bass kernel for nn_Block (pre-LN transformer block with dense
self-attention where q=k=v=LN1(x), followed by a GELU MLP).

Sharding: data-parallel over batch B=8 across the 8 NeuronCores (one batch
element per core). Weights are replicated; host-side prep transposes/casts the
MLP weights to bf16 and pre-broadcasts the small LN/bias vectors so the device
kernel does pure compute.

Device algorithm per core (x: [2048, 768] fp32):
  1. LN1 -> y (bf16 natural + transposed via TensorE transpose + ACT copy).
     LN stats via bn_stats/bn_aggr (equal 384-wide chunks); when the LN
     weight/bias inputs are exactly ones/zeros (they are for this problem)
     the scale/shift application is skipped entirely.
  2. E := exp(SCALE*S - c) with a host-computed constant shift
     c = SCALE*(sqrt(C)*max|w1| + ||b1||_2)^2 >= max S (Cauchy-Schwarz on the
     LN-normalized rows), so E is SYMMETRIC and the second attention matmul
     reads E tiles directly as lhsT without transposing the score matrix.
     S = y@y^T via accumulating K=128 matmuls into 512-wide PSUM quarters.
  3. a_unnorm = E @ [y | 1] (ones column makes the softmax denominator Z a
     free extra output column); x2 = x + a_unnorm/Z, streamed to an HBM
     scratch; LN2 fused per block -> zT (TensorE transpose).
  4. hT = gelu(fc1 @ z^T + b1) computed transposed (double-buffered per
     512-column chunk); out = x2 + fc2 @ h^T + b2.
"""

import os
import sys
from contextlib import ExitStack

for _p in ("/opt/trn_rl_repo",):
    if _p not in sys.path:
        sys.path.append(_p)

import numpy as np
import ml_dtypes

import concourse.bass as bass
import concourse.bacc as bacc
import concourse.tile as tile
import concourse.mybir as mybir
from concourse.bass_utils import run_bass_kernel_spmd

f32 = mybir.dt.float32
bf16 = mybir.dt.bfloat16
AF = mybir.ActivationFunctionType
ALU = mybir.AluOpType
AX = mybir.AxisListType

B, N, C, H = 8, 2048, 768, 3072
P = 128
NB = N // P        # 16 row blocks of 128
CCK = C // P       # 6 channel chunks of 128
JB = H // P        # 24 hidden blocks of 128
NQ = 4             # MLP sequence chunks
QW = N // NQ       # 512 columns per MLP chunk
SQ = 4             # S-phase quarters per row block
SW = N // SQ       # 512
YW = C + 4         # y block stride (768 data + ones column + pad)
HEADS = 12
SCALE = 1.0 / float(np.sqrt(C // HEADS))   # 0.125
EPS = 1e-5

_cache = {}


def _ln_normalize(nc, stats, uvscr, xt_ap, w_t, b_t, out_ap, eps_t, skip_wb):
    """out = LN(xt) (*w + b unless skip_wb). out_ap may be bf16."""
    st = stats.tile([P, 12], f32, tag="bn")
    nc.vector.bn_stats(st[:, 0:6], xt_ap[:, 0:384])
    nc.vector.bn_stats(st[:, 6:12], xt_ap[:, 384:768])
    mv = stats.tile([P, 2], f32, tag="mv")
    nc.vector.bn_aggr(mv[:], st[:])
    std = stats.tile([P, 1], f32, tag="std")
    nc.scalar.activation(std[:], mv[:, 1:2], AF.Sqrt, bias=eps_t[:, 0:1])
    rstd = stats.tile([P, 1], f32, tag="rstd")
    nc.vector.reciprocal(rstd[:], std[:])
    negmr = stats.tile([P, 1], f32, tag="negmr")         # -mean*rstd
    nc.vector.tensor_scalar(negmr[:], mv[:, 0:1], rstd[:, 0:1], -1.0,
                            ALU.mult, ALU.mult)
    if skip_wb:
        nc.vector.tensor_scalar(out_ap, xt_ap, rstd[:, 0:1], negmr[:, 0:1],
                                ALU.mult, ALU.add)
    else:
        u = uvscr.tile([P, C], f32, tag="u")
        nc.vector.tensor_scalar(u[:], xt_ap, rstd[:, 0:1], negmr[:, 0:1],
                                ALU.mult, ALU.add)
        v = uvscr.tile([P, C], f32, tag="v")
        nc.vector.scalar_tensor_tensor(v[:], u[:], 1.0, w_t[:],
                                       ALU.mult, ALU.mult)
        nc.vector.scalar_tensor_tensor(out_ap, v[:], 1.0, b_t[:],
                                       ALU.mult, ALU.add)


def _emit(nc, tc, hs, flags):
    skip1, skip2, skipb2 = flags
    ctx = ExitStack()
    with ctx:
        small = ctx.enter_context(tc.tile_pool(name="small", bufs=1))
        general = not (skip1 and skip2)
        stats = ctx.enter_context(tc.tile_pool(name="stats", bufs=6))
        lnscr = ctx.enter_context(
            tc.tile_pool(name="lnscr", bufs=2 if general else 3))
        xio = ctx.enter_context(
            tc.tile_pool(name="xio", bufs=2 if general else 4))
        uvscr = (ctx.enter_context(tc.tile_pool(name="uvscr", bufs=2))
                 if general else None)

        def param(name, shape, tag):
            t = small.tile(shape, f32, tag=tag)
            nc.sync.dma_start(t[:], hs[name].ap())
            return t

        ln1w_t = ln1b_t = ln2w_t = ln2b_t = None
        if not skip1:
            ln1w_t = param("ln1w_b", [P, C], "ln1w")
            ln1b_t = param("ln1b_b", [P, C], "ln1b")
        if not skip2:
            ln2w_t = param("ln2w_b", [P, C], "ln2w")
            ln2b_t = param("ln2b_b", [P, C], "ln2b")
        fc2b_t = None
        if not skipb2:
            fc2b_t = param("fc2b_b", [P, C], "fc2b")
        fc1b_t = param("fc1b_r", [P, JB], "fc1b")
        expb_t = param("expb", [P, 1], "expb")
        if general:
            # Device-computed softmax shift: -SCALE * max_n ||y_n||^2 (the
            # host bound is only tight when ln1 w/b are neutral).
            import concourse.bass_isa as bass_isa
            D_t = small.tile([P, NB], f32, tag="D")
            expbd_t = small.tile([P, 1], f32, tag="expbd")
        identb = small.tile([P, P], bf16, tag="identb")
        nc.sync.dma_start(identb[:], hs["identb"].ap())

        eps_t = small.tile([P, 1], f32, tag="eps")
        nc.vector.memset(eps_t[:], EPS)

        x_ap = hs["x"].ap()
        out_ap = hs["out"].ap()
        x2s = nc.dram_tensor("x2scratch", [N, C], f32)
        x2s_ap = x2s.ap()

        y_pool = tc.alloc_tile_pool(name="ybig", bufs=1)
        y_sb = y_pool.tile([P, NB * YW], bf16, tag="y")
        # ones column at offset C per block (strided memset of pad cols only)
        nc.vector.memset(
            y_sb[:].rearrange("p (i w) -> p i w", w=YW)[:, :, C:YW], 1.0)
        yT_pool = tc.alloc_tile_pool(name="yTbig", bufs=1, side="right")
        yT_sb = yT_pool.tile([P, CCK * N], bf16, tag="yT")

        tp_pool = tc.alloc_tile_pool(name="tpsum", bufs=2, space="PSUM",
                                     side="right")

        # ---- Stage 1: LN1 -> y (bf16) + yT (PE transpose) ----
        for i in range(NB):
            xt = xio.tile([P, C], f32, tag="xio")
            nc.sync.dma_start(xt[:], x_ap[i * P:(i + 1) * P, :])
            ysl = y_sb[:, i * YW: i * YW + C]
            _ln_normalize(nc, stats, uvscr, xt[:], ln1w_t, ln1b_t, ysl,
                          eps_t, skip1)
            if general:
                ysq = lnscr.tile([P, C], bf16, tag="znat")
                nc.scalar.activation(ysq[:], ysl, AF.Square,
                                     accum_out=D_t[:, i:i + 1])
            for c in range(CCK):
                tp = tp_pool.tile([P, P], bf16, tag="tp")
                nc.tensor.transpose(
                    tp[:], y_sb[:, i * YW + c * P: i * YW + (c + 1) * P],
                    identb[:])
                nc.scalar.copy(
                    yT_sb[:, c * N + i * P: c * N + (i + 1) * P], tp[:])

        if general:
            dmax = stats.tile([P, 1], f32, tag="dmax")
            nc.vector.tensor_reduce(dmax[:], D_t[:, 0:NB], AX.X, ALU.max)
            gall = stats.tile([P, 1], f32, tag="gall")
            nc.gpsimd.partition_all_reduce(gall[:], dmax[:], channels=P,
                                           reduce_op=bass_isa.ReduceOp.max)
            nc.vector.tensor_scalar(expbd_t[:], gall[:], -SCALE, None,
                                    ALU.mult)
            expb_t = expbd_t

        # ---- Stage 2: S quarters + Exp -> E (bf16) ----
        # S is symmetric: compute only quarters covering m-blocks >= i
        # (q >= i//4), then mirror the strictly-lower 128x128 tiles via
        # TensorE transpose + DVE copy.
        E_pool = tc.alloc_tile_pool(name="Ebig", bufs=1)
        E_sb = E_pool.tile([P, NB * N], bf16, tag="E")
        with tc.tile_pool(name="spsum", bufs=6, space="PSUM") as sp_pool:
            # Emit quarters in input-availability order: quarter (i, q) needs
            # LN1 tiles <= max(i, 4q+3), so sweep q ascending, i ascending.
            for q in range(SQ):
                for i in range(4 * q + 4) if q < SQ - 1 else range(NB):
                    if q < i // 4:
                        continue
                    s_ps = sp_pool.tile([P, SW], f32, tag="s",
                                        name=f"s_{i}_{q}")
                    for c in range(CCK):
                        nc.tensor.matmul(
                            s_ps[:],
                            yT_sb[:, c * N + i * P: c * N + (i + 1) * P],
                            yT_sb[:, c * N + q * SW: c * N + (q + 1) * SW],
                            start=(c == 0), stop=(c == CCK - 1))
                    nc.scalar.activation(
                        E_sb[:, i * N + q * SW: i * N + (q + 1) * SW],
                        s_ps[:], AF.Exp, bias=expb_t[:, 0:1], scale=SCALE)
                    # Mirror lower tiles (r, i) fed by this quarter, split
                    # across ACT and DVE so neither stalls the a-phase.
                    for r in range(max(i + 1, 4 * q), 4 * q + 4):
                        tp = tp_pool.tile([P, P], bf16, tag="tp",
                                          name=f"tp_{r}_{i}")
                        nc.tensor.transpose(
                            tp[:], E_sb[:, i * N + r * P: i * N + (r + 1) * P],
                            identb[:])
                        dst = E_sb[:, r * N + i * P: r * N + (i + 1) * P]
                        if (r + i) % 2 == 0:
                            nc.vector.tensor_copy(dst, tp[:])
                        else:
                            nc.scalar.copy(dst, tp[:])

        # ---- Stage 3 (fused): a|Z = E@[y|1]; x2 = x + a/Z -> HBM; LN2 -> zT
        yT_pool.release()
        zT_pool = tc.alloc_tile_pool(name="zTbig", bufs=1, side="right")
        zT_sb = zT_pool.tile([P, CCK * N], bf16, tag="zT")
        # fc1T on the right stack so its loads overlap the a-phase (the left
        # stack still holds E until the MLP starts).
        w1_pool = tc.alloc_tile_pool(name="w1big", bufs=1, side="right")
        fc1T_sb = w1_pool.tile([P, CCK * H], bf16, tag="fc1T")
        for c in range(CCK):
            nc.sync.dma_start(fc1T_sb[:, c * H:(c + 1) * H],
                              hs["fc1t"].ap()[c * P:(c + 1) * P, :])
        with tc.tile_pool(name="apsum", bufs=3, space="PSUM") as a_pool:
            for i in range(NB):
                a_ps = a_pool.tile([P, 1024], f32, tag="a")
                for j in range(NB):
                    lhsT = E_sb[:, j * N + i * P: j * N + (i + 1) * P]
                    nc.tensor.matmul(a_ps[:, 0:512], lhsT,
                                     y_sb[:, j * YW: j * YW + 512],
                                     start=(j == 0), stop=(j == NB - 1))
                    nc.tensor.matmul(a_ps[:, 512:769], lhsT,
                                     y_sb[:, j * YW + 512: j * YW + C + 1],
                                     start=(j == 0), stop=(j == NB - 1))
                rZ = stats.tile([P, 1], f32, tag="rZ")
                if general:
                    zc = stats.tile([P, 1], f32, tag="zc")
                    nc.vector.tensor_scalar(zc[:], a_ps[:, 768:769], 1e-30,
                                            None, ALU.max)
                    nc.vector.reciprocal(rZ[:], zc[:])
                else:
                    nc.vector.reciprocal(rZ[:], a_ps[:, 768:769])
                xt = xio.tile([P, C], f32, tag="xio")
                nc.sync.dma_start(xt[:], x_ap[i * P:(i + 1) * P, :])
                x2t = lnscr.tile([P, C], f32, tag="x2t")
                nc.vector.scalar_tensor_tensor(
                    x2t[:], a_ps[:, 0:C], rZ[:, 0:1], xt[:],
                    ALU.mult, ALU.add)
                nc.sync.dma_start(x2s_ap[i * P:(i + 1) * P, :], x2t[:])
                znat = lnscr.tile([P, C], bf16, tag="znat")
                _ln_normalize(nc, stats, uvscr, x2t[:], ln2w_t, ln2b_t,
                              znat[:], eps_t, skip2)
                for c in range(CCK):
                    tp = tp_pool.tile([P, P], bf16, tag="tp")
                    nc.tensor.transpose(tp[:], znat[:, c * P:(c + 1) * P],
                                        identb[:])
                    nc.scalar.copy(
                        zT_sb[:, c * N + i * P: c * N + (i + 1) * P], tp[:])

        # ---- Stage 4: MLP ----
        E_pool.release()
        y_pool.release()
        tp_pool.release()
        w_pool = tc.alloc_tile_pool(name="wbig", bufs=1)
        fc2T_sb = w_pool.tile([P, JB * C], bf16, tag="fc2T")
        for j in range(JB):
            nc.sync.dma_start(fc2T_sb[:, j * C:(j + 1) * C],
                              hs["fc2t"].ap()[j * P:(j + 1) * P, :])

        hT_pool = tc.alloc_tile_pool(name="hTbig", bufs=1 if general else 2)
        with tc.tile_pool(name="hpsum", bufs=4, space="PSUM") as h_pool, \
             tc.tile_pool(name="opsum", bufs=2, space="PSUM") as o_pool:
            for q in range(NQ):
                hT_sb = hT_pool.tile([P, JB * QW], bf16, tag="hT")
                for j in range(JB):
                    h_ps = h_pool.tile([P, QW], f32, tag="h")
                    for c in range(CCK):
                        nc.tensor.matmul(
                            h_ps[:],
                            fc1T_sb[:, c * H + j * P: c * H + (j + 1) * P],
                            zT_sb[:, c * N + q * QW: c * N + (q + 1) * QW],
                            start=(c == 0), stop=(c == CCK - 1))
                    nc.scalar.activation(hT_sb[:, j * QW:(j + 1) * QW],
                                         h_ps[:], AF.Gelu,
                                         bias=fc1b_t[:, j:j + 1])
                for t in range(QW // P):
                    i = q * (QW // P) + t
                    o_ps = o_pool.tile([P, 1024], f32, tag="o")
                    for j in range(JB):
                        lhsT = hT_sb[:, j * QW + t * P: j * QW + (t + 1) * P]
                        nc.tensor.matmul(o_ps[:, 0:512], lhsT,
                                         fc2T_sb[:, j * C: j * C + 512],
                                         start=(j == 0), stop=(j == JB - 1))
                        nc.tensor.matmul(o_ps[:, 512:768], lhsT,
                                         fc2T_sb[:, j * C + 512: j * C + C],
                                         start=(j == 0), stop=(j == JB - 1))
                    xre = xio.tile([P, C], f32, tag="xio")
                    nc.sync.dma_start(xre[:], x2s_ap[i * P:(i + 1) * P, :])
                    if skipb2:
                        o2 = lnscr.tile([P, C], f32, tag="o2")
                        nc.vector.scalar_tensor_tensor(
                            o2[:], o_ps[:, 0:C], 1.0, xre[:],
                            ALU.mult, ALU.add)
                    else:
                        o1 = lnscr.tile([P, C], f32, tag="o1")
                        nc.vector.scalar_tensor_tensor(
                            o1[:], o_ps[:, 0:C], 1.0, fc2b_t[:],
                            ALU.mult, ALU.add)
                        o2 = lnscr.tile([P, C], f32, tag="o2")
                        nc.vector.scalar_tensor_tensor(
                            o2[:], o1[:], 1.0, xre[:], ALU.mult, ALU.add)
                    nc.sync.dma_start(out_ap[i * P:(i + 1) * P, :], o2[:])

        hT_pool.release()
        w_pool.release()
        w1_pool.release()
        zT_pool.release()


def _build(flags):
    nc = bacc.Bacc("TRN2", target_bir_lowering=False, debug=False, num_devices=8)
    hs = {}
    skip1, skip2, skipb2 = flags
    hs["x"] = nc.declare_dram_parameter("x", [N, C], f32, isOutput=False)
    if not skip1:
        hs["ln1w_b"] = nc.declare_dram_parameter("ln1w_b", [P, C], f32, isOutput=False)
        hs["ln1b_b"] = nc.declare_dram_parameter("ln1b_b", [P, C], f32, isOutput=False)
    if not skip2:
        hs["ln2w_b"] = nc.declare_dram_parameter("ln2w_b", [P, C], f32, isOutput=False)
        hs["ln2b_b"] = nc.declare_dram_parameter("ln2b_b", [P, C], f32, isOutput=False)
    hs["fc1t"] = nc.declare_dram_parameter("fc1t", [C, H], bf16, isOutput=False)
    hs["fc2t"] = nc.declare_dram_parameter("fc2t", [H, C], bf16, isOutput=False)
    hs["fc1b_r"] = nc.declare_dram_parameter("fc1b_r", [P, JB], f32, isOutput=False)
    if not skipb2:
        hs["fc2b_b"] = nc.declare_dram_parameter("fc2b_b", [P, C], f32, isOutput=False)
    hs["expb"] = nc.declare_dram_parameter("expb", [P, 1], f32, isOutput=False)
    hs["identb"] = nc.declare_dram_parameter("identb", [P, P], bf16, isOutput=False)
    hs["out"] = nc.declare_dram_parameter("out", [N, C], f32, isOutput=True)
    with tile.TileContext(nc) as tc:
        _emit(nc, tc, hs, flags)
    nc.compile()
    return nc


def _maybe_install_ntff_hook():
    """Optional: lets BASS_TRACE=1 capture NTFF profiles under axon."""
    try:
        import types
        if "antenv.axon_hooks" in sys.modules:
            return
        import antenv
        mod = types.ModuleType("antenv.axon_hooks")
        _hook = [None]
        mod.set_axon_ntff_profile_hook = lambda h: _hook.__setitem__(0, h)
        mod.get_axon_ntff_profile_hook = lambda: _hook[0]
        sys.modules["antenv.axon_hooks"] = mod
        antenv.axon_hooks = mod
        from trn_agent_boot.trn_boot import _ntff_profile_via_ctypes
        mod.set_axon_ntff_profile_hook(
            _ntff_profile_via_ctypes("/opt/axon/libaxon_pjrt.so"))
    except Exception:
        pass


_last_results = None


def kernel(x, ln1_w, ln1_b, ln2_w, ln2_b, fc1_w, fc1_b, fc2_w, fc2_b):
    global _last_results
    bfl = ml_dtypes.bfloat16
    x = np.asarray(x, dtype=np.float32)
    ln1_w = np.asarray(ln1_w, np.float32)
    ln1_b = np.asarray(ln1_b, np.float32)
    ln2_w = np.asarray(ln2_w, np.float32)
    ln2_b = np.asarray(ln2_b, np.float32)
    fc2_b = np.asarray(fc2_b, np.float32)
    skip1 = bool(np.all(ln1_w == 1.0) and np.all(ln1_b == 0.0))
    skip2 = bool(np.all(ln2_w == 1.0) and np.all(ln2_b == 0.0))
    skipb2 = bool(np.all(fc2_b == 0.0))
    flags = (skip1, skip2, skipb2)
    if flags not in _cache:
        _cache[flags] = _build(flags)
    nc = _cache[flags]

    # Constant softmax shift: SCALE*(sqrt(C)*max|w| + ||b||_2)^2 upper-bounds
    # every score S[n,m] (Cauchy-Schwarz on rows of y = LN(x)*w + b, each of
    # which has ||y_n|| <= sqrt(C)*max|w| + ||b||), so exp never overflows and
    # the shift is row-constant => softmax is exact and E stays symmetric.
    ybound = float(np.sqrt(C) * np.abs(ln1_w).max() + np.linalg.norm(ln1_b))
    expb = np.full((P, 1), -SCALE * ybound * ybound, np.float32)
    prep = {
        "fc1t": np.ascontiguousarray(np.asarray(fc1_w, np.float32).T.astype(bfl)),
        "fc2t": np.ascontiguousarray(np.asarray(fc2_w, np.float32).T.astype(bfl)),
        "fc1b_r": np.ascontiguousarray(
            np.asarray(fc1_b, np.float32).reshape(JB, P).T),
        "expb": expb,
        "identb": np.eye(P, dtype=np.float32).astype(bfl),
    }
    if not skip1:
        prep["ln1w_b"] = np.ascontiguousarray(np.broadcast_to(ln1_w, (P, C)))
        prep["ln1b_b"] = np.ascontiguousarray(np.broadcast_to(ln1_b, (P, C)))
    if not skip2:
        prep["ln2w_b"] = np.ascontiguousarray(np.broadcast_to(ln2_w, (P, C)))
        prep["ln2b_b"] = np.ascontiguousarray(np.broadcast_to(ln2_b, (P, C)))
    if not skipb2:
        prep["fc2b_b"] = np.ascontiguousarray(np.broadcast_to(fc2_b, (P, C)))
    in_maps = [dict(prep, x=np.ascontiguousarray(x[b])) for b in range(B)]

    trace = bool(os.environ.get("BASS_TRACE"))
    if trace:
        _maybe_install_ntff_hook()
    res = run_bass_kernel_spmd(nc, in_maps, list(range(B)), trace=trace)
    _last_results = res
    return np.stack([res.results[b]["out"] for b in range(B)], axis=0)


# revision 30
# speedup vs baseline: 1.6563x; 1.0211x over previous
"""Trainium2 Bass kernel for nn_Block (pre-LN transformer block with dense
self-attention where q=k=v=LN1(x), followed by a GELU MLP).

Sharding: data-parallel over batch B=8 across the 8 NeuronCores (one batch
element per core). Weights are replicated; host-side prep transposes/casts the
MLP weights to bf16 and pre-broadcasts the small LN/bias vectors so the device
kernel does pure compute.

Device algorithm per core (x: [2048, 768] fp32):
  1. LN1 -> y (bf16 natural + transposed via TensorE transpose + ACT copy).
     LN stats via bn_stats/bn_aggr (equal 384-wide chunks); when the LN
     weight/bias inputs are exactly ones/zeros (they are for this problem)
     the scale/shift application is skipped entirely.
  2. E := exp(SCALE*S - c) with a host-computed constant shift
     c = SCALE*(sqrt(C)*max|w1| + ||b1||_2)^2 >= max S (Cauchy-Schwarz on the
     LN-normalized rows), so E is SYMMETRIC and the second attention matmul
     reads E tiles directly as lhsT without transposing the score matrix.
     S = y@y^T via accumulating K=128 matmuls into 512-wide PSUM quarters.
  3. a_unnorm = E @ [y | 1] (ones column makes the softmax denominator Z a
     free extra output column); x2 = x + a_unnorm/Z, streamed to an HBM
     scratch; LN2 fused per block -> zT (TensorE transpose).
  4. hT = gelu(fc1 @ z^T + b1) computed transposed (double-buffered per
     512-column chunk); out = x2 + fc2 @ h^T + b2.
"""

import os
import sys
from contextlib import ExitStack

for _p in ("/opt/trn_rl_repo",):
    if _p not in sys.path:
        sys.path.append(_p)

import numpy as np
import ml_dtypes

import concourse.bass as bass
import concourse.bacc as bacc
import concourse.tile as tile
import concourse.mybir as mybir
from concourse.bass_utils import run_bass_kernel_spmd

f32 = mybir.dt.float32
bf16 = mybir.dt.bfloat16
AF = mybir.ActivationFunctionType
ALU = mybir.AluOpType
AX = mybir.AxisListType

B, N, C, H = 8, 2048, 768, 3072
P = 128
NB = N // P        # 16 row blocks of 128
CCK = C // P       # 6 channel chunks of 128
JB = H // P        # 24 hidden blocks of 128
NQ = 4             # MLP sequence chunks
QW = N // NQ       # 512 columns per MLP chunk
SQ = 4             # S-phase quarters per row block
SW = N // SQ       # 512
YW = C + 4         # y block stride (768 data + ones column + pad)
HEADS = 12
SCALE = 1.0 / float(np.sqrt(C // HEADS))   # 0.125
EPS = 1e-5

_cache = {}


def _ln_normalize(nc, stats, uvscr, xt_ap, w_t, b_t, out_ap, eps_t, skip_wb):
    """out = LN(xt) (*w + b unless skip_wb). out_ap may be bf16."""
    st = stats.tile([P, 12], f32, tag="bn")
    nc.vector.bn_stats(st[:, 0:6], xt_ap[:, 0:384])
    nc.vector.bn_stats(st[:, 6:12], xt_ap[:, 384:768])
    mv = stats.tile([P, 2], f32, tag="mv")
    nc.vector.bn_aggr(mv[:], st[:])
    std = stats.tile([P, 1], f32, tag="std")
    nc.scalar.activation(std[:], mv[:, 1:2], AF.Sqrt, bias=eps_t[:, 0:1])
    rstd = stats.tile([P, 1], f32, tag="rstd")
    nc.vector.reciprocal(rstd[:], std[:])
    negmr = stats.tile([P, 1], f32, tag="negmr")         # -mean*rstd
    nc.vector.tensor_scalar(negmr[:], mv[:, 0:1], rstd[:, 0:1], -1.0,
                            ALU.mult, ALU.mult)
    if skip_wb:
        nc.vector.tensor_scalar(out_ap, xt_ap, rstd[:, 0:1], negmr[:, 0:1],
                                ALU.mult, ALU.add)
    else:
        u = uvscr.tile([P, C], f32, tag="u")
        nc.vector.tensor_scalar(u[:], xt_ap, rstd[:, 0:1], negmr[:, 0:1],
                                ALU.mult, ALU.add)
        v = uvscr.tile([P, C], f32, tag="v")
        nc.vector.scalar_tensor_tensor(v[:], u[:], 1.0, w_t[:],
                                       ALU.mult, ALU.mult)
        nc.vector.scalar_tensor_tensor(out_ap, v[:], 1.0, b_t[:],
                                       ALU.mult, ALU.add)


def _emit(nc, tc, hs, flags):
    skip1, skip2, skipb2 = flags
    ctx = ExitStack()
    with ctx:
        small = ctx.enter_context(tc.tile_pool(name="small", bufs=1))
        general = not (skip1 and skip2)
        stats = ctx.enter_context(tc.tile_pool(name="stats", bufs=6))
        lnscr = ctx.enter_context(
            tc.tile_pool(name="lnscr", bufs=2 if general else 3))
        xio = ctx.enter_context(
            tc.tile_pool(name="xio", bufs=2 if general else 4))
        uvscr = (ctx.enter_context(tc.tile_pool(name="uvscr", bufs=2))
                 if general else None)

        def param(name, shape, tag):
            t = small.tile(shape, f32, tag=tag)
            nc.sync.dma_start(t[:], hs[name].ap())
            return t

        ln1w_t = ln1b_t = ln2w_t = ln2b_t = None
        if not skip1:
            ln1w_t = param("ln1w_b", [P, C], "ln1w")
            ln1b_t = param("ln1b_b", [P, C], "ln1b")
        if not skip2:
            ln2w_t = param("ln2w_b", [P, C], "ln2w")
            ln2b_t = param("ln2b_b", [P, C], "ln2b")
        fc2b_t = None
        if not skipb2:
            fc2b_t = param("fc2b_b", [P, C], "fc2b")
        fc1b_t = param("fc1b_r", [P, JB], "fc1b")
        expb_t = param("expb", [P, 1], "expb")
        if general:
            # Device-computed softmax shift: -SCALE * max_n ||y_n||^2 (the
            # host bound is only tight when ln1 w/b are neutral).
            import concourse.bass_isa as bass_isa
            D_t = small.tile([P, NB], f32, tag="D")
            expbd_t = small.tile([P, 1], f32, tag="expbd")
        identb = small.tile([P, P], bf16, tag="identb")
        nc.sync.dma_start(identb[:], hs["identb"].ap())

        eps_t = small.tile([P, 1], f32, tag="eps")
        nc.vector.memset(eps_t[:], EPS)

        x_ap = hs["x"].ap()
        out_ap = hs["out"].ap()
        x2s = nc.dram_tensor("x2scratch", [N, C], f32)
        x2s_ap = x2s.ap()

        y_pool = tc.alloc_tile_pool(name="ybig", bufs=1)
        y_sb = y_pool.tile([P, NB * YW], bf16, tag="y")
        # ones column at offset C per block (strided memset of pad cols only)
        nc.vector.memset(
            y_sb[:].rearrange("p (i w) -> p i w", w=YW)[:, :, C:YW], 1.0)
        yT_pool = tc.alloc_tile_pool(name="yTbig", bufs=1, side="right")
        yT_sb = yT_pool.tile([P, CCK * N], bf16, tag="yT")

        tp_pool = tc.alloc_tile_pool(name="tpsum", bufs=2, space="PSUM",
                                     side="right")

        # ---- Stage 1: LN1 -> y (bf16) + yT (PE transpose) ----
        for i in range(NB):
            xt = xio.tile([P, C], f32, tag="xio")
            nc.sync.dma_start(xt[:], x_ap[i * P:(i + 1) * P, :])
            ysl = y_sb[:, i * YW: i * YW + C]
            _ln_normalize(nc, stats, uvscr, xt[:], ln1w_t, ln1b_t, ysl,
                          eps_t, skip1)
            if general:
                ysq = lnscr.tile([P, C], bf16, tag="znat")
                nc.scalar.activation(ysq[:], ysl, AF.Square,
                                     accum_out=D_t[:, i:i + 1])
            for c in range(CCK):
                tp = tp_pool.tile([P, P], bf16, tag="tp")
                nc.tensor.transpose(
                    tp[:], y_sb[:, i * YW + c * P: i * YW + (c + 1) * P],
                    identb[:])
                nc.scalar.copy(
                    yT_sb[:, c * N + i * P: c * N + (i + 1) * P], tp[:])

        if general:
            dmax = stats.tile([P, 1], f32, tag="dmax")
            nc.vector.tensor_reduce(dmax[:], D_t[:, 0:NB], AX.X, ALU.max)
            gall = stats.tile([P, 1], f32, tag="gall")
            nc.gpsimd.partition_all_reduce(gall[:], dmax[:], channels=P,
                                           reduce_op=bass_isa.ReduceOp.max)
            nc.vector.tensor_scalar(expbd_t[:], gall[:], -SCALE, None,
                                    ALU.mult)
            expb_t = expbd_t

        # ---- Stage 2: S quarters + Exp -> E (bf16) ----
        # S is symmetric: compute only quarters covering m-blocks >= i
        # (q >= i//4), then mirror the strictly-lower 128x128 tiles via
        # TensorE transpose + DVE copy.
        E_pool = tc.alloc_tile_pool(name="Ebig", bufs=1)
        E_sb = E_pool.tile([P, NB * N], bf16, tag="E")
        with tc.tile_pool(name="spsum", bufs=6, space="PSUM") as sp_pool:
            # Emit quarters in input-availability order: quarter (i, q) needs
            # LN1 tiles <= max(i, 4q+3), so sweep q ascending, i ascending.
            for q in range(SQ):
                for i in range(4 * q + 4) if q < SQ - 1 else range(NB):
                    if q < i // 4:
                        continue
                    # Diagonal quarters: columns left of the diagonal tile are
                    # mirror-filled, so start at the diagonal (narrower MMs,
                    # no WAW with the mirror copies).
                    off = (i - 4 * q) * P if q == i // 4 else 0
                    w = SW - off
                    s_ps = sp_pool.tile([P, SW], f32, tag="s",
                                        name=f"s_{i}_{q}")
                    for c in range(CCK):
                        nc.tensor.matmul(
                            s_ps[:, 0:w],
                            yT_sb[:, c * N + i * P: c * N + (i + 1) * P],
                            yT_sb[:, c * N + q * SW + off:
                                  c * N + (q + 1) * SW],
                            start=(c == 0), stop=(c == CCK - 1))
                    nc.scalar.activation(
                        E_sb[:, i * N + q * SW + off: i * N + (q + 1) * SW],
                        s_ps[:, 0:w], AF.Exp, bias=expb_t[:, 0:1], scale=SCALE)
                    # Mirror lower tiles (r, i) fed by this quarter, split
                    # across ACT and DVE so neither stalls the a-phase.
                    for r in range(max(i + 1, 4 * q), 4 * q + 4):
                        tp = tp_pool.tile([P, P], bf16, tag="tp",
                                          name=f"tp_{r}_{i}")
                        nc.tensor.transpose(
                            tp[:], E_sb[:, i * N + r * P: i * N + (r + 1) * P],
                            identb[:])
                        dst = E_sb[:, r * N + i * P: r * N + (i + 1) * P]
                        if (r + i) % 2 == 0:
                            nc.vector.tensor_copy(dst, tp[:])
                        else:
                            nc.scalar.copy(dst, tp[:])

        # ---- Stage 3 (fused): a|Z = E@[y|1]; x2 = x + a/Z -> HBM; LN2 -> zT
        yT_pool.release()
        zT_pool = tc.alloc_tile_pool(name="zTbig", bufs=1, side="right")
        zT_sb = zT_pool.tile([P, CCK * N], bf16, tag="zT")
        # fc1T on the right stack so its loads overlap the a-phase (the left
        # stack still holds E until the MLP starts).
        w1_pool = tc.alloc_tile_pool(name="w1big", bufs=1, side="right")
        fc1T_sb = w1_pool.tile([P, CCK * H], bf16, tag="fc1T")
        for c in range(CCK):
            nc.sync.dma_start(fc1T_sb[:, c * H:(c + 1) * H],
                              hs["fc1t"].ap()[c * P:(c + 1) * P, :])
        with tc.tile_pool(name="apsum", bufs=3, space="PSUM") as a_pool:
            for i in range(NB):
                a_ps = a_pool.tile([P, 1024], f32, tag="a")
                for j in range(NB):
                    lhsT = E_sb[:, j * N + i * P: j * N + (i + 1) * P]
                    nc.tensor.matmul(a_ps[:, 0:512], lhsT,
                                     y_sb[:, j * YW: j * YW + 512],
                                     start=(j == 0), stop=(j == NB - 1))
                    nc.tensor.matmul(a_ps[:, 512:769], lhsT,
                                     y_sb[:, j * YW + 512: j * YW + C + 1],
                                     start=(j == 0), stop=(j == NB - 1))
                rZ = stats.tile([P, 1], f32, tag="rZ")
                if general:
                    zc = stats.tile([P, 1], f32, tag="zc")
                    nc.vector.tensor_scalar(zc[:], a_ps[:, 768:769], 1e-30,
                                            None, ALU.max)
                    nc.vector.reciprocal(rZ[:], zc[:])
                else:
                    nc.vector.reciprocal(rZ[:], a_ps[:, 768:769])
                xt = xio.tile([P, C], f32, tag="xio")
                nc.sync.dma_start(xt[:], x_ap[i * P:(i + 1) * P, :])
                x2t = lnscr.tile([P, C], f32, tag="x2t")
                nc.vector.scalar_tensor_tensor(
                    x2t[:], a_ps[:, 0:C], rZ[:, 0:1], xt[:],
                    ALU.mult, ALU.add)
                nc.sync.dma_start(x2s_ap[i * P:(i + 1) * P, :], x2t[:])
                znat = lnscr.tile([P, C], bf16, tag="znat")
                _ln_normalize(nc, stats, uvscr, x2t[:], ln2w_t, ln2b_t,
                              znat[:], eps_t, skip2)
                for c in range(CCK):
                    tp = tp_pool.tile([P, P], bf16, tag="tp")
                    nc.tensor.transpose(tp[:], znat[:, c * P:(c + 1) * P],
                                        identb[:])
                    nc.scalar.copy(
                        zT_sb[:, c * N + i * P: c * N + (i + 1) * P], tp[:])

        # ---- Stage 4: MLP ----
        E_pool.release()
        y_pool.release()
        tp_pool.release()
        w_pool = tc.alloc_tile_pool(name="wbig", bufs=1)
        fc2T_sb = w_pool.tile([P, JB * C], bf16, tag="fc2T")
        for j in range(JB):
            nc.sync.dma_start(fc2T_sb[:, j * C:(j + 1) * C],
                              hs["fc2t"].ap()[j * P:(j + 1) * P, :])

        hT_pool = tc.alloc_tile_pool(name="hTbig", bufs=1 if general else 2)
        with tc.tile_pool(name="hpsum", bufs=4, space="PSUM") as h_pool, \
             tc.tile_pool(name="opsum", bufs=2, space="PSUM") as o_pool:
            for q in range(NQ):
                hT_sb = hT_pool.tile([P, JB * QW], bf16, tag="hT")
                for j in range(JB):
                    h_ps = h_pool.tile([P, QW], f32, tag="h")
                    for c in range(CCK):
                        nc.tensor.matmul(
                            h_ps[:],
                            fc1T_sb[:, c * H + j * P: c * H + (j + 1) * P],
                            zT_sb[:, c * N + q * QW: c * N + (q + 1) * QW],
                            start=(c == 0), stop=(c == CCK - 1))
                    nc.scalar.activation(hT_sb[:, j * QW:(j + 1) * QW],
                                         h_ps[:], AF.Gelu,
                                         bias=fc1b_t[:, j:j + 1])
                for t in range(QW // P):
                    i = q * (QW // P) + t
                    o_ps = o_pool.tile([P, 1024], f32, tag="o")
                    for j in range(JB):
                        lhsT = hT_sb[:, j * QW + t * P: j * QW + (t + 1) * P]
                        nc.tensor.matmul(o_ps[:, 0:512], lhsT,
                                         fc2T_sb[:, j * C: j * C + 512],
                                         start=(j == 0), stop=(j == JB - 1))
                        nc.tensor.matmul(o_ps[:, 512:768], lhsT,
                                         fc2T_sb[:, j * C + 512: j * C + C],
                                         start=(j == 0), stop=(j == JB - 1))
                    xre = xio.tile([P, C], f32, tag="xio")
                    nc.sync.dma_start(xre[:], x2s_ap[i * P:(i + 1) * P, :])
                    if skipb2:
                        o2 = lnscr.tile([P, C], f32, tag="o2")
                        nc.vector.scalar_tensor_tensor(
                            o2[:], o_ps[:, 0:C], 1.0, xre[:],
                            ALU.mult, ALU.add)
                    else:
                        o1 = lnscr.tile([P, C], f32, tag="o1")
                        nc.vector.scalar_tensor_tensor(
                            o1[:], o_ps[:, 0:C], 1.0, fc2b_t[:],
                            ALU.mult, ALU.add)
                        o2 = lnscr.tile([P, C], f32, tag="o2")
                        nc.vector.scalar_tensor_tensor(
                            o2[:], o1[:], 1.0, xre[:], ALU.mult, ALU.add)
                    nc.sync.dma_start(out_ap[i * P:(i + 1) * P, :], o2[:])

        hT_pool.release()
        w_pool.release()
        w1_pool.release()
        zT_pool.release()


def _build(flags):
    nc = bacc.Bacc("TRN2", target_bir_lowering=False, debug=False, num_devices=8)
    hs = {}
    skip1, skip2, skipb2 = flags
    hs["x"] = nc.declare_dram_parameter("x", [N, C], f32, isOutput=False)
    if not skip1:
        hs["ln1w_b"] = nc.declare_dram_parameter("ln1w_b", [P, C], f32, isOutput=False)
        hs["ln1b_b"] = nc.declare_dram_parameter("ln1b_b", [P, C], f32, isOutput=False)
    if not skip2:
        hs["ln2w_b"] = nc.declare_dram_parameter("ln2w_b", [P, C], f32, isOutput=False)
        hs["ln2b_b"] = nc.declare_dram_parameter("ln2b_b", [P, C], f32, isOutput=False)
    hs["fc1t"] = nc.declare_dram_parameter("fc1t", [C, H], bf16, isOutput=False)
    hs["fc2t"] = nc.declare_dram_parameter("fc2t", [H, C], bf16, isOutput=False)
    hs["fc1b_r"] = nc.declare_dram_parameter("fc1b_r", [P, JB], f32, isOutput=False)
    if not skipb2:
        hs["fc2b_b"] = nc.declare_dram_parameter("fc2b_b", [P, C], f32, isOutput=False)
    hs["expb"] = nc.declare_dram_parameter("expb", [P, 1], f32, isOutput=False)
    hs["identb"] = nc.declare_dram_parameter("identb", [P, P], bf16, isOutput=False)
    hs["out"] = nc.declare_dram_parameter("out", [N, C], f32, isOutput=True)
    with tile.TileContext(nc) as tc:
        _emit(nc, tc, hs, flags)
    nc.compile()
    return nc


def _maybe_install_ntff_hook():
    """Optional: lets BASS_TRACE=1 capture NTFF profiles under axon."""
    try:
        import types
        if "antenv.axon_hooks" in sys.modules:
            return
        import antenv
        mod = types.ModuleType("antenv.axon_hooks")
        _hook = [None]
        mod.set_axon_ntff_profile_hook = lambda h: _hook.__setitem__(0, h)
        mod.get_axon_ntff_profile_hook = lambda: _hook[0]
        sys.modules["antenv.axon_hooks"] = mod
        antenv.axon_hooks = mod
        from trn_agent_boot.trn_boot import _ntff_profile_via_ctypes
        mod.set_axon_ntff_profile_hook(
            _ntff_profile_via_ctypes("/opt/axon/libaxon_pjrt.so"))
    except Exception:
        pass


_last_results = None


def kernel(x, ln1_w, ln1_b, ln2_w, ln2_b, fc1_w, fc1_b, fc2_w, fc2_b):
    global _last_results
    bfl = ml_dtypes.bfloat16
    x = np.asarray(x, dtype=np.float32)
    ln1_w = np.asarray(ln1_w, np.float32)
    ln1_b = np.asarray(ln1_b, np.float32)
    ln2_w = np.asarray(ln2_w, np.float32)
    ln2_b = np.asarray(ln2_b, np.float32)
    fc2_b = np.asarray(fc2_b, np.float32)
    skip1 = bool(np.all(ln1_w == 1.0) and np.all(ln1_b == 0.0))
    skip2 = bool(np.all(ln2_w == 1.0) and np.all(ln2_b == 0.0))
    skipb2 = bool(np.all(fc2_b == 0.0))
    flags = (skip1, skip2, skipb2)
    if flags not in _cache:
        _cache[flags] = _build(flags)
    nc = _cache[flags]

    # Constant softmax shift: SCALE*(sqrt(C)*max|w| + ||b||_2)^2 upper-bounds
    # every score S[n,m] (Cauchy-Schwarz on rows of y = LN(x)*w + b, each of
    # which has ||y_n|| <= sqrt(C)*max|w| + ||b||), so exp never overflows and
    # the shift is row-constant => softmax is exact and E stays symmetric.
    ybound = float(np.sqrt(C) * np.abs(ln1_w).max() + np.linalg.norm(ln1_b))
    expb = np.full((P, 1), -SCALE * ybound * ybound, np.float32)
    prep = {
        "fc1t": np.ascontiguousarray(np.asarray(fc1_w, np.float32).T.astype(bfl)),
        "fc2t": np.ascontiguousarray(np.asarray(fc2_w, np.float32).T.astype(bfl)),
        "fc1b_r": np.ascontiguousarray(
            np.asarray(fc1_b, np.float32).reshape(JB, P).T),
        "expb": expb,
        "identb": np.eye(P, dtype=np.float32).astype(bfl),
    }
    if not skip1:
        prep["ln1w_b"] = np.ascontiguousarray(np.broadcast_to(ln1_w, (P, C)))
        prep["ln1b_b"] = np.ascontiguousarray(np.broadcast_to(ln1_b, (P, C)))
    if not skip2:
        prep["ln2w_b"] = np.ascontiguousarray(np.broadcast_to(ln2_w, (P, C)))
        prep["ln2b_b"] = np.ascontiguousarray(np.broadcast_to(ln2_b, (P, C)))
    if not skipb2:
        prep["fc2b_b"] = np.ascontiguousarray(np.broadcast_to(fc2_b, (P, C)))
    in_maps = [dict(prep, x=np.ascontiguousarray(x[b])) for b in range(B)]

    trace = bool(os.environ.get("BASS_TRACE"))
    if trace:
        _maybe_install_ntff_hook()
    res = run_bass_kernel_spmd(nc, in_maps, list(range(B)), trace=trace)
    _last_results = res
    return np.stack([res.results[b]["out"] for b in range(B)], axis=0)


# revision 31
# speedup vs baseline: 1.6638x; 1.0045x over previous
"""Trainium2 Bass kernel for nn_Block (pre-LN transformer block with dense
self-attention where q=k=v=LN1(x), followed by a GELU MLP).

Sharding: data-parallel over batch B=8 across the 8 NeuronCores (one batch
element per core). Weights are replicated; host-side prep transposes/casts the
MLP weights to bf16 and pre-broadcasts the small LN/bias vectors so the device
kernel does pure compute.

Device algorithm per core (x: [2048, 768] fp32):
  1. LN1 -> y (bf16 natural + transposed via TensorE transpose + ACT copy).
     LN stats via bn_stats/bn_aggr (equal 384-wide chunks); when the LN
     weight/bias inputs are exactly ones/zeros (they are for this problem)
     the scale/shift application is skipped entirely.
  2. E := exp(SCALE*S - c) with a host-computed constant shift
     c = SCALE*(sqrt(C)*max|w1| + ||b1||_2)^2 >= max S (Cauchy-Schwarz on the
     LN-normalized rows), so E is SYMMETRIC and the second attention matmul
     reads E tiles directly as lhsT without transposing the score matrix.
     S = y@y^T via accumulating K=128 matmuls into 512-wide PSUM quarters.
  3. a_unnorm = E @ [y | 1] (ones column makes the softmax denominator Z a
     free extra output column); x2 = x + a_unnorm/Z, streamed to an HBM
     scratch; LN2 fused per block -> zT (TensorE transpose).
  4. hT = gelu(fc1 @ z^T + b1) computed transposed (double-buffered per
     512-column chunk); out = x2 + fc2 @ h^T + b2.
"""

import os
import sys
from contextlib import ExitStack

for _p in ("/opt/trn_rl_repo",):
    if _p not in sys.path:
        sys.path.append(_p)

import numpy as np
import ml_dtypes

import concourse.bass as bass
import concourse.bacc as bacc
import concourse.tile as tile
import concourse.mybir as mybir
from concourse.bass_utils import run_bass_kernel_spmd

f32 = mybir.dt.float32
bf16 = mybir.dt.bfloat16
AF = mybir.ActivationFunctionType
ALU = mybir.AluOpType
AX = mybir.AxisListType

B, N, C, H = 8, 2048, 768, 3072
P = 128
NB = N // P        # 16 row blocks of 128
CCK = C // P       # 6 channel chunks of 128
JB = H // P        # 24 hidden blocks of 128
NQ = 4             # MLP sequence chunks
QW = N // NQ       # 512 columns per MLP chunk
SQ = 4             # S-phase quarters per row block
SW = N // SQ       # 512
YW = C + 4         # y block stride (768 data + ones column + pad)
HEADS = 12
SCALE = 1.0 / float(np.sqrt(C // HEADS))   # 0.125
EPS = 1e-5

_cache = {}


def _ln_normalize(nc, stats, uvscr, xt_ap, w_t, b_t, out_ap, eps_t, skip_wb):
    """out = LN(xt) (*w + b unless skip_wb). out_ap may be bf16."""
    st = stats.tile([P, 12], f32, tag="bn")
    nc.vector.bn_stats(st[:, 0:6], xt_ap[:, 0:384])
    nc.vector.bn_stats(st[:, 6:12], xt_ap[:, 384:768])
    mv = stats.tile([P, 2], f32, tag="mv")
    nc.vector.bn_aggr(mv[:], st[:])
    std = stats.tile([P, 1], f32, tag="std")
    nc.scalar.activation(std[:], mv[:, 1:2], AF.Sqrt, bias=eps_t[:, 0:1])
    rstd = stats.tile([P, 1], f32, tag="rstd")
    nc.vector.reciprocal(rstd[:], std[:])
    negmr = stats.tile([P, 1], f32, tag="negmr")         # -mean*rstd
    nc.vector.tensor_scalar(negmr[:], mv[:, 0:1], rstd[:, 0:1], -1.0,
                            ALU.mult, ALU.mult)
    if skip_wb:
        nc.vector.tensor_scalar(out_ap, xt_ap, rstd[:, 0:1], negmr[:, 0:1],
                                ALU.mult, ALU.add)
    else:
        u = uvscr.tile([P, C], f32, tag="u")
        nc.vector.tensor_scalar(u[:], xt_ap, rstd[:, 0:1], negmr[:, 0:1],
                                ALU.mult, ALU.add)
        v = uvscr.tile([P, C], f32, tag="v")
        nc.vector.scalar_tensor_tensor(v[:], u[:], 1.0, w_t[:],
                                       ALU.mult, ALU.mult)
        nc.vector.scalar_tensor_tensor(out_ap, v[:], 1.0, b_t[:],
                                       ALU.mult, ALU.add)


def _emit(nc, tc, hs, flags):
    skip1, skip2, skipb2 = flags
    ctx = ExitStack()
    with ctx:
        small = ctx.enter_context(tc.tile_pool(name="small", bufs=1))
        general = not (skip1 and skip2)
        stats = ctx.enter_context(tc.tile_pool(name="stats", bufs=8))
        lnscr = ctx.enter_context(
            tc.tile_pool(name="lnscr", bufs=2 if general else 4))
        xio = ctx.enter_context(
            tc.tile_pool(name="xio", bufs=2 if general else 6))
        uvscr = (ctx.enter_context(tc.tile_pool(name="uvscr", bufs=2))
                 if general else None)

        def param(name, shape, tag):
            t = small.tile(shape, f32, tag=tag)
            nc.sync.dma_start(t[:], hs[name].ap())
            return t

        ln1w_t = ln1b_t = ln2w_t = ln2b_t = None
        if not skip1:
            ln1w_t = param("ln1w_b", [P, C], "ln1w")
            ln1b_t = param("ln1b_b", [P, C], "ln1b")
        if not skip2:
            ln2w_t = param("ln2w_b", [P, C], "ln2w")
            ln2b_t = param("ln2b_b", [P, C], "ln2b")
        fc2b_t = None
        if not skipb2:
            fc2b_t = param("fc2b_b", [P, C], "fc2b")
        fc1b_t = param("fc1b_r", [P, JB], "fc1b")
        expb_t = param("expb", [P, 1], "expb")
        if general:
            # Device-computed softmax shift: -SCALE * max_n ||y_n||^2 (the
            # host bound is only tight when ln1 w/b are neutral).
            import concourse.bass_isa as bass_isa
            D_t = small.tile([P, NB], f32, tag="D")
            expbd_t = small.tile([P, 1], f32, tag="expbd")
        identb = small.tile([P, P], bf16, tag="identb")
        nc.sync.dma_start(identb[:], hs["identb"].ap())

        eps_t = small.tile([P, 1], f32, tag="eps")
        nc.vector.memset(eps_t[:], EPS)

        x_ap = hs["x"].ap()
        out_ap = hs["out"].ap()
        x2s = nc.dram_tensor("x2scratch", [N, C], f32)
        x2s_ap = x2s.ap()

        y_pool = tc.alloc_tile_pool(name="ybig", bufs=1)
        y_sb = y_pool.tile([P, NB * YW], bf16, tag="y")
        # ones column at offset C per block (strided memset of pad cols only)
        nc.vector.memset(
            y_sb[:].rearrange("p (i w) -> p i w", w=YW)[:, :, C:YW], 1.0)
        yT_pool = tc.alloc_tile_pool(name="yTbig", bufs=1, side="right")
        yT_sb = yT_pool.tile([P, CCK * N], bf16, tag="yT")

        tp_pool = tc.alloc_tile_pool(name="tpsum", bufs=2, space="PSUM",
                                     side="right")

        # ---- Stage 1: LN1 -> y (bf16) + yT (PE transpose) ----
        for i in range(NB):
            xt = xio.tile([P, C], f32, tag="xio")
            nc.sync.dma_start(xt[:], x_ap[i * P:(i + 1) * P, :])
            ysl = y_sb[:, i * YW: i * YW + C]
            _ln_normalize(nc, stats, uvscr, xt[:], ln1w_t, ln1b_t, ysl,
                          eps_t, skip1)
            if general:
                ysq = lnscr.tile([P, C], bf16, tag="znat")
                nc.scalar.activation(ysq[:], ysl, AF.Square,
                                     accum_out=D_t[:, i:i + 1])
            for c in range(CCK):
                tp = tp_pool.tile([P, P], bf16, tag="tp")
                nc.tensor.transpose(
                    tp[:], y_sb[:, i * YW + c * P: i * YW + (c + 1) * P],
                    identb[:])
                nc.scalar.copy(
                    yT_sb[:, c * N + i * P: c * N + (i + 1) * P], tp[:])

        if general:
            dmax = stats.tile([P, 1], f32, tag="dmax")
            nc.vector.tensor_reduce(dmax[:], D_t[:, 0:NB], AX.X, ALU.max)
            gall = stats.tile([P, 1], f32, tag="gall")
            nc.gpsimd.partition_all_reduce(gall[:], dmax[:], channels=P,
                                           reduce_op=bass_isa.ReduceOp.max)
            nc.vector.tensor_scalar(expbd_t[:], gall[:], -SCALE, None,
                                    ALU.mult)
            expb_t = expbd_t

        # ---- Stage 2: S quarters + Exp -> E (bf16) ----
        # S is symmetric: compute only quarters covering m-blocks >= i
        # (q >= i//4), then mirror the strictly-lower 128x128 tiles via
        # TensorE transpose + DVE copy.
        E_pool = tc.alloc_tile_pool(name="Ebig", bufs=1)
        E_sb = E_pool.tile([P, NB * N], bf16, tag="E")
        with tc.tile_pool(name="spsum", bufs=6, space="PSUM") as sp_pool:
            # Emit quarters in input-availability order: quarter (i, q) needs
            # LN1 tiles <= max(i, 4q+3), so sweep q ascending, i ascending.
            for q in range(SQ):
                for i in range(4 * q + 4) if q < SQ - 1 else range(NB):
                    if q < i // 4:
                        continue
                    # Diagonal quarters: columns left of the diagonal tile are
                    # mirror-filled, so start at the diagonal (narrower MMs,
                    # no WAW with the mirror copies).
                    off = (i - 4 * q) * P if q == i // 4 else 0
                    w = SW - off
                    s_ps = sp_pool.tile([P, SW], f32, tag="s",
                                        name=f"s_{i}_{q}")
                    for c in range(CCK):
                        nc.tensor.matmul(
                            s_ps[:, 0:w],
                            yT_sb[:, c * N + i * P: c * N + (i + 1) * P],
                            yT_sb[:, c * N + q * SW + off:
                                  c * N + (q + 1) * SW],
                            start=(c == 0), stop=(c == CCK - 1))
                    nc.scalar.activation(
                        E_sb[:, i * N + q * SW + off: i * N + (q + 1) * SW],
                        s_ps[:, 0:w], AF.Exp, bias=expb_t[:, 0:1], scale=SCALE)
                    # Mirror lower tiles (r, i) fed by this quarter, split
                    # across ACT and DVE so neither stalls the a-phase.
                    for r in range(max(i + 1, 4 * q), 4 * q + 4):
                        tp = tp_pool.tile([P, P], bf16, tag="tp",
                                          name=f"tp_{r}_{i}")
                        nc.tensor.transpose(
                            tp[:], E_sb[:, i * N + r * P: i * N + (r + 1) * P],
                            identb[:])
                        dst = E_sb[:, r * N + i * P: r * N + (i + 1) * P]
                        if (r + i) % 2 == 0:
                            nc.vector.tensor_copy(dst, tp[:])
                        else:
                            nc.scalar.copy(dst, tp[:])

        # ---- Stage 3 (fused): a|Z = E@[y|1]; x2 = x + a/Z -> HBM; LN2 -> zT
        yT_pool.release()
        zT_pool = tc.alloc_tile_pool(name="zTbig", bufs=1, side="right")
        zT_sb = zT_pool.tile([P, CCK * N], bf16, tag="zT")
        # fc1T on the right stack so its loads overlap the a-phase (the left
        # stack still holds E until the MLP starts).
        w1_pool = tc.alloc_tile_pool(name="w1big", bufs=1, side="right")
        fc1T_sb = w1_pool.tile([P, CCK * H], bf16, tag="fc1T")
        for c in range(CCK):
            nc.sync.dma_start(fc1T_sb[:, c * H:(c + 1) * H],
                              hs["fc1t"].ap()[c * P:(c + 1) * P, :])
        with tc.tile_pool(name="apsum", bufs=3, space="PSUM") as a_pool:
            for i in range(NB):
                a_ps = a_pool.tile([P, 1024], f32, tag="a")
                for j in range(NB):
                    lhsT = E_sb[:, j * N + i * P: j * N + (i + 1) * P]
                    nc.tensor.matmul(a_ps[:, 0:512], lhsT,
                                     y_sb[:, j * YW: j * YW + 512],
                                     start=(j == 0), stop=(j == NB - 1))
                    nc.tensor.matmul(a_ps[:, 512:769], lhsT,
                                     y_sb[:, j * YW + 512: j * YW + C + 1],
                                     start=(j == 0), stop=(j == NB - 1))
                rZ = stats.tile([P, 1], f32, tag="rZ")
                if general:
                    zc = stats.tile([P, 1], f32, tag="zc")
                    nc.vector.tensor_scalar(zc[:], a_ps[:, 768:769], 1e-30,
                                            None, ALU.max)
                    nc.vector.reciprocal(rZ[:], zc[:])
                else:
                    nc.vector.reciprocal(rZ[:], a_ps[:, 768:769])
                xt = xio.tile([P, C], f32, tag="xio")
                nc.sync.dma_start(xt[:], x_ap[i * P:(i + 1) * P, :])
                x2t = lnscr.tile([P, C], f32, tag="x2t")
                nc.vector.scalar_tensor_tensor(
                    x2t[:], a_ps[:, 0:C], rZ[:, 0:1], xt[:],
                    ALU.mult, ALU.add)
                nc.sync.dma_start(x2s_ap[i * P:(i + 1) * P, :], x2t[:])
                znat = lnscr.tile([P, C], bf16, tag="znat")
                _ln_normalize(nc, stats, uvscr, x2t[:], ln2w_t, ln2b_t,
                              znat[:], eps_t, skip2)
                for c in range(CCK):
                    tp = tp_pool.tile([P, P], bf16, tag="tp")
                    nc.tensor.transpose(tp[:], znat[:, c * P:(c + 1) * P],
                                        identb[:])
                    nc.scalar.copy(
                        zT_sb[:, c * N + i * P: c * N + (i + 1) * P], tp[:])

        # ---- Stage 4: MLP ----
        E_pool.release()
        y_pool.release()
        tp_pool.release()
        w_pool = tc.alloc_tile_pool(name="wbig", bufs=1)
        fc2T_sb = w_pool.tile([P, JB * C], bf16, tag="fc2T")
        for j in range(JB):
            nc.sync.dma_start(fc2T_sb[:, j * C:(j + 1) * C],
                              hs["fc2t"].ap()[j * P:(j + 1) * P, :])

        hT_pool = tc.alloc_tile_pool(name="hTbig", bufs=1 if general else 2)
        with tc.tile_pool(name="hpsum", bufs=4, space="PSUM") as h_pool, \
             tc.tile_pool(name="opsum", bufs=2, space="PSUM") as o_pool:
            for q in range(NQ):
                hT_sb = hT_pool.tile([P, JB * QW], bf16, tag="hT")
                for j in range(JB):
                    h_ps = h_pool.tile([P, QW], f32, tag="h")
                    for c in range(CCK):
                        nc.tensor.matmul(
                            h_ps[:],
                            fc1T_sb[:, c * H + j * P: c * H + (j + 1) * P],
                            zT_sb[:, c * N + q * QW: c * N + (q + 1) * QW],
                            start=(c == 0), stop=(c == CCK - 1))
                    nc.scalar.activation(hT_sb[:, j * QW:(j + 1) * QW],
                                         h_ps[:], AF.Gelu,
                                         bias=fc1b_t[:, j:j + 1])
                for t in range(QW // P):
                    i = q * (QW // P) + t
                    o_ps = o_pool.tile([P, 1024], f32, tag="o")
                    for j in range(JB):
                        lhsT = hT_sb[:, j * QW + t * P: j * QW + (t + 1) * P]
                        nc.tensor.matmul(o_ps[:, 0:512], lhsT,
                                         fc2T_sb[:, j * C: j * C + 512],
                                         start=(j == 0), stop=(j == JB - 1))
                        nc.tensor.matmul(o_ps[:, 512:768], lhsT,
                                         fc2T_sb[:, j * C + 512: j * C + C],
                                         start=(j == 0), stop=(j == JB - 1))
                    xre = xio.tile([P, C], f32, tag="xio")
                    nc.sync.dma_start(xre[:], x2s_ap[i * P:(i + 1) * P, :])
                    if skipb2:
                        o2 = lnscr.tile([P, C], f32, tag="o2")
                        nc.vector.scalar_tensor_tensor(
                            o2[:], o_ps[:, 0:C], 1.0, xre[:],
                            ALU.mult, ALU.add)
                    else:
                        o1 = lnscr.tile([P, C], f32, tag="o1")
                        nc.vector.scalar_tensor_tensor(
                            o1[:], o_ps[:, 0:C], 1.0, fc2b_t[:],
                            ALU.mult, ALU.add)
                        o2 = lnscr.tile([P, C], f32, tag="o2")
                        nc.vector.scalar_tensor_tensor(
                            o2[:], o1[:], 1.0, xre[:], ALU.mult, ALU.add)
                    nc.sync.dma_start(out_ap[i * P:(i + 1) * P, :], o2[:])

        hT_pool.release()
        w_pool.release()
        w1_pool.release()
        zT_pool.release()


def _build(flags):
    nc = bacc.Bacc("TRN2", target_bir_lowering=False, debug=False, num_devices=8)
    hs = {}
    skip1, skip2, skipb2 = flags
    hs["x"] = nc.declare_dram_parameter("x", [N, C], f32, isOutput=False)
    if not skip1:
        hs["ln1w_b"] = nc.declare_dram_parameter("ln1w_b", [P, C], f32, isOutput=False)
        hs["ln1b_b"] = nc.declare_dram_parameter("ln1b_b", [P, C], f32, isOutput=False)
    if not skip2:
        hs["ln2w_b"] = nc.declare_dram_parameter("ln2w_b", [P, C], f32, isOutput=False)
        hs["ln2b_b"] = nc.declare_dram_parameter("ln2b_b", [P, C], f32, isOutput=False)
    hs["fc1t"] = nc.declare_dram_parameter("fc1t", [C, H], bf16, isOutput=False)
    hs["fc2t"] = nc.declare_dram_parameter("fc2t", [H, C], bf16, isOutput=False)
    hs["fc1b_r"] = nc.declare_dram_parameter("fc1b_r", [P, JB], f32, isOutput=False)
    if not skipb2:
        hs["fc2b_b"] = nc.declare_dram_parameter("fc2b_b", [P, C], f32, isOutput=False)
    hs["expb"] = nc.declare_dram_parameter("expb", [P, 1], f32, isOutput=False)
    hs["identb"] = nc.declare_dram_parameter("identb", [P, P], bf16, isOutput=False)
    hs["out"] = nc.declare_dram_parameter("out", [N, C], f32, isOutput=True)
    with tile.TileContext(nc) as tc:
        _emit(nc, tc, hs, flags)
    nc.compile()
    return nc


def _maybe_install_ntff_hook():
    """Optional: lets BASS_TRACE=1 capture NTFF profiles under axon."""
    try:
        import types
        if "antenv.axon_hooks" in sys.modules:
            return
        import antenv
        mod = types.ModuleType("antenv.axon_hooks")
        _hook = [None]
        mod.set_axon_ntff_profile_hook = lambda h: _hook.__setitem__(0, h)
        mod.get_axon_ntff_profile_hook = lambda: _hook[0]
        sys.modules["antenv.axon_hooks"] = mod
        antenv.axon_hooks = mod
        from trn_agent_boot.trn_boot import _ntff_profile_via_ctypes
        mod.set_axon_ntff_profile_hook(
            _ntff_profile_via_ctypes("/opt/axon/libaxon_pjrt.so"))
    except Exception:
        pass


_last_results = None


def kernel(x, ln1_w, ln1_b, ln2_w, ln2_b, fc1_w, fc1_b, fc2_w, fc2_b):
    global _last_results
    bfl = ml_dtypes.bfloat16
    x = np.asarray(x, dtype=np.float32)
    ln1_w = np.asarray(ln1_w, np.float32)
    ln1_b = np.asarray(ln1_b, np.float32)
    ln2_w = np.asarray(ln2_w, np.float32)
    ln2_b = np.asarray(ln2_b, np.float32)
    fc2_b = np.asarray(fc2_b, np.float32)
    skip1 = bool(np.all(ln1_w == 1.0) and np.all(ln1_b == 0.0))
    skip2 = bool(np.all(ln2_w == 1.0) and np.all(ln2_b == 0.0))
    skipb2 = bool(np.all(fc2_b == 0.0))
    flags = (skip1, skip2, skipb2)
    if flags not in _cache:
        _cache[flags] = _build(flags)
    nc = _cache[flags]

    # Constant softmax shift: SCALE*(sqrt(C)*max|w| + ||b||_2)^2 upper-bounds
    # every score S[n,m] (Cauchy-Schwarz on rows of y = LN(x)*w + b, each of
    # which has ||y_n|| <= sqrt(C)*max|w| + ||b||), so exp never overflows and
    # the shift is row-constant => softmax is exact and E stays symmetric.
    ybound = float(np.sqrt(C) * np.abs(ln1_w).max() + np.linalg.norm(ln1_b))
    expb = np.full((P, 1), -SCALE * ybound * ybound, np.float32)
    prep = {
        "fc1t": np.ascontiguousarray(np.asarray(fc1_w, np.float32).T.astype(bfl)),
        "fc2t": np.ascontiguousarray(np.asarray(fc2_w, np.float32).T.astype(bfl)),
        "fc1b_r": np.ascontiguousarray(
            np.asarray(fc1_b, np.float32).reshape(JB, P).T),
        "expb": expb,
        "identb": np.eye(P, dtype=np.float32).astype(bfl),
    }
    if not skip1:
        prep["ln1w_b"] = np.ascontiguousarray(np.broadcast_to(ln1_w, (P, C)))
        prep["ln1b_b"] = np.ascontiguousarray(np.broadcast_to(ln1_b, (P, C)))
    if not skip2:
        prep["ln2w_b"] = np.ascontiguousarray(np.broadcast_to(ln2_w, (P, C)))
        prep["ln2b_b"] = np.ascontiguousarray(np.broadcast_to(ln2_b, (P, C)))
    if not skipb2:
        prep["fc2b_b"] = np.ascontiguousarray(np.broadcast_to(fc2_b, (P, C)))
    in_maps = [dict(prep, x=np.ascontiguousarray(x[b])) for b in range(B)]

    trace = bool(os.environ.get("BASS_TRACE"))
    if trace:
        _maybe_install_ntff_hook()
    res = run_bass_kernel_spmd(nc, in_maps, list(range(B)), trace=trace)
    _last_results = res
    return np.stack([res.results[b]["out"] for b in range(B)], axis=0)
